# revision 1
# baseline (speedup 1.0000x reference)
"""Trainium2 Bass kernel for the GRU autoencoder.

Distribution strategy (8 NeuronCores):
  Encode : chain-parallel x batch-parallel. Core j handles GRU chain j//2
           (xf, xb, ef, eb) on batch half j%2 (128 rows), running a uniform
           100-step loop. The 50-step x-chains get 50 leading exact identity
           steps (update-gate pre-activation forced to +BIG => z=1 => h'=h).
           Input-side gates, biases and the pad flag ride the same matmul via
           augmented stationary rows (features + ones-row + flag-row).
  Reshard: AllToAll moves 16-row slices so every core assembles the hidden
           states for its own 32-row decode shard at *fixed* (SPMD-uniform)
           indices: core j decodes global rows [16j:16j+16] u [128+16j:+16].
  Middle : per-shard MLP (em1/em2/eo) + decoder const precompute (bf16).
  Decode : 60 autoregressive steps per core on its 32-row shard (fp32r).

All GRU-path matmuls use fp32r (full PE rate at N>=256, ~1e-4 rounding); the
middle MLP uses bf16 weights. PSUM accumulates in fp32 throughout.

PSUM budget (8 banks): gA 2 | gB 2x1 | gC 2x1 | tr 2x1.
"""

import sys

sys.path.insert(0, "/opt/trn_rl_repo")

import numpy as np

import concourse.bass as bass
import concourse.mybir as mybir
import concourse.tile as tile
from concourse import bacc
from concourse.masks import make_identity

dt = mybir.dt
AF = mybir.ActivationFunctionType
OP = mybir.AluOpType

B, TX, TY, NX, NY, H, HOR = 256, 50, 100, 64, 64, 512, 60
M1, M2 = 1024, 512
G = 3 * H
NCORE = 8
BE = 128   # encoder batch rows per core
BD = 32    # decoder batch rows per core
BIG = 30000.0

F32, F32R, BF16 = dt.float32, dt.float32r, dt.bfloat16


def build_nc(et=100, x_real=50, hor=60):
    nc = bacc.Bacc("TRN2", target_bir_lowering=False, debug=False,
                   num_devices=NCORE)

    # ---- DRAM parameters (identical names on every core; content differs) --
    d_xin = nc.dram_tensor("xin", [66, et * BE], F32, kind="ExternalInput")
    d_wih = nc.dram_tensor("wih_aug", [66, G], F32, kind="ExternalInput")
    d_whh = nc.dram_tensor("whh_t", [H, G], F32, kind="ExternalInput")
    d_bhhn = nc.dram_tensor("bhhn_row", [1, H], F32, kind="ExternalInput")

    d_em1 = nc.dram_tensor("em_w1t", [2 * H, M1], F32, kind="ExternalInput")
    d_em1b = nc.dram_tensor("em_b1row", [1, M1], F32, kind="ExternalInput")
    d_em2 = nc.dram_tensor("em_w2t", [M1, M2], F32, kind="ExternalInput")
    d_em2b = nc.dram_tensor("em_b2row", [1, M2], F32, kind="ExternalInput")
    d_eow = nc.dram_tensor("eo_wt", [M2, H], F32, kind="ExternalInput")
    d_eob = nc.dram_tensor("eo_brow", [1, H], F32, kind="ExternalInput")
    d_dcw = nc.dram_tensor("dc_wt", [2 * H, G], F32, kind="ExternalInput")
    d_dcb = nc.dram_tensor("dc_brow", [1, G], F32, kind="ExternalInput")

    d_dwy = nc.dram_tensor("dwy_t", [NY, G], F32, kind="ExternalInput")
    d_dwhh = nc.dram_tensor("dwhh_t", [H, G], F32, kind="ExternalInput")
    d_dbhhn = nc.dram_tensor("dbhhn_row", [1, H], F32, kind="ExternalInput")
    d_dm1 = nc.dram_tensor("dm_w1t", [H, M1], F32, kind="ExternalInput")
    d_dm1b = nc.dram_tensor("dm_b1row", [1, M1], F32, kind="ExternalInput")
    d_dm2 = nc.dram_tensor("dm_w2t", [M1, M2], F32, kind="ExternalInput")
    d_dm2b = nc.dram_tensor("dm_b2row", [1, M2], F32, kind="ExternalInput")
    d_dow = nc.dram_tensor("do_wt", [M2, NY], F32, kind="ExternalInput")
    d_dob = nc.dram_tensor("do_brow", [1, NY], F32, kind="ExternalInput")
    d_xlast = nc.dram_tensor("xlast_t", [NX, BD], F32, kind="ExternalInput")

    d_out = nc.dram_tensor("out", [BD, hor * NY], F32, kind="ExternalOutput")

    cc_in = nc.dram_tensor("cc_in", [BE, H], F32)
    cc_out = nc.dram_tensor("cc_out", [NCORE, 16, H], F32)

    with tile.TileContext(nc) as tc:
        with tc.tile_pool(name="wenc", bufs=1) as wenc, \
             tc.tile_pool(name="wstream", bufs=2) as ws, \
             tc.tile_pool(name="wdec", bufs=1) as wdec, \
             tc.tile_pool(name="state", bufs=2) as st, \
             tc.tile_pool(name="tmp", bufs=2) as tp, \
             tc.tile_pool(name="tmp1", bufs=1) as tq, \
             tc.tile_pool(name="mid", bufs=1) as md, \
             tc.tile_pool(name="ldtmp", bufs=2) as ld, \
             tc.tile_pool(name="persist", bufs=1) as pe, \
             tc.tile_pool(name="psA", bufs=2, space="PSUM") as psA, \
             tc.tile_pool(name="psB", bufs=1, space="PSUM") as psB, \
             tc.tile_pool(name="psC", bufs=2, space="PSUM") as psC, \
             tc.tile_pool(name="psTR", bufs=1, space="PSUM") as psTR:

            # ---------- constants ----------
            idf = pe.tile([128, 128], F32, tag="idf")
            make_identity(nc, idf[:])
            id32 = idf[0:32, 0:32]
            ones_f = pe.tile([1, 128], F32, tag="ones_f")
            nc.gpsimd.memset(ones_f[:], 1.0)
            ones_r = pe.tile([1, 128], F32R, tag="ones_r")
            nc.vector.tensor_copy(ones_r[:], ones_f[:])
            zero_f = pe.tile([128, 128], F32, tag="zero_f")
            nc.gpsimd.memset(zero_f[:], 0.0)
            id_r = pe.tile([32, 32], F32R, tag="id_r")
            nc.vector.tensor_copy(id_r[:], id32)
            ones_b = pe.tile([1, 128], BF16, tag="ones_b")
            nc.gpsimd.tensor_copy(ones_b[:], ones_f[:])

            def load_r(pool, dram_ap, rows, cols, tag, rdt=F32R, eng=None):
                r = pool.tile([rows, cols], rdt, tag=tag)
                for lo in range(0, cols, 768):
                    hi = min(cols, lo + 768)
                    t = ld.tile([rows, hi - lo], F32, tag="ldtmp")
                    nc.sync.dma_start(t[:], dram_ap[:, lo:hi])
                    (eng or nc.gpsimd).tensor_copy(r[:, lo:hi], t[:])
                return r

            # ---------- encoder weights (needed immediately; DVE rounds) ----
            wih_r = load_r(wenc, d_wih[:], 66, G, "wih", eng=nc.vector)
            whh_r = [load_r(wenc, d_whh[128 * c:128 * (c + 1), :], 128, G,
                            f"whh{c}", eng=nc.vector) for c in range(4)]

            # ---------- encoder state ----------
            hT = pe.tile([128, H], F32R, tag="hT0")
            for c in range(4):
                nc.vector.tensor_copy(hT[:, 128 * c:128 * (c + 1)], zero_f[:])
            h_bh = pe.tile([BE, H], F32, tag="h0")
            nc.gpsimd.memset(h_bh[:], 0.0)

            # ---------- middle/decoder weights (gpsimd rounds in background)
            # column-packed bf16 biases: dcb|em1b|em2b|eob
            bias_b = pe.tile([1, 3584], BF16, tag="bias_b")
            for dtn, base, w in ((d_dcb, 0, G), (d_em1b, G, M1),
                                 (d_em2b, G + M1, M2), (d_eob, 3072, M2)):
                for lo in range(0, w, 768):
                    hi = min(w, lo + 768)
                    t = ld.tile([1, hi - lo], F32, tag="ldtmp")
                    nc.sync.dma_start(t[:], dtn[:, lo:hi])
                    nc.gpsimd.tensor_copy(bias_b[0:1, base + lo:base + hi],
                                          t[:])

            dwyc = wdec.tile([96, G], F32R, tag="dwyc")
            for lo in range(0, G, 768):
                hi = lo + 768
                t = ld.tile([NY, 768], F32, tag="ldtmp")
                nc.sync.dma_start(t[:], d_dwy[:, lo:hi])
                nc.gpsimd.tensor_copy(dwyc[0:NY, lo:hi], t[:])
            # identity block staged at partitions 64:96 (for ypT_aug rows)
            id_hi_f = pe.tile([96, 32], F32, tag="id_hi_f")
            nc.sync.dma_start(id_hi_f[64:96, :], idf[0:32, 0:32])
            dwhh_r = [load_r(wdec, d_dwhh[128 * c:128 * (c + 1), :], 128, G,
                             f"dwhh{c}") for c in range(4)]
            dm1_r = [load_r(wdec, d_dm1[128 * c:128 * (c + 1), :], 128, M1,
                            f"dm1_{c}") for c in range(4)]
            dm2_r = [load_r(wdec, d_dm2[128 * c:128 * (c + 1), :], 128, M2,
                            f"dm2_{c}") for c in range(8)]
            dow_r = [load_r(wdec, d_dow[128 * c:128 * (c + 1), :], 128, NY,
                            f"dow_{c}", BF16) for c in range(4)]
            # column-packed f32r biases: bhhn|dbhhn|dm1b|dm2b|dob
            bias_r = pe.tile([1, 2624], F32R, tag="bias_r")
            for dtn, base, w in ((d_bhhn, 0, H), (d_dbhhn, H, H),
                                 (d_dm1b, 1024, M1), (d_dm2b, 2048, M2),
                                 (d_dob, 2560, NY)):
                for lo in range(0, w, 768):
                    hi = min(w, lo + 768)
                    t = ld.tile([1, hi - lo], F32, tag="ldtmp")
                    nc.sync.dma_start(t[:], dtn[:, lo:hi])
                    nc.gpsimd.tensor_copy(bias_r[0:1, base + lo:base + hi],
                                          t[:])
            xlast_r = load_r(wdec, d_xlast[:], NX, BD, "xlastr")

            # ---------- encode loop ----------
            for t in range(et):
                xs_f = tp.tile([66, 128], F32, tag="xs_f")
                nc.sync.dma_start(xs_f[:], d_xin[:, t * BE:(t + 1) * BE])
                xs = tp.tile([66, 128], F32R, tag="xs_r")
                nc.vector.tensor_copy(xs[:], xs_f[:])

                g1a = psA.tile([BE, 512], F32, tag="gA1")
                g1b = psA.tile([BE, 512], F32, tag="gA2")
                g2 = psB.tile([BE, 512], F32, tag="gB")
                g3 = psC.tile([BE, 512], F32, tag="gC")
                # bank-grouped: finish each PSUM bank before switching
                nc.tensor.matmul(g1a[:], xs[:], wih_r[:, 0:512],
                                 start=True, stop=False)
                for c in range(4):
                    nc.tensor.matmul(g1a[:], hT[:, 128 * c:128 * (c + 1)],
                                     whh_r[c][:, 0:512],
                                     start=False, stop=(c == 3))
                nc.tensor.matmul(g1b[:], xs[:], wih_r[:, 512:1024],
                                 start=True, stop=False)
                for c in range(4):
                    nc.tensor.matmul(g1b[:], hT[:, 128 * c:128 * (c + 1)],
                                     whh_r[c][:, 512:1024],
                                     start=False, stop=(c == 3))
                for c in range(4):
                    nc.tensor.matmul(g2[:], hT[:, 128 * c:128 * (c + 1)],
                                     whh_r[c][:, 1024:1536],
                                     start=(c == 0), stop=False)
                nc.tensor.matmul(g2[:], ones_r[0:1, 0:BE],
                                 bias_r[0:1, 0:512], start=False, stop=True)
                nc.tensor.matmul(g3[:], xs[:], wih_r[:, 1024:1536],
                                 start=True, stop=True)

                r_t = tp.tile([BE, 512], F32, tag="r")
                z_t = tp.tile([BE, 512], F32, tag="z")
                omz = tp.tile([BE, 512], F32, tag="omz")
                nc.scalar.activation(r_t[:], g1a[:], AF.Sigmoid)
                nc.scalar.activation(z_t[:], g1b[:], AF.Sigmoid)
                nc.scalar.activation(omz[:], g1b[:], AF.Sigmoid,
                                     scale=-1.0)
                rhn = tp.tile([BE, 512], F32, tag="rhn")
                nc.vector.tensor_mul(rhn[:], r_t[:], g2[:])
                npre = tp.tile([BE, 512], F32, tag="npre")
                nc.vector.tensor_add(npre[:], rhn[:], g3[:])
                n_t = tp.tile([BE, 512], F32, tag="n")
                nc.scalar.activation(n_t[:], npre[:], AF.Tanh)
                a_t = tp.tile([BE, 512], F32, tag="a")
                nc.vector.tensor_mul(a_t[:], omz[:], n_t[:])
                b_t = tp.tile([BE, 512], F32, tag="b")
                nc.vector.tensor_mul(b_t[:], z_t[:], h_bh[:])
                h_new = st.tile([BE, H], F32, tag="h")
                nc.vector.tensor_add(h_new[:], a_t[:], b_t[:])

                ptr = psTR.tile([128, 512], F32, tag="tr")
                for c in range(4):
                    nc.tensor.transpose(ptr[:, 128 * c:128 * (c + 1)],
                                        h_new[:, 128 * c:128 * (c + 1)],
                                        idf[:])
                hT_new = st.tile([128, H], F32R, tag="hT")
                nc.scalar.copy(hT_new[:], ptr[:])
                hT, h_bh = hT_new, h_new

            # ---------- reshard: AllToAll of 16-row slices ----------
            nc.sync.dma_start(cc_in[:], h_bh[:])
            nc.gpsimd.collective_compute(
                "AllToAll", OP.bypass,
                replica_groups=[list(range(NCORE))],
                ins=[cc_in[:]], outs=[cc_out[:]])

            pxa = md.tile([BD, H], F32, tag="pA")
            pxb = md.tile([BD, H], F32, tag="pB")
            pya = md.tile([BD, H], F32, tag="pA")
            pyb = md.tile([BD, H], F32, tag="pB")
            nc.sync.dma_start(pxa[0:16, :], cc_out[0][:])
            nc.sync.dma_start(pxa[16:32, :], cc_out[1][:])
            nc.sync.dma_start(pxb[0:16, :], cc_out[2][:])
            nc.sync.dma_start(pxb[16:32, :], cc_out[3][:])
            nc.sync.dma_start(pya[0:16, :], cc_out[4][:])
            nc.sync.dma_start(pya[16:32, :], cc_out[5][:])
            nc.sync.dma_start(pyb[0:16, :], cc_out[6][:])
            nc.sync.dma_start(pyb[16:32, :], cc_out[7][:])
            hx = md.tile([BD, H], F32, tag="hx")
            hy = md.tile([BD, H], F32, tag="hy")
            nc.vector.tensor_add(hx[:], pxa[:], pxb[:])
            nc.vector.tensor_add(hy[:], pya[:], pyb[:])

            def trsp_b(src, cols, tag):
                """src [BD, cols] f32 -> bf16 [128, (cols//128)*BD]."""
                nch = cols // 128
                p = psTR.tile([128, nch * BD], F32, tag="tr")
                for c in range(nch):
                    nc.tensor.transpose(p[:, BD * c:BD * (c + 1)],
                                        src[:, 128 * c:128 * (c + 1)], id32)
                o = pe.tile([128, nch * BD], BF16, tag=tag)
                nc.scalar.copy(o[:], p[:])
                return o

            hxT = trsp_b(hx, H, "hxT")
            hyT = trsp_b(hy, H, "hyT")

            m1a = psA.tile([BD, 512], F32, tag="gA1")
            m1b = psA.tile([BD, 512], F32, tag="gA2")
            for c in range(8):
                wt = load_r(ws, d_em1[128 * c:128 * (c + 1), :], 128, M1,
                            f"wstr{c % 2}", BF16)
                s = (hxT if c < 4 else hyT)[:, BD * (c % 4):BD * (c % 4 + 1)]
                nc.tensor.matmul(m1a[:], s, wt[:, 0:512],
                                 start=(c == 0), stop=False)
                nc.tensor.matmul(m1b[:], s, wt[:, 512:1024],
                                 start=(c == 0), stop=False)
            nc.tensor.matmul(m1a[:], ones_b[0:1, 0:BD],
                             bias_b[0:1, 1536:2048], start=False, stop=True)
            nc.tensor.matmul(m1b[:], ones_b[0:1, 0:BD],
                             bias_b[0:1, 2048:2560], start=False, stop=True)
            hm1 = tq.tile([BD, M1], F32, tag="hm1")
            nc.scalar.activation(hm1[:, 0:512], m1a[:], AF.Relu)
            nc.scalar.activation(hm1[:, 512:1024], m1b[:], AF.Relu)
            hm1T = trsp_b(hm1, M1, "hm1T_m")

            m2 = psB.tile([BD, M2], F32, tag="gB")
            for c in range(8):
                wt = load_r(ws, d_em2[128 * c:128 * (c + 1), :], 128, M2,
                            f"wstr{c % 2}", BF16)
                nc.tensor.matmul(m2[:], hm1T[:, BD * c:BD * (c + 1)],
                                 wt[:], start=(c == 0), stop=False)
            nc.tensor.matmul(m2[:], ones_b[0:1, 0:BD], bias_b[0:1, 2560:3072],
                             start=False, stop=True)
            hm2 = tq.tile([BD, M2], F32, tag="hm2")
            nc.scalar.activation(hm2[:], m2[:], AF.Relu)
            hm2T = trsp_b(hm2, M2, "hm2T_m")

            zp = psC.tile([BD, H], F32, tag="gC")
            for c in range(4):
                wt = load_r(ws, d_eow[128 * c:128 * (c + 1), :], 128, H,
                            f"wstr{c % 2}", BF16)
                nc.tensor.matmul(zp[:], hm2T[:, BD * c:BD * (c + 1)],
                                 wt[:], start=(c == 0), stop=False)
            nc.tensor.matmul(zp[:], ones_b[0:1, 0:BD], bias_b[0:1, 3072:3584],
                             start=False, stop=True)
            z_sb = md.tile([BD, H], F32, tag="z_sb")
            nc.scalar.copy(z_sb[:], zp[:])
            zT = trsp_b(z_sb, H, "zT")

            # const = cat(h_x, z) @ d_Wih[:, :2H].T + d_bih + d_bhh(r,z)
            cpa = psA.tile([96, 512], F32, tag="gA1")
            cpa = cpa[64:96, :]
            cpb = psA.tile([96, 512], F32, tag="gA2")
            cpb = cpb[64:96, :]
            cpn = psB.tile([96, 512], F32, tag="gB")
            cpn = cpn[64:96, :]
            for c in range(8):
                wt = load_r(ws, d_dcw[128 * c:128 * (c + 1), :], 128, G,
                            f"wstr{c % 2}", BF16)
                s = (hxT if c < 4 else zT)[:, BD * (c % 4):BD * (c % 4 + 1)]
                nc.tensor.matmul(cpa[:], s, wt[:, 0:512],
                                 start=(c == 0), stop=False)
                nc.tensor.matmul(cpb[:], s, wt[:, 512:1024],
                                 start=(c == 0), stop=False)
                nc.tensor.matmul(cpn[:], s, wt[:, 1024:1536],
                                 start=(c == 0), stop=False)
            nc.tensor.matmul(cpa[:], ones_b[0:1, 0:BD],
                             bias_b[0:1, 0:512], start=False, stop=True)
            nc.tensor.matmul(cpb[:], ones_b[0:1, 0:BD],
                             bias_b[0:1, 512:1024], start=False, stop=True)
            nc.tensor.matmul(cpn[:], ones_b[0:1, 0:BD],
                             bias_b[0:1, 1024:1536], start=False, stop=True)
            nc.vector.tensor_copy(dwyc[64:96, 0:512], cpa[:])
            nc.vector.tensor_copy(dwyc[64:96, 512:1024], cpb[:])
            nc.vector.tensor_copy(dwyc[64:96, 1024:1536], cpn[:])

            # decoder init
            hdT = st.tile([128, 4 * BD], F32R, tag="hdT")
            nc.vector.tensor_copy(hdT[:], zero_f[:])
            hd = st.tile([BD, H], F32, tag="hd")
            nc.gpsimd.memset(hd[:], 0.0)
            ypT = st.tile([96, BD], F32R, tag="ypT")
            nc.vector.tensor_copy(ypT[0:NX, :], xlast_r[:])
            nc.vector.tensor_copy(ypT[64:96, :], id_hi_f[64:96, :])

            # ---------- decode loop ----------
            for t in range(hor):
                g1a = psA.tile([BD, 512], F32, tag="gA1")
                g1b = psA.tile([BD, 512], F32, tag="gA2")
                g2 = psB.tile([BD, 512], F32, tag="gB")
                g3 = psC.tile([BD, 512], F32, tag="gC")
                # h-side first: depends only on hdT (ready since last GRU
                # phase), so these stream during the previous step's MLP.
                # The yp/const matmuls close each group once ypT lands.
                for c in range(4):
                    nc.tensor.matmul(g1a[:], hdT[:, BD * c:BD * (c + 1)],
                                     dwhh_r[c][:, 0:512],
                                     start=(c == 0), stop=False)
                for c in range(4):
                    nc.tensor.matmul(g2[:], hdT[:, BD * c:BD * (c + 1)],
                                     dwhh_r[c][:, 1024:1536],
                                     start=(c == 0), stop=False)
                nc.tensor.matmul(g2[:], ones_r[0:1, 0:BD],
                                 bias_r[0:1, 512:1024], start=False, stop=True)
                for c in range(4):
                    nc.tensor.matmul(g1b[:], hdT[:, BD * c:BD * (c + 1)],
                                     dwhh_r[c][:, 512:1024],
                                     start=(c == 0), stop=False)
                nc.tensor.matmul(g1a[:], ypT[:], dwyc[:, 0:512],
                                 start=False, stop=True)
                nc.tensor.matmul(g3[:], ypT[:], dwyc[:, 1024:1536],
                                 start=True, stop=True)
                nc.tensor.matmul(g1b[:], ypT[:], dwyc[:, 512:1024],
                                 start=False, stop=True)

                r_t = tp.tile([BD, 512], F32, tag="r")
                z_t = tp.tile([BD, 512], F32, tag="z")
                omz = tp.tile([BD, 512], F32, tag="omz")
                nc.scalar.activation(r_t[:], g1a[:], AF.Sigmoid)
                nc.scalar.activation(z_t[:], g1b[:], AF.Sigmoid)
                nc.scalar.activation(omz[:], g1b[:], AF.Sigmoid,
                                     scale=-1.0)
                rhn = tp.tile([BD, 512], F32, tag="rhn")
                nc.vector.tensor_mul(rhn[:], r_t[:], g2[:])
                npre = tp.tile([BD, 512], F32, tag="npre")
                nc.vector.tensor_add(npre[:], rhn[:], g3[:])
                n_t = tp.tile([BD, 512], F32, tag="n")
                nc.scalar.activation(n_t[:], npre[:], AF.Tanh)
                a_t = tp.tile([BD, 512], F32, tag="a")
                nc.vector.tensor_mul(a_t[:], omz[:], n_t[:])
                b_t = tp.tile([BD, 512], F32, tag="b")
                nc.gpsimd.tensor_mul(b_t[:], z_t[:], hd[:])
                hd_new = st.tile([BD, H], F32, tag="hd")
                nc.vector.tensor_add(hd_new[:], a_t[:], b_t[:])

                ptr = psTR.tile([128, 4 * BD], F32, tag="tr")
                for c in range(4):
                    nc.tensor.transpose(ptr[:, BD * c:BD * (c + 1)],
                                        hd_new[:, 128 * c:128 * (c + 1)],
                                        id32)
                hdT_new = st.tile([128, 4 * BD], F32R, tag="hdT")
                nc.scalar.copy(hdT_new[:], ptr[:])
                hdT, hd = hdT_new, hd_new

                m1a = psA.tile([BD, 512], F32, tag="gA1")
                m1b = psA.tile([BD, 512], F32, tag="gA2")
                for c in range(4):
                    nc.tensor.matmul(m1a[:], hdT[:, BD * c:BD * (c + 1)],
                                     dm1_r[c][:, 0:512],
                                     start=(c == 0), stop=False)
                nc.tensor.matmul(m1a[:], ones_r[0:1, 0:BD],
                                 bias_r[0:1, 1024:1536], start=False, stop=True)
                hm1 = tq.tile([BD, M1], F32, tag="hm1")
                nc.scalar.activation(hm1[:, 0:512], m1a[:], AF.Relu)
                for c in range(4):
                    nc.tensor.matmul(m1b[:],
                                     hdT[:, BD * c:BD * (c + 1)],
                                     dm1_r[c][:, 512:1024],
                                     start=(c == 0), stop=False)
                nc.tensor.matmul(m1b[:], ones_r[0:1, 0:BD],
                                 bias_r[0:1, 1536:2048], start=False, stop=True)
                nc.scalar.activation(hm1[:, 512:1024], m1b[:], AF.Relu)
                hm1Ta = tq.tile([128, 4 * BD], F32R, tag="hm1Ta")
                hm1Tb = tq.tile([128, 4 * BD], F32R, tag="hm1Tb")
                p1 = psTR.tile([128, 4 * BD], F32, tag="tr")
                for c in range(4):
                    nc.tensor.transpose(p1[:, BD * c:BD * (c + 1)],
                                        hm1[:, 128 * c:128 * (c + 1)], id32)
                nc.vector.tensor_copy(hm1Ta[:], p1[:])
                p1b = psTR.tile([128, 4 * BD], F32, tag="tr")
                for c in range(4):
                    nc.tensor.transpose(p1b[:, BD * c:BD * (c + 1)],
                                        hm1[:, 512 + 128 * c:640 + 128 * c],
                                        id32)
                nc.vector.tensor_copy(hm1Tb[:], p1b[:])

                m2 = psB.tile([BD, M2], F32, tag="gB")
                for c in range(8):
                    s = (hm1Ta if c < 4 else hm1Tb)[:, BD * (c % 4):
                                                    BD * (c % 4 + 1)]
                    nc.tensor.matmul(m2[:], s, dm2_r[c][:],
                                     start=(c == 0), stop=False)
                nc.tensor.matmul(m2[:], ones_r[0:1, 0:BD],
                                 bias_r[0:1, 2048:2560], start=False, stop=True)
                hm2 = tq.tile([BD, M2], F32, tag="hm2")
                nc.scalar.activation(hm2[:], m2[:], AF.Relu)
                p2 = psTR.tile([128, 4 * BD], F32, tag="tr")
                for c in range(4):
                    nc.tensor.transpose(p2[:, BD * c:BD * (c + 1)],
                                        hm2[:, 128 * c:128 * (c + 1)], id32)
                hm2T = tq.tile([128, 4 * BD], BF16, tag="hm2T")
                nc.vector.tensor_copy(hm2T[:], p2[:])

                yp_ps = psC.tile([BD, NY], F32, tag="gC")
                for c in range(4):
                    nc.tensor.matmul(yp_ps[:], hm2T[:, BD * c:BD * (c + 1)],
                                     dow_r[c][:], start=(c == 0), stop=False)
                nc.tensor.matmul(yp_ps[:], ones_r[0:1, 0:BD],
                                 bias_r[0:1, 2560:2624],
                                 start=False, stop=True)
                y_sb = tp.tile([BD, NY], F32, tag="y_sb")
                nc.scalar.copy(y_sb[:], yp_ps[:])
                nc.sync.dma_start(d_out[:, NY * t:NY * (t + 1)], y_sb[:])
                if t + 1 < hor:
                    p3 = psTR.tile([NX, BD], F32, tag="tr")
                    nc.tensor.transpose(p3[:], y_sb[:], id32)
                    ypT_new = st.tile([96, BD], F32R, tag="ypT")
                    nc.scalar.copy(ypT_new[0:NX, :], p3[:])
                    nc.vector.tensor_copy(ypT_new[64:96, :],
                                          id_hi_f[64:96, :])
                    ypT = ypT_new

    nc.compile()
    return nc


# ---------------------------------------------------------------------------
# Host-side sharding
# ---------------------------------------------------------------------------

def shard_inputs(inp, et=100, x_real=50, hor=60):
    f32 = np.float32
    x, y = np.asarray(inp["x"], f32), np.asarray(inp["y"], f32)
    tx = x.shape[1]
    chains = [("xf", False, x), ("xb", True, x),
              ("ef", False, y), ("eb", True, y)]
    in_maps = []
    shared = {}

    def wih_aug(pre):
        wih = np.asarray(inp[pre + "_Wih"], f32)
        bih = np.asarray(inp[pre + "_bih"], f32)
        bhh = np.asarray(inp[pre + "_bhh"], f32)
        aug = np.zeros((66, G), f32)
        aug[0:64, :] = wih.T
        bias = bih.copy()
        bias[0:2 * H] += bhh[0:2 * H]
        aug[64, :] = bias
        aug[65, H:2 * H] = BIG
        return aug

    d_Wih = np.asarray(inp["d_Wih"], f32)
    d_bih = np.asarray(inp["d_bih"], f32)
    d_bhh = np.asarray(inp["d_bhh"], f32)
    dc_b = d_bih.copy()
    dc_b[0:2 * H] += d_bhh[0:2 * H]

    shared["em_w1t"] = np.ascontiguousarray(np.asarray(inp["em_W1"], f32).T)
    shared["em_b1row"] = np.asarray(inp["em_b1"], f32)[None, :]
    shared["em_w2t"] = np.ascontiguousarray(np.asarray(inp["em_W2"], f32).T)
    shared["em_b2row"] = np.asarray(inp["em_b2"], f32)[None, :]
    shared["eo_wt"] = np.ascontiguousarray(np.asarray(inp["eo_W"], f32).T)
    shared["eo_brow"] = np.asarray(inp["eo_b"], f32)[None, :]
    shared["dc_wt"] = np.ascontiguousarray(d_Wih[:, 0:2 * H].T)
    shared["dc_brow"] = dc_b[None, :]
    shared["dwy_t"] = np.ascontiguousarray(d_Wih[:, 2 * H:].T)
    shared["dwhh_t"] = np.ascontiguousarray(np.asarray(inp["d_Whh"], f32).T)
    shared["dbhhn_row"] = np.ascontiguousarray(d_bhh[None, 2 * H:])
    shared["dm_w1t"] = np.ascontiguousarray(np.asarray(inp["dm_W1"], f32).T)
    shared["dm_b1row"] = np.asarray(inp["dm_b1"], f32)[None, :]
    shared["dm_w2t"] = np.ascontiguousarray(np.asarray(inp["dm_W2"], f32).T)
    shared["dm_b2row"] = np.asarray(inp["dm_b2"], f32)[None, :]
    shared["do_wt"] = np.ascontiguousarray(np.asarray(inp["do_W"], f32).T)
    shared["do_brow"] = np.asarray(inp["do_b"], f32)[None, :]

    for j in range(NCORE):
        chain, half = j // 2, j % 2
        pre, rev, seq = chains[chain]
        T = seq.shape[1]
        s = seq[128 * half:128 * (half + 1)]          # [128, T, 64]
        xin = np.zeros((66, et, BE), f32)
        xin[64, :, :] = 1.0
        pad = et - T
        if pad:
            xin[65, 0:pad, :] = 1.0
        order = np.arange(T)[::-1] if rev else np.arange(T)
        xin[0:64, pad:, :] = s[:, order, :].transpose(2, 1, 0)
        m = dict(shared)
        m["xin"] = np.ascontiguousarray(xin.reshape(66, et * BE))
        m["wih_aug"] = wih_aug(pre)
        m["whh_t"] = np.ascontiguousarray(np.asarray(inp[pre + "_Whh"],
                                                     f32).T)
        m["bhhn_row"] = np.ascontiguousarray(
            np.asarray(inp[pre + "_bhh"], f32)[None, 2 * H:])
        xl = np.concatenate([x[16 * j:16 * j + 16, -1, :],
                             x[128 + 16 * j:128 + 16 * j + 16, -1, :]])
        m["xlast_t"] = np.ascontiguousarray(xl.T)
        in_maps.append(m)
    return in_maps


def unshard(results, hor=60):
    out = np.zeros((B, hor, NY), np.float32)
    for j in range(NCORE):
        o = results[j]["out"].reshape(BD, hor, NY)
        out[16 * j:16 * j + 16] = o[0:16]
        out[128 + 16 * j:128 + 16 * j + 16] = o[16:32]
    return out


_NC = None


def kernel(**inputs):
    global _NC
    from concourse.bass_utils import run_bass_kernel_spmd
    if _NC is None:
        _NC = build_nc()
    in_maps = shard_inputs(inputs)
    res = run_bass_kernel_spmd(_NC, in_maps, core_ids=list(range(NCORE)))
    return unshard(res.results)



# revision 10
# speedup vs baseline: 1.6479x; 1.6479x over previous
"""Trainium2 Bass kernel for the GRU autoencoder (v2, bf16 + transposed decode).

Distribution (8 NeuronCores), unchanged from v1:
  Encode : chain-parallel x batch-parallel. Core j handles GRU chain j//2
           (xf, xb, ef, eb) on batch half j%2 (128 rows), uniform 100-step
           loop; the 50-step x-chains get 50 exact identity steps (z forced
           to 1 via a +BIG flag row). AllToAll reshards 16-row slices so each
           core decodes global rows [16j:16j+16] u [128+16j:+16].

v2 changes (from trace analysis of v1 @2.55ms):
  * All matmuls bf16 (v1's fp32r ran as fp32_mode=HIGH).
  * Encoder: input-side gate matmuls for step t+1 pre-issued during step t's
    eltwise tail; eltwise chain reordered/bf16 to shrink the 6us PE-idle gap
    that kept HAM re-throttling the PE to 1.2 GHz.
  * Decoder: fully transposed (weights-stationary) layout — gates/hidden/
    outputs all [feature, batch]. Kills all 17 per-step PE transposes, uses
    N=32 moving matmuls (measured 38ns issue gaps), per-step biases ride as
    K=1 stationary-row matmuls pre-issued while the previous step finishes.
"""

import sys

sys.path.insert(0, "/opt/trn_rl_repo")

import numpy as np

import concourse.bass as bass
import concourse.mybir as mybir
import concourse.tile as tile
from concourse import bacc
from concourse.masks import make_identity

dt = mybir.dt
AF = mybir.ActivationFunctionType
OP = mybir.AluOpType

B, TX, TY, NX, NY, H, HOR = 256, 50, 100, 64, 64, 512, 60
M1, M2 = 1024, 512
G = 3 * H
NCORE = 8
BE = 128   # encoder batch rows per core
BD = 32    # decoder batch rows per core
BIG = 30000.0

F32, BF16 = dt.float32, dt.bfloat16


def build_nc(et=100, hor=60):
    nc = bacc.Bacc("TRN2", target_bir_lowering=False, debug=False,
                   num_devices=NCORE)

    # ---- DRAM parameters (identical names on every core; content differs) --
    d_xin = nc.dram_tensor("xin", [66, et * BE], F32, kind="ExternalInput")
    d_wih = nc.dram_tensor("wih_aug", [66, G], F32, kind="ExternalInput")
    d_whh = nc.dram_tensor("whh_t", [H, G], F32, kind="ExternalInput")
    d_bhhn = nc.dram_tensor("bhhn_row", [1, H], F32, kind="ExternalInput")

    d_em1 = nc.dram_tensor("em_w1t", [2 * H, M1], F32, kind="ExternalInput")
    d_em1b = nc.dram_tensor("em_b1row", [1, M1], F32, kind="ExternalInput")
    d_em2 = nc.dram_tensor("em_w2t", [M1, M2], F32, kind="ExternalInput")
    d_em2b = nc.dram_tensor("em_b2row", [1, M2], F32, kind="ExternalInput")
    d_eow = nc.dram_tensor("eo_wt", [M2, H], F32, kind="ExternalInput")
    d_eob = nc.dram_tensor("eo_brow", [1, H], F32, kind="ExternalInput")
    d_dcw = nc.dram_tensor("dc_wt", [2 * H, G], F32, kind="ExternalInput")
    d_dcb = nc.dram_tensor("dc_brow", [1, G], F32, kind="ExternalInput")

    d_dwy = nc.dram_tensor("dwy_t", [NY, G], F32, kind="ExternalInput")
    d_dwhh = nc.dram_tensor("dwhh_t", [H, G], F32, kind="ExternalInput")
    d_dbhhn = nc.dram_tensor("dbhhn_row", [1, H], F32, kind="ExternalInput")
    d_dm1 = nc.dram_tensor("dm_w1t", [H, M1], F32, kind="ExternalInput")
    d_dm1b = nc.dram_tensor("dm_b1row", [1, M1], F32, kind="ExternalInput")
    d_dm2 = nc.dram_tensor("dm_w2t", [M1, M2], F32, kind="ExternalInput")
    d_dm2b = nc.dram_tensor("dm_b2row", [1, M2], F32, kind="ExternalInput")
    d_dow = nc.dram_tensor("do_wt", [M2, NY], F32, kind="ExternalInput")
    d_dobc = nc.dram_tensor("do_bcol", [NY, 1], F32, kind="ExternalInput")
    d_xlast = nc.dram_tensor("xlast_t", [NX, BD], F32, kind="ExternalInput")

    d_out = nc.dram_tensor("out", [hor * NY, BD], F32, kind="ExternalOutput")

    cc_in = nc.dram_tensor("cc_in", [BE, H], BF16)
    cc_out = nc.dram_tensor("cc_out", [NCORE, 16, H], BF16)

    with tile.TileContext(nc) as tc:
        with tc.tile_pool(name="pe", bufs=1) as pe, \
             tc.tile_pool(name="wts", bufs=1) as wts, \
             tc.tile_pool(name="ld", bufs=2) as ld, \
             tc.tile_pool(name="xsp", bufs=2) as xsp, \
             tc.tile_pool(name="st", bufs=2) as st, \
             tc.tile_pool(name="tp", bufs=2) as tp, \
             tc.tile_pool(name="md", bufs=1) as md, \
             tc.tile_pool(name="pA", bufs=2, space="PSUM") as pA, \
             tc.tile_pool(name="pB", bufs=2, space="PSUM") as pB, \
             tc.tile_pool(name="pC", bufs=2, space="PSUM") as pC, \
             tc.tile_pool(name="pD", bufs=1, space="PSUM") as pD, \
             tc.tile_pool(name="pTR", bufs=1, space="PSUM") as pTR:

            # ---------- constants ----------
            idf = pe.tile([128, 128], F32, tag="idf")
            make_identity(nc, idf[:])
            idb = pe.tile([128, 128], BF16, tag="idb")
            nc.gpsimd.tensor_copy(idb[:], idf[:])
            ones_b = pe.tile([1, 128], BF16, tag="ones_b")
            nc.gpsimd.memset(ones_b[:], 1.0)
            zero_b = pe.tile([128, 512], BF16, tag="zero_b")
            nc.gpsimd.memset(zero_b[:], 0.0)

            def load_b(pool, dram_ap, rows, cols, tag, rdt=BF16, eng=None):
                r = pool.tile([rows, cols], rdt, tag=tag)
                for lo in range(0, cols, 768):
                    hi = min(cols, lo + 768)
                    t = ld.tile([rows, hi - lo], F32, tag="ldtmp")
                    nc.sync.dma_start(t[:], dram_ap[:, lo:hi])
                    (eng or nc.gpsimd).tensor_copy(r[:, lo:hi], t[:])
                return r

            # ---------- encoder weights (needed immediately; DVE casts) ----
            wih_b = load_b(wts, d_wih[:], 66, G, "wih", eng=nc.vector)
            whh_b = [load_b(wts, d_whh[128 * c:128 * (c + 1), :], 128, G,
                            f"whh{c}", eng=nc.vector) for c in range(4)]
            ebhhn = load_b(wts, d_bhhn[:], 1, H, "ebhhn", eng=nc.vector)

            # ---------- encoder state ----------
            hT = pe.tile([128, H], BF16, tag="hT0")       # [feat%128, 4x128b]
            nc.vector.tensor_copy(hT[:], zero_b[:])
            h_bh = pe.tile([BE, H], BF16, tag="h0")       # [batch, feat]
            nc.gpsimd.memset(h_bh[:], 0.0)

            # ---------- middle + decoder weights (gpsimd casts, background) -
            em1_b = [load_b(wts, d_em1[128 * c:128 * (c + 1), :], 128, M1,
                            f"em1_{c}") for c in range(8)]
            em2_b = [load_b(wts, d_em2[128 * c:128 * (c + 1), :], 128, M2,
                            f"em2_{c}") for c in range(8)]
            eo_b = [load_b(wts, d_eow[128 * c:128 * (c + 1), :], 128, H,
                           f"eo_{c}") for c in range(4)]
            dcw_b = [load_b(wts, d_dcw[128 * c:128 * (c + 1), :], 128, G,
                            f"dcw_{c}") for c in range(8)]
            # column-packed bf16 biases: dcb|em1b|em2b|eob (for middle MLP)
            bias_b = pe.tile([1, 3584], BF16, tag="bias_b")
            for dtn, base, w in ((d_dcb, 0, G), (d_em1b, G, M1),
                                 (d_em2b, G + M1, M2), (d_eob, 3072, M2)):
                for lo in range(0, w, 768):
                    hi = min(w, lo + 768)
                    t = ld.tile([1, hi - lo], F32, tag="ldtmp")
                    nc.sync.dma_start(t[:], dtn[:, lo:hi])
                    nc.gpsimd.tensor_copy(bias_b[0:1, base + lo:base + hi],
                                          t[:])

            dwhh_b = [load_b(wts, d_dwhh[128 * c:128 * (c + 1), :], 128, G,
                             f"dwhh{c}") for c in range(4)]
            dm1_b = [load_b(wts, d_dm1[128 * c:128 * (c + 1), :], 128, M1,
                            f"dm1_{c}") for c in range(4)]
            dm2_b = [load_b(wts, d_dm2[128 * c:128 * (c + 1), :], 128, M2,
                            f"dm2_{c}") for c in range(8)]
            dow_b = [load_b(wts, d_dow[128 * c:128 * (c + 1), :], 128, NY,
                            f"dow_{c}") for c in range(4)]
            dbhhn_s = load_b(wts, d_dbhhn[:], 1, H, "dbhhn")
            dm1b_s = load_b(wts, d_dm1b[:], 1, M1, "dm1b")
            dm2b_s = load_b(wts, d_dm2b[:], 1, M2, "dm2b")
            dob_c = wts.tile([NY, 1], F32, tag="dobc")
            nc.sync.dma_start(dob_c[:], d_dobc[:])
            # ycw: rows 0:64 = Wy^T (d_Wih[:, 2H:]^T), rows 64:96 = const
            # (filled after the middle MLP). Stationary for decode yc matmuls.
            ycw = pe.tile([96, G], BF16, tag="ycw")
            for lo in range(0, G, 768):
                hi = lo + 768
                t = ld.tile([NY, 768], F32, tag="ldtmp")
                nc.sync.dma_start(t[:], d_dwy[:, lo:hi])
                nc.gpsimd.tensor_copy(ycw[0:NY, lo:hi], t[:])
            # ypc: moving operand for yc matmuls. rows 0:64 = y_t (bf16),
            # rows 64:96 = I32 (selects const rows of ycw).
            ypc = pe.tile([96, BD], BF16, tag="ypc")
            xlast_f = wts.tile([NX, BD], F32, tag="xlastf")
            nc.sync.dma_start(xlast_f[:], d_xlast[:])
            nc.gpsimd.tensor_copy(ypc[0:NX, :], xlast_f[:])
            nc.gpsimd.tensor_copy(ypc[64:96, :], idb[0:32, 0:32])

            # =======================================================
            # Encode loop: gates batch-major [128b, 512f], bf16 MMs.
            # xs/bias matmuls for step t+1 pre-issued during step t's
            # eltwise tail; h-side matmuls wait on hT.
            # =======================================================
            def enc_alloc():
                ga = pA.tile([BE, 512], F32, tag="A")
                gb = pB.tile([BE, 512], F32, tag="B")
                gc = pC.tile([BE, 512], F32, tag="C")
                gd = pD.tile([BE, 512], F32, tag="D")
                return ga, gb, gc, gd

            def enc_xs_mms(xs, ga, gb, gc, gd):
                # input-side gates + n-gate h-bias; all independent of h.
                nc.tensor.matmul(ga[:], xs[:], wih_b[:, 0:512],
                                 start=True, stop=False)
                nc.tensor.matmul(gb[:], xs[:], wih_b[:, 512:1024],
                                 start=True, stop=False)
                nc.tensor.matmul(gc[:], xs[:], wih_b[:, 1024:1536],
                                 start=True, stop=True)
                nc.tensor.matmul(gd[:], ones_b[0:1, 0:BE], ebhhn[:],
                                 start=True, stop=False)

            def load_xs(t):
                xf = xsp.tile([66, 128], F32, tag="xs_f")
                nc.sync.dma_start(xf[:], d_xin[:, t * BE:(t + 1) * BE])
                xb = xsp.tile([66, 128], BF16, tag="xs_b")
                nc.gpsimd.tensor_copy(xb[:], xf[:])
                return xb

            xs = load_xs(0)
            ga, gb, gc, gd = enc_alloc()
            enc_xs_mms(xs, ga, gb, gc, gd)

            for t in range(et):
                last = (t == et - 1)
                # h-side matmuls: r-gates first (eltwise chain head), then
                # n-gate h-side, then z-gates.
                for c in range(4):
                    nc.tensor.matmul(ga[:], hT[:, 128 * c:128 * (c + 1)],
                                     whh_b[c][:, 0:512],
                                     start=False, stop=(c == 3))
                for c in range(4):
                    nc.tensor.matmul(gd[:], hT[:, 128 * c:128 * (c + 1)],
                                     whh_b[c][:, 1024:1536],
                                     start=False, stop=(c == 3))
                for c in range(4):
                    nc.tensor.matmul(gb[:], hT[:, 128 * c:128 * (c + 1)],
                                     whh_b[c][:, 512:1024],
                                     start=False, stop=(c == 3))
                if not last:
                    xs_n = load_xs(t + 1)
                    ga_n, gb_n, gc_n, gd_n = enc_alloc()
                    enc_xs_mms(xs_n, ga_n, gb_n, gc_n, gd_n)

                # ---- eltwise: h' = (1-z)*n + z*h, all bf16 ----
                r_t = tp.tile([BE, 512], BF16, tag="r")
                nc.scalar.activation(r_t[:], ga[:], AF.Sigmoid)
                rhn = tp.tile([BE, 512], BF16, tag="rhn")
                nc.vector.tensor_mul(rhn[:], r_t[:], gd[:])
                npre = tp.tile([BE, 512], F32, tag="npre")
                nc.vector.tensor_add(npre[:], rhn[:], gc[:])
                z_t = tp.tile([BE, 512], BF16, tag="z")
                nc.scalar.activation(z_t[:], gb[:], AF.Sigmoid)
                omz = tp.tile([BE, 512], BF16, tag="omz")
                nc.vector.tensor_scalar(omz[:], z_t[:], -1.0, 1.0,
                                        OP.mult, OP.add)
                b_t = tp.tile([BE, 512], BF16, tag="b")
                nc.gpsimd.tensor_mul(b_t[:], z_t[:], h_bh[:])
                n_t = tp.tile([BE, 512], BF16, tag="n")
                nc.scalar.activation(n_t[:], npre[:], AF.Tanh)
                a_t = tp.tile([BE, 512], BF16, tag="a")
                nc.vector.tensor_mul(a_t[:], omz[:], n_t[:])
                h_new = st.tile([BE, H], BF16, tag="h")
                nc.vector.tensor_add(h_new[:], a_t[:], b_t[:])

                if not last:
                    ptr = pTR.tile([128, 512], BF16, tag="TR")
                    for c in range(4):
                        nc.tensor.transpose(ptr[:, 128 * c:128 * (c + 1)],
                                            h_new[:, 128 * c:128 * (c + 1)],
                                            idb[:])
                    hT_new = st.tile([128, H], BF16, tag="hT")
                    nc.vector.tensor_copy(hT_new[:], ptr[:])
                    hT = hT_new
                    ga, gb, gc, gd = ga_n, gb_n, gc_n, gd_n
                h_bh = h_new

            # ---------- reshard: AllToAll of 16-row slices (bf16) ----------
            nc.sync.dma_start(cc_in[:], h_bh[:])
            nc.gpsimd.collective_compute(
                "AllToAll", OP.bypass,
                replica_groups=[list(range(NCORE))],
                ins=[cc_in[:]], outs=[cc_out[:]])

            pxa = md.tile([BD, H], BF16, tag="pA")
            pxb = md.tile([BD, H], BF16, tag="pB")
            pya = md.tile([BD, H], BF16, tag="pA")
            pyb = md.tile([BD, H], BF16, tag="pB")
            nc.sync.dma_start(pxa[0:16, :], cc_out[0][:])
            nc.sync.dma_start(pxa[16:32, :], cc_out[1][:])
            nc.sync.dma_start(pxb[0:16, :], cc_out[2][:])
            nc.sync.dma_start(pxb[16:32, :], cc_out[3][:])
            nc.sync.dma_start(pya[0:16, :], cc_out[4][:])
            nc.sync.dma_start(pya[16:32, :], cc_out[5][:])
            nc.sync.dma_start(pyb[0:16, :], cc_out[6][:])
            nc.sync.dma_start(pyb[16:32, :], cc_out[7][:])
            hx = md.tile([BD, H], F32, tag="hx")
            hy = md.tile([BD, H], F32, tag="hy")
            nc.vector.tensor_add(hx[:], pxa[:], pxb[:])
            nc.vector.tensor_add(hy[:], pya[:], pyb[:])

            def trsp_b(src, cols, tag):
                """src [BD, cols] -> bf16 [128, (cols//128)*BD] via PE."""
                nch = cols // 128
                p = pTR.tile([128, 512], F32, tag="TR")
                for c in range(nch):
                    nc.tensor.transpose(p[:, BD * c:BD * (c + 1)],
                                        src[:, 128 * c:128 * (c + 1)],
                                        idf[0:32, 0:32])
                o = md.tile([128, nch * BD], BF16, tag=tag)
                nc.scalar.copy(o[:], p[:, 0:nch * BD])
                return o

            hxT = trsp_b(hx, H, "hxT")
            hyT = trsp_b(hy, H, "hyT")

            # ---- middle MLP (batch-major, activations stationary) ----
            m1a = pA.tile([BD, 512], F32, tag="A")
            m1b = pB.tile([BD, 512], F32, tag="B")
            for c in range(8):
                wt = em1_b[c]
                s = (hxT if c < 4 else hyT)[:, BD * (c % 4):BD * (c % 4 + 1)]
                nc.tensor.matmul(m1a[:], s, wt[:, 0:512],
                                 start=(c == 0), stop=False)
                nc.tensor.matmul(m1b[:], s, wt[:, 512:1024],
                                 start=(c == 0), stop=False)
            nc.tensor.matmul(m1a[:], ones_b[0:1, 0:BD],
                             bias_b[0:1, 1536:2048], start=False, stop=True)
            nc.tensor.matmul(m1b[:], ones_b[0:1, 0:BD],
                             bias_b[0:1, 2048:2560], start=False, stop=True)
            hm1 = md.tile([BD, M1], F32, tag="hm1")
            nc.scalar.activation(hm1[:, 0:512], m1a[:], AF.Relu)
            nc.scalar.activation(hm1[:, 512:1024], m1b[:], AF.Relu)
            hm1T = trsp_b(hm1, M1, "hm1T_m")

            m2 = pC.tile([BD, M2], F32, tag="C")
            for c in range(8):
                nc.tensor.matmul(m2[:], hm1T[:, BD * c:BD * (c + 1)],
                                 em2_b[c][:], start=(c == 0), stop=False)
            nc.tensor.matmul(m2[:], ones_b[0:1, 0:BD], bias_b[0:1, 2560:3072],
                             start=False, stop=True)
            hm2 = md.tile([BD, M2], F32, tag="hm2")
            nc.scalar.activation(hm2[:], m2[:], AF.Relu)
            hm2T = trsp_b(hm2, M2, "hm2T_m")

            zp = pD.tile([BD, 512], F32, tag="D")
            for c in range(4):
                nc.tensor.matmul(zp[:, 0:H], hm2T[:, BD * c:BD * (c + 1)],
                                 eo_b[c][:], start=(c == 0), stop=False)
            nc.tensor.matmul(zp[:, 0:H], ones_b[0:1, 0:BD],
                             bias_b[0:1, 3072:3584], start=False, stop=True)
            z_sb = md.tile([BD, H], F32, tag="z_sb")
            nc.scalar.copy(z_sb[:], zp[:, 0:H])
            zT = trsp_b(z_sb, H, "zT")

            # const = cat(h_x, z) @ d_Wih[:, :2H].T + d_bih + d_bhh(r,z)
            # lands batch-major [32, 1536] == exactly ycw rows 64:96.
            cpa = pA.tile([BD, 512], F32, tag="A")
            cpb = pB.tile([BD, 512], F32, tag="B")
            cpn = pC.tile([BD, 512], F32, tag="C")
            for c in range(8):
                wt = dcw_b[c]
                s = (hxT if c < 4 else zT)[:, BD * (c % 4):BD * (c % 4 + 1)]
                nc.tensor.matmul(cpa[:], s, wt[:, 0:512],
                                 start=(c == 0), stop=False)
                nc.tensor.matmul(cpb[:], s, wt[:, 512:1024],
                                 start=(c == 0), stop=False)
                nc.tensor.matmul(cpn[:], s, wt[:, 1024:1536],
                                 start=(c == 0), stop=False)
            nc.tensor.matmul(cpa[:], ones_b[0:1, 0:BD],
                             bias_b[0:1, 0:512], start=False, stop=True)
            nc.tensor.matmul(cpb[:], ones_b[0:1, 0:BD],
                             bias_b[0:1, 512:1024], start=False, stop=True)
            nc.tensor.matmul(cpn[:], ones_b[0:1, 0:BD],
                             bias_b[0:1, 1024:1536], start=False, stop=True)
            nc.vector.tensor_copy(ycw[64:96, 0:512], cpa[:])
            nc.vector.tensor_copy(ycw[64:96, 512:1024], cpb[:])
            nc.vector.tensor_copy(ycw[64:96, 1024:1536], cpn[:])

            # =======================================================
            # Decode loop: fully transposed. Gates [1536f, 32b] across
            # three PSUM banks: rz [128, 8*32], hgn [128, 4*32],
            # an [128, 4*32]. h lives as [128, 4*32] (f32 carry + bf16).
            # =======================================================
            hcar = st.tile([128, 4 * BD], F32, tag="hcar")
            nc.gpsimd.memset(hcar[:], 0.0)
            hbf = st.tile([128, 4 * BD], BF16, tag="hbf")
            nc.gpsimd.memset(hbf[:], 0.0)

            def dec_whh(rz, hgn, hbf_src, first):
                # h-side gate matmuls + n-gate hh-bias; pre-issued for the
                # NEXT step (data-ready as soon as hbf lands).
                for j in range(8):
                    for k in range(4):
                        nc.tensor.matmul(
                            rz[:, BD * j:BD * (j + 1)],
                            dwhh_b[k][:, 128 * j:128 * (j + 1)],
                            hbf_src[:, BD * k:BD * (k + 1)],
                            start=(j == 0 and k == 0), stop=False)
                for j in range(4):
                    for k in range(4):
                        nc.tensor.matmul(
                            hgn[:, BD * j:BD * (j + 1)],
                            dwhh_b[k][:, 128 * (8 + j):128 * (9 + j)],
                            hbf_src[:, BD * k:BD * (k + 1)],
                            start=(j == 0 and k == 0), stop=False)
                    nc.tensor.matmul(hgn[:, BD * j:BD * (j + 1)],
                                     dbhhn_s[0:1, 128 * j:128 * (j + 1)],
                                     ones_b[0:1, 0:BD],
                                     start=False, stop=(j == 3))

            def dec_m1bias(m1):
                for j in range(8):
                    nc.tensor.matmul(m1[:, BD * j:BD * (j + 1)],
                                     dm1b_s[0:1, 128 * j:128 * (j + 1)],
                                     ones_b[0:1, 0:BD],
                                     start=(j == 0), stop=False)

            def dec_m2bias(m2d):
                for j in range(4):
                    nc.tensor.matmul(m2d[:, BD * j:BD * (j + 1)],
                                     dm2b_s[0:1, 128 * j:128 * (j + 1)],
                                     ones_b[0:1, 0:BD],
                                     start=(j == 0), stop=False)

            rz = pA.tile([128, 512], F32, tag="A")
            hgn = pB.tile([128, 512], F32, tag="B")
            m1 = pD.tile([128, 512], F32, tag="D")
            m2d = pTR.tile([128, 512], F32, tag="TR")
            dec_m1bias(m1)
            dec_m2bias(m2d)
            # t=0: h=0, so no Whh matmuls; hgn(0) = bias only.
            for j in range(4):
                nc.tensor.matmul(hgn[:, BD * j:BD * (j + 1)],
                                 dbhhn_s[0:1, 128 * j:128 * (j + 1)],
                                 ones_b[0:1, 0:BD],
                                 start=(j == 0), stop=(j == 3))

            for t in range(hor):
                lastd = (t == hor - 1)
                # ---- y/const-side gate matmuls ----
                an = pC.tile([128, 512], F32, tag="C")
                for j in range(8):
                    nc.tensor.matmul(rz[:, BD * j:BD * (j + 1)],
                                     ycw[:, 128 * j:128 * (j + 1)], ypc[:],
                                     start=(t == 0 and j == 0),
                                     stop=(j == 7))
                for j in range(4):
                    nc.tensor.matmul(an[:, BD * j:BD * (j + 1)],
                                     ycw[:, 128 * (8 + j):128 * (9 + j)],
                                     ypc[:], start=(j == 0), stop=(j == 3))

                # ---- GRU eltwise, transposed layout [128, 4*32] ----
                r_t = tp.tile([128, 4 * BD], BF16, tag="dr")
                nc.scalar.activation(r_t[:], rz[:, 0:4 * BD], AF.Sigmoid)
                t2 = tp.tile([128, 4 * BD], F32, tag="dt2")
                nc.vector.tensor_mul(t2[:], r_t[:], hgn[:, 0:4 * BD])
                npre = tp.tile([128, 4 * BD], F32, tag="dnp")
                nc.vector.tensor_add(npre[:], t2[:], an[:, 0:4 * BD])
                z_t = tp.tile([128, 4 * BD], BF16, tag="dz")
                nc.scalar.activation(z_t[:], rz[:, 4 * BD:8 * BD], AF.Sigmoid)
                omz = tp.tile([128, 4 * BD], BF16, tag="domz")
                nc.vector.tensor_scalar(omz[:], z_t[:], -1.0, 1.0,
                                        OP.mult, OP.add)
                u_t = tp.tile([128, 4 * BD], F32, tag="du")
                nc.gpsimd.tensor_mul(u_t[:], z_t[:], hcar[:])
                n_t = tp.tile([128, 4 * BD], BF16, tag="dn")
                nc.scalar.activation(n_t[:], npre[:], AF.Tanh)
                a_t = tp.tile([128, 4 * BD], F32, tag="da")
                nc.vector.tensor_mul(a_t[:], omz[:], n_t[:])
                hcar_n = st.tile([128, 4 * BD], F32, tag="hcar")
                nc.vector.tensor_add(hcar_n[:], a_t[:], u_t[:])
                hbf_n = st.tile([128, 4 * BD], BF16, tag="hbf")
                nc.gpsimd.tensor_copy(hbf_n[:], hcar_n[:])
                hcar, hbf = hcar_n, hbf_n

                # ---- M1 (k-outer so chunks can start as h lands) ----
                for k in range(4):
                    for j in range(8):
                        nc.tensor.matmul(m1[:, BD * j:BD * (j + 1)],
                                         dm1_b[k][:, 128 * j:128 * (j + 1)],
                                         hbf[:, BD * k:BD * (k + 1)],
                                         start=False,
                                         stop=(k == 3 and j == 7))
                hm1_s = tp.tile([128, 8 * BD], BF16, tag="dhm1")
                nc.scalar.activation(hm1_s[:], m1[:, 0:8 * BD], AF.Relu)

                # ---- M2 ----
                for k in range(8):
                    for j in range(4):
                        nc.tensor.matmul(m2d[:, BD * j:BD * (j + 1)],
                                         dm2_b[k][:, 128 * j:128 * (j + 1)],
                                         hm1_s[:, BD * k:BD * (k + 1)],
                                         start=False,
                                         stop=(k == 7 and j == 3))
                hm2_s = tp.tile([128, 4 * BD], BF16, tag="dhm2")
                nc.scalar.activation(hm2_s[:], m2d[:, 0:4 * BD], AF.Relu)

                # bias pre-issue for t+1 (fills the out-matmul wait)
                if not lastd:
                    m1_n = pD.tile([128, 512], F32, tag="D")
                    dec_m1bias(m1_n)
                    m2_n = pTR.tile([128, 512], F32, tag="TR")
                    dec_m2bias(m2_n)

                # ---- output head: y [64, 32] ----
                yb = pC.tile([128, 512], F32, tag="C")
                for k in range(4):
                    nc.tensor.matmul(yb[0:NY, 0:BD],
                                     dow_b[k][:, 0:NY],
                                     hm2_s[:, BD * k:BD * (k + 1)],
                                     start=(k == 0), stop=(k == 3))
                y_f = tp.tile([NY, BD], F32, tag="dy")
                nc.scalar.activation(y_f[:], yb[0:NY, 0:BD], AF.Identity,
                                     bias=dob_c[:])
                nc.sync.dma_start(d_out[NY * t:NY * (t + 1), :], y_f[:])
                if not lastd:
                    nc.vector.tensor_copy(ypc[0:NY, :], y_f[:])
                    # pre-issue next step's h-side matmuls
                    rz_n = pA.tile([128, 512], F32, tag="A")
                    hgn_n = pB.tile([128, 512], F32, tag="B")
                    dec_whh(rz_n, hgn_n, hbf, False)
                    rz, hgn, m1, m2d = rz_n, hgn_n, m1_n, m2_n

    nc.compile()
    return nc


# ---------------------------------------------------------------------------
# Host-side sharding
# ---------------------------------------------------------------------------

def shard_inputs(inp, et=100, hor=60):
    f32 = np.float32
    x, y = np.asarray(inp["x"], f32), np.asarray(inp["y"], f32)
    chains = [("xf", False, x), ("xb", True, x),
              ("ef", False, y), ("eb", True, y)]
    in_maps = []
    shared = {}

    def wih_aug(pre):
        wih = np.asarray(inp[pre + "_Wih"], f32)
        bih = np.asarray(inp[pre + "_bih"], f32)
        bhh = np.asarray(inp[pre + "_bhh"], f32)
        aug = np.zeros((66, G), f32)
        aug[0:64, :] = wih.T
        bias = bih.copy()
        bias[0:2 * H] += bhh[0:2 * H]
        aug[64, :] = bias
        aug[65, H:2 * H] = BIG
        return aug

    d_Wih = np.asarray(inp["d_Wih"], f32)
    d_bih = np.asarray(inp["d_bih"], f32)
    d_bhh = np.asarray(inp["d_bhh"], f32)
    dc_b = d_bih.copy()
    dc_b[0:2 * H] += d_bhh[0:2 * H]

    shared["em_w1t"] = np.ascontiguousarray(np.asarray(inp["em_W1"], f32).T)
    shared["em_b1row"] = np.asarray(inp["em_b1"], f32)[None, :]
    shared["em_w2t"] = np.ascontiguousarray(np.asarray(inp["em_W2"], f32).T)
    shared["em_b2row"] = np.asarray(inp["em_b2"], f32)[None, :]
    shared["eo_wt"] = np.ascontiguousarray(np.asarray(inp["eo_W"], f32).T)
    shared["eo_brow"] = np.asarray(inp["eo_b"], f32)[None, :]
    shared["dc_wt"] = np.ascontiguousarray(d_Wih[:, 0:2 * H].T)
    shared["dc_brow"] = dc_b[None, :]
    shared["dwy_t"] = np.ascontiguousarray(d_Wih[:, 2 * H:].T)
    shared["dwhh_t"] = np.ascontiguousarray(np.asarray(inp["d_Whh"], f32).T)
    shared["dbhhn_row"] = np.ascontiguousarray(d_bhh[None, 2 * H:])
    shared["dm_w1t"] = np.ascontiguousarray(np.asarray(inp["dm_W1"], f32).T)
    shared["dm_b1row"] = np.asarray(inp["dm_b1"], f32)[None, :]
    shared["dm_w2t"] = np.ascontiguousarray(np.asarray(inp["dm_W2"], f32).T)
    shared["dm_b2row"] = np.asarray(inp["dm_b2"], f32)[None, :]
    shared["do_wt"] = np.ascontiguousarray(np.asarray(inp["do_W"], f32).T)
    shared["do_bcol"] = np.ascontiguousarray(
        np.asarray(inp["do_b"], f32)[:, None])

    for j in range(NCORE):
        chain, half = j // 2, j % 2
        pre, rev, seq = chains[chain]
        T = seq.shape[1]
        s = seq[128 * half:128 * (half + 1)]          # [128, T, 64]
        xin = np.zeros((66, et, BE), f32)
        xin[64, :, :] = 1.0
        pad = et - T
        if pad:
            xin[65, 0:pad, :] = 1.0
        order = np.arange(T)[::-1] if rev else np.arange(T)
        xin[0:64, pad:, :] = s[:, order, :].transpose(2, 1, 0)
        m = dict(shared)
        m["xin"] = np.ascontiguousarray(xin.reshape(66, et * BE))
        m["wih_aug"] = wih_aug(pre)
        m["whh_t"] = np.ascontiguousarray(np.asarray(inp[pre + "_Whh"],
                                                     f32).T)
        m["bhhn_row"] = np.ascontiguousarray(
            np.asarray(inp[pre + "_bhh"], f32)[None, 2 * H:])
        xl = np.concatenate([x[16 * j:16 * j + 16, -1, :],
                             x[128 + 16 * j:128 + 16 * j + 16, -1, :]])
        m["xlast_t"] = np.ascontiguousarray(xl.T)
        in_maps.append(m)
    return in_maps


def unshard(results, hor=60):
    out = np.zeros((B, hor, NY), np.float32)
    for j in range(NCORE):
        o = results[j]["out"].reshape(hor, NY, BD).transpose(2, 0, 1)
        out[16 * j:16 * j + 16] = o[0:16]
        out[128 + 16 * j:128 + 16 * j + 16] = o[16:32]
    return out


_NC = None


def kernel(**inputs):
    global _NC
    from concourse.bass_utils import run_bass_kernel_spmd
    if _NC is None:
        _NC = build_nc()
    in_maps = shard_inputs(inputs)
    res = run_bass_kernel_spmd(_NC, in_maps, core_ids=list(range(NCORE)))
    return unshard(res.results)


# revision 16
# speedup vs baseline: 1.7602x; 1.0682x over previous
"""Trainium2 Bass kernel for the GRU autoencoder (v2, bf16 + transposed decode).

Distribution (8 NeuronCores), unchanged from v1:
  Encode : chain-parallel x batch-parallel. Core j handles GRU chain j//2
           (xf, xb, ef, eb) on batch half j%2 (128 rows), uniform 100-step
           loop; the 50-step x-chains get 50 exact identity steps (z forced
           to 1 via a +BIG flag row). AllToAll reshards 16-row slices so each
           core decodes global rows [16j:16j+16] u [128+16j:+16].

v2 changes (from trace analysis of v1 @2.55ms):
  * All matmuls bf16 (v1's fp32r ran as fp32_mode=HIGH).
  * Encoder: input-side gate matmuls for step t+1 pre-issued during step t's
    eltwise tail; eltwise chain reordered/bf16 to shrink the 6us PE-idle gap
    that kept HAM re-throttling the PE to 1.2 GHz.
  * Decoder: fully transposed (weights-stationary) layout — gates/hidden/
    outputs all [feature, batch]. Kills all 17 per-step PE transposes, uses
    N=32 moving matmuls (measured 38ns issue gaps), per-step biases ride as
    K=1 stationary-row matmuls pre-issued while the previous step finishes.
"""

import sys

sys.path.insert(0, "/opt/trn_rl_repo")

import numpy as np

import concourse.bass as bass
import concourse.mybir as mybir
import concourse.tile as tile
from concourse import bacc
from concourse.masks import make_identity

dt = mybir.dt
AF = mybir.ActivationFunctionType
OP = mybir.AluOpType

B, TX, TY, NX, NY, H, HOR = 256, 50, 100, 64, 64, 512, 60
M1, M2 = 1024, 512
G = 3 * H
NCORE = 8
BE = 128   # encoder batch rows per core
BD = 32    # decoder batch rows per core
BIG = 30000.0

F32, BF16 = dt.float32, dt.bfloat16


def build_nc(et=100, hor=60):
    nc = bacc.Bacc("TRN2", target_bir_lowering=False, debug=False,
                   num_devices=NCORE)

    # ---- DRAM parameters (identical names on every core; content differs) --
    d_xin = nc.dram_tensor("xin", [66, et * BE], F32, kind="ExternalInput")
    d_wih = nc.dram_tensor("wih_aug", [66, G], F32, kind="ExternalInput")
    d_whh = nc.dram_tensor("whh_t", [H, G], F32, kind="ExternalInput")
    d_bhhn = nc.dram_tensor("bhhn_row", [1, H], F32, kind="ExternalInput")

    d_em1 = nc.dram_tensor("em_w1t", [2 * H, M1], F32, kind="ExternalInput")
    d_em1b = nc.dram_tensor("em_b1row", [1, M1], F32, kind="ExternalInput")
    d_em2 = nc.dram_tensor("em_w2t", [M1, M2], F32, kind="ExternalInput")
    d_em2b = nc.dram_tensor("em_b2row", [1, M2], F32, kind="ExternalInput")
    d_eow = nc.dram_tensor("eo_wt", [M2, H], F32, kind="ExternalInput")
    d_eob = nc.dram_tensor("eo_brow", [1, H], F32, kind="ExternalInput")
    d_dcw = nc.dram_tensor("dc_wt", [2 * H, G], F32, kind="ExternalInput")
    d_dcb = nc.dram_tensor("dc_brow", [1, G], F32, kind="ExternalInput")

    d_dwy = nc.dram_tensor("dwy_t", [NY, G], F32, kind="ExternalInput")
    d_dwhh = nc.dram_tensor("dwhh_t", [H, G], F32, kind="ExternalInput")
    d_dbhhn = nc.dram_tensor("dbhhn_row", [1, H], F32, kind="ExternalInput")
    d_dm1 = nc.dram_tensor("dm_w1t", [H, M1], F32, kind="ExternalInput")
    d_dm1b = nc.dram_tensor("dm_b1row", [1, M1], F32, kind="ExternalInput")
    d_dm2 = nc.dram_tensor("dm_w2t", [M1, M2], F32, kind="ExternalInput")
    d_dm2b = nc.dram_tensor("dm_b2row", [1, M2], F32, kind="ExternalInput")
    d_dow = nc.dram_tensor("do_wt", [M2, NY], F32, kind="ExternalInput")
    d_dobc = nc.dram_tensor("do_bcol", [NY, 1], F32, kind="ExternalInput")
    d_xlast = nc.dram_tensor("xlast_t", [NX, BD], F32, kind="ExternalInput")

    d_out = nc.dram_tensor("out", [hor * NY, BD], F32, kind="ExternalOutput")

    cc_in = nc.dram_tensor("cc_in", [BE, H], BF16)
    cc_out = nc.dram_tensor("cc_out", [NCORE, 16, H], BF16)

    with tile.TileContext(nc) as tc:
        with tc.tile_pool(name="pe", bufs=1) as pe, \
             tc.tile_pool(name="wts", bufs=1) as wts, \
             tc.tile_pool(name="ld", bufs=2) as ld, \
             tc.tile_pool(name="xsp", bufs=2) as xsp, \
             tc.tile_pool(name="st", bufs=2) as st, \
             tc.tile_pool(name="tp", bufs=2) as tp, \
             tc.tile_pool(name="md", bufs=1) as md, \
             tc.tile_pool(name="pA", bufs=2, space="PSUM") as pA, \
             tc.tile_pool(name="pB", bufs=2, space="PSUM") as pB, \
             tc.tile_pool(name="pC", bufs=2, space="PSUM") as pC, \
             tc.tile_pool(name="pD", bufs=1, space="PSUM") as pD, \
             tc.tile_pool(name="pTR", bufs=1, space="PSUM") as pTR:

            # ---------- constants ----------
            idf = pe.tile([128, 128], F32, tag="idf")
            make_identity(nc, idf[:])
            idb = pe.tile([128, 128], BF16, tag="idb")
            nc.gpsimd.tensor_copy(idb[:], idf[:])
            ones_b = pe.tile([1, 128], BF16, tag="ones_b")
            nc.gpsimd.memset(ones_b[:], 1.0)
            zero_b = pe.tile([128, 512], BF16, tag="zero_b")
            nc.gpsimd.memset(zero_b[:], 0.0)

            def load_b(pool, dram_ap, rows, cols, tag, rdt=BF16, eng=None):
                r = pool.tile([rows, cols], rdt, tag=tag)
                for lo in range(0, cols, 768):
                    hi = min(cols, lo + 768)
                    t = ld.tile([rows, hi - lo], F32, tag="ldtmp")
                    nc.sync.dma_start(t[:], dram_ap[:, lo:hi])
                    (eng or nc.gpsimd).tensor_copy(r[:, lo:hi], t[:])
                return r

            # ---------- encoder weights (needed immediately; DVE casts) ----
            wih_b = load_b(wts, d_wih[:], 66, G, "wih", eng=nc.vector)
            whh_b = [load_b(wts, d_whh[128 * c:128 * (c + 1), :], 128, G,
                            f"whh{c}", eng=nc.vector) for c in range(4)]
            ebhhn = load_b(wts, d_bhhn[:], 1, H, "ebhhn", eng=nc.vector)

            # ---------- encoder state ----------
            hT = pe.tile([128, H], BF16, tag="hT0")       # [feat%128, 4x128b]
            nc.vector.tensor_copy(hT[:], zero_b[:])
            h_bh = pe.tile([BE, H], BF16, tag="h0")       # [batch, feat]
            nc.gpsimd.memset(h_bh[:], 0.0)

            # ---------- middle + decoder weights (gpsimd casts, background) -
            em1_b = [load_b(wts, d_em1[128 * c:128 * (c + 1), :], 128, M1,
                            f"em1_{c}") for c in range(8)]
            em2_b = [load_b(wts, d_em2[128 * c:128 * (c + 1), :], 128, M2,
                            f"em2_{c}") for c in range(8)]
            eo_b = [load_b(wts, d_eow[128 * c:128 * (c + 1), :], 128, H,
                           f"eo_{c}") for c in range(4)]
            dcw_b = [load_b(wts, d_dcw[128 * c:128 * (c + 1), :], 128, G,
                            f"dcw_{c}") for c in range(8)]
            # column-packed bf16 biases: dcb|em1b|em2b|eob (for middle MLP)
            bias_b = pe.tile([1, 3584], BF16, tag="bias_b")
            for dtn, base, w in ((d_dcb, 0, G), (d_em1b, G, M1),
                                 (d_em2b, G + M1, M2), (d_eob, 3072, M2)):
                for lo in range(0, w, 768):
                    hi = min(w, lo + 768)
                    t = ld.tile([1, hi - lo], F32, tag="ldtmp")
                    nc.sync.dma_start(t[:], dtn[:, lo:hi])
                    nc.gpsimd.tensor_copy(bias_b[0:1, base + lo:base + hi],
                                          t[:])

            dwhh_b = [load_b(wts, d_dwhh[128 * c:128 * (c + 1), :], 128, G,
                             f"dwhh{c}") for c in range(4)]
            dm1_b = [load_b(wts, d_dm1[128 * c:128 * (c + 1), :], 128, M1,
                            f"dm1_{c}") for c in range(4)]
            dm2_b = [load_b(wts, d_dm2[128 * c:128 * (c + 1), :], 128, M2,
                            f"dm2_{c}") for c in range(8)]
            dow_b = [load_b(wts, d_dow[128 * c:128 * (c + 1), :], 128, NY,
                            f"dow_{c}") for c in range(4)]
            dbhhn_s = load_b(wts, d_dbhhn[:], 1, H, "dbhhn")
            dm1b_s = load_b(wts, d_dm1b[:], 1, M1, "dm1b")
            dm2b_s = load_b(wts, d_dm2b[:], 1, M2, "dm2b")
            dob_c = wts.tile([NY, 1], F32, tag="dobc")
            nc.sync.dma_start(dob_c[:], d_dobc[:])
            # ycw: rows 0:64 = Wy^T (d_Wih[:, 2H:]^T), rows 64:96 = const
            # (filled after the middle MLP). Stationary for decode yc matmuls.
            ycw = pe.tile([96, G], BF16, tag="ycw")
            for lo in range(0, G, 768):
                hi = lo + 768
                t = ld.tile([NY, 768], F32, tag="ldtmp")
                nc.sync.dma_start(t[:], d_dwy[:, lo:hi])
                nc.gpsimd.tensor_copy(ycw[0:NY, lo:hi], t[:])
            # ypc: moving operand for yc matmuls. rows 0:64 = y_t (bf16),
            # rows 64:96 = I32 (selects const rows of ycw).
            ypc = pe.tile([96, BD], BF16, tag="ypc")
            xlast_f = wts.tile([NX, BD], F32, tag="xlastf")
            nc.sync.dma_start(xlast_f[:], d_xlast[:])
            nc.gpsimd.tensor_copy(ypc[0:NX, :], xlast_f[:])
            nc.gpsimd.tensor_copy(ypc[64:96, :], idb[0:32, 0:32])

            # =======================================================
            # Encode loop: gates batch-major [128b, 512f], bf16 MMs.
            # xs/bias matmuls for step t+1 pre-issued during step t's
            # eltwise tail; h-side matmuls wait on hT.
            # =======================================================
            def enc_alloc():
                ga = pA.tile([BE, 512], F32, tag="A")
                gb = pB.tile([BE, 512], F32, tag="B")
                gc = pC.tile([BE, 512], F32, tag="C")
                gd = pD.tile([BE, 512], F32, tag="D")
                return ga, gb, gc, gd

            def enc_xs_mms(xs, ga, gb, gc, gd):
                # input-side gates + n-gate h-bias; all independent of h.
                nc.tensor.matmul(ga[:], xs[:], wih_b[:, 0:512],
                                 start=True, stop=False)
                nc.tensor.matmul(gb[:], xs[:], wih_b[:, 512:1024],
                                 start=True, stop=False)
                nc.tensor.matmul(gc[:], xs[:], wih_b[:, 1024:1536],
                                 start=True, stop=True)
                nc.tensor.matmul(gd[:], ones_b[0:1, 0:BE], ebhhn[:],
                                 start=True, stop=False)

            def load_xs(t):
                xf = xsp.tile([66, 128], F32, tag="xs_f")
                nc.sync.dma_start(xf[:], d_xin[:, t * BE:(t + 1) * BE])
                xb = xsp.tile([66, 128], BF16, tag="xs_b")
                nc.gpsimd.tensor_copy(xb[:], xf[:])
                return xb

            xs = load_xs(0)
            ga, gb, gc, gd = enc_alloc()
            enc_xs_mms(xs, ga, gb, gc, gd)

            for t in range(et):
                last = (t == et - 1)
                # h-side matmuls: r-gates first (eltwise chain head), then
                # n-gate h-side, then z-gates.
                for c in range(4):
                    nc.tensor.matmul(ga[:], hT[:, 128 * c:128 * (c + 1)],
                                     whh_b[c][:, 0:512],
                                     start=False, stop=(c == 3))
                for c in range(4):
                    nc.tensor.matmul(gd[:], hT[:, 128 * c:128 * (c + 1)],
                                     whh_b[c][:, 1024:1536],
                                     start=False, stop=(c == 3))
                for c in range(4):
                    nc.tensor.matmul(gb[:], hT[:, 128 * c:128 * (c + 1)],
                                     whh_b[c][:, 512:1024],
                                     start=False, stop=(c == 3))
                if not last:
                    xs_n = load_xs(t + 1)
                    ga_n, gb_n, gc_n, gd_n = enc_alloc()
                    enc_xs_mms(xs_n, ga_n, gb_n, gc_n, gd_n)

                # ---- eltwise: h' = (1-z)*n + z*h, all bf16; the a/h'/
                # transpose/copy tail runs split in halves so chunk 0/1 of
                # hT lands (and next-step matmuls start) while half 1 of
                # the tail still computes.
                r_t = tp.tile([BE, 512], BF16, tag="r")
                nc.scalar.activation(r_t[:], ga[:], AF.Sigmoid)
                rhn = tp.tile([BE, 512], BF16, tag="rhn")
                nc.vector.tensor_mul(rhn[:], r_t[:], gd[:])
                npre = tp.tile([BE, 512], BF16, tag="npre")
                nc.vector.tensor_add(npre[:], rhn[:], gc[:])
                z_t = tp.tile([BE, 512], BF16, tag="z")
                nc.scalar.activation(z_t[:], gb[:], AF.Sigmoid)
                omz = tp.tile([BE, 512], BF16, tag="omz")
                nc.vector.tensor_scalar(omz[:], z_t[:], -1.0, 1.0,
                                        OP.mult, OP.add)
                b_t = tp.tile([BE, 512], BF16, tag="b")
                nc.gpsimd.tensor_mul(b_t[:, 0:256], z_t[:, 0:256],
                                     h_bh[:, 0:256])
                nc.gpsimd.tensor_mul(b_t[:, 256:512], z_t[:, 256:512],
                                     h_bh[:, 256:512])
                n_t = tp.tile([BE, 512], BF16, tag="n")
                nc.scalar.activation(n_t[:], npre[:], AF.Tanh)
                a_t = tp.tile([BE, 512], BF16, tag="a")
                h_new = st.tile([BE, H], BF16, tag="h")
                ptr = pTR.tile([128, 512], BF16, tag="TR")
                hT_new = st.tile([128, H], BF16, tag="hT")
                for s in range(2):
                    lo, hi = 256 * s, 256 * (s + 1)
                    nc.vector.tensor_mul(a_t[:, lo:hi], omz[:, lo:hi],
                                         n_t[:, lo:hi])
                    nc.vector.tensor_add(h_new[:, lo:hi], a_t[:, lo:hi],
                                         b_t[:, lo:hi])
                    if not last:
                        for c in (2 * s, 2 * s + 1):
                            nc.tensor.transpose(
                                ptr[:, 128 * c:128 * (c + 1)],
                                h_new[:, 128 * c:128 * (c + 1)], idb[:])
                        if s == 0:
                            nc.scalar.copy(hT_new[:, lo:hi], ptr[:, lo:hi])
                        else:
                            nc.vector.tensor_copy(hT_new[:, lo:hi],
                                                  ptr[:, lo:hi])
                if not last:
                    hT = hT_new
                    ga, gb, gc, gd = ga_n, gb_n, gc_n, gd_n
                h_bh = h_new

            # ---------- reshard: AllToAll of 16-row slices (bf16) ----------
            nc.sync.dma_start(cc_in[:], h_bh[:])
            nc.gpsimd.collective_compute(
                "AllToAll", OP.bypass,
                replica_groups=[list(range(NCORE))],
                ins=[cc_in[:]], outs=[cc_out[:]])

            pxa = md.tile([BD, H], BF16, tag="pA")
            pxb = md.tile([BD, H], BF16, tag="pB")
            pya = md.tile([BD, H], BF16, tag="pA")
            pyb = md.tile([BD, H], BF16, tag="pB")
            nc.sync.dma_start(pxa[0:16, :], cc_out[0][:])
            nc.sync.dma_start(pxa[16:32, :], cc_out[1][:])
            nc.sync.dma_start(pxb[0:16, :], cc_out[2][:])
            nc.sync.dma_start(pxb[16:32, :], cc_out[3][:])
            nc.sync.dma_start(pya[0:16, :], cc_out[4][:])
            nc.sync.dma_start(pya[16:32, :], cc_out[5][:])
            nc.sync.dma_start(pyb[0:16, :], cc_out[6][:])
            nc.sync.dma_start(pyb[16:32, :], cc_out[7][:])
            hx = md.tile([BD, H], F32, tag="hx")
            hy = md.tile([BD, H], F32, tag="hy")
            nc.vector.tensor_add(hx[:], pxa[:], pxb[:])
            nc.vector.tensor_add(hy[:], pya[:], pyb[:])

            def trsp_b(src, cols, tag):
                """src [BD, cols] -> bf16 [128, (cols//128)*BD] via PE."""
                nch = cols // 128
                p = pTR.tile([128, 512], F32, tag="TR")
                for c in range(nch):
                    nc.tensor.transpose(p[:, BD * c:BD * (c + 1)],
                                        src[:, 128 * c:128 * (c + 1)],
                                        idf[0:32, 0:32])
                o = md.tile([128, nch * BD], BF16, tag=tag)
                nc.scalar.copy(o[:], p[:, 0:nch * BD])
                return o

            hxT = trsp_b(hx, H, "hxT")
            hyT = trsp_b(hy, H, "hyT")

            # ---- middle MLP (batch-major, activations stationary) ----
            m1a = pA.tile([BD, 512], F32, tag="A")
            m1b = pB.tile([BD, 512], F32, tag="B")
            for c in range(8):
                wt = em1_b[c]
                s = (hxT if c < 4 else hyT)[:, BD * (c % 4):BD * (c % 4 + 1)]
                nc.tensor.matmul(m1a[:], s, wt[:, 0:512],
                                 start=(c == 0), stop=False)
                nc.tensor.matmul(m1b[:], s, wt[:, 512:1024],
                                 start=(c == 0), stop=False)
            nc.tensor.matmul(m1a[:], ones_b[0:1, 0:BD],
                             bias_b[0:1, 1536:2048], start=False, stop=True)
            nc.tensor.matmul(m1b[:], ones_b[0:1, 0:BD],
                             bias_b[0:1, 2048:2560], start=False, stop=True)
            hm1 = md.tile([BD, M1], F32, tag="hm1")
            nc.scalar.activation(hm1[:, 0:512], m1a[:], AF.Relu)
            nc.scalar.activation(hm1[:, 512:1024], m1b[:], AF.Relu)
            hm1T = trsp_b(hm1, M1, "hm1T_m")

            m2 = pC.tile([BD, M2], F32, tag="C")
            for c in range(8):
                nc.tensor.matmul(m2[:], hm1T[:, BD * c:BD * (c + 1)],
                                 em2_b[c][:], start=(c == 0), stop=False)
            nc.tensor.matmul(m2[:], ones_b[0:1, 0:BD], bias_b[0:1, 2560:3072],
                             start=False, stop=True)
            hm2 = md.tile([BD, M2], F32, tag="hm2")
            nc.scalar.activation(hm2[:], m2[:], AF.Relu)
            hm2T = trsp_b(hm2, M2, "hm2T_m")

            zp = pD.tile([BD, 512], F32, tag="D")
            for c in range(4):
                nc.tensor.matmul(zp[:, 0:H], hm2T[:, BD * c:BD * (c + 1)],
                                 eo_b[c][:], start=(c == 0), stop=False)
            nc.tensor.matmul(zp[:, 0:H], ones_b[0:1, 0:BD],
                             bias_b[0:1, 3072:3584], start=False, stop=True)
            z_sb = md.tile([BD, H], F32, tag="z_sb")
            nc.scalar.copy(z_sb[:], zp[:, 0:H])
            zT = trsp_b(z_sb, H, "zT")

            # const = cat(h_x, z) @ d_Wih[:, :2H].T + d_bih + d_bhh(r,z)
            # lands batch-major [32, 1536] == exactly ycw rows 64:96.
            cpa = pA.tile([BD, 512], F32, tag="A")
            cpb = pB.tile([BD, 512], F32, tag="B")
            cpn = pC.tile([BD, 512], F32, tag="C")
            for c in range(8):
                wt = dcw_b[c]
                s = (hxT if c < 4 else zT)[:, BD * (c % 4):BD * (c % 4 + 1)]
                nc.tensor.matmul(cpa[:], s, wt[:, 0:512],
                                 start=(c == 0), stop=False)
                nc.tensor.matmul(cpb[:], s, wt[:, 512:1024],
                                 start=(c == 0), stop=False)
                nc.tensor.matmul(cpn[:], s, wt[:, 1024:1536],
                                 start=(c == 0), stop=False)
            nc.tensor.matmul(cpa[:], ones_b[0:1, 0:BD],
                             bias_b[0:1, 0:512], start=False, stop=True)
            nc.tensor.matmul(cpb[:], ones_b[0:1, 0:BD],
                             bias_b[0:1, 512:1024], start=False, stop=True)
            nc.tensor.matmul(cpn[:], ones_b[0:1, 0:BD],
                             bias_b[0:1, 1024:1536], start=False, stop=True)
            nc.vector.tensor_copy(ycw[64:96, 0:512], cpa[:])
            nc.vector.tensor_copy(ycw[64:96, 512:1024], cpb[:])
            nc.vector.tensor_copy(ycw[64:96, 1024:1536], cpn[:])

            # =======================================================
            # Decode loop: fully transposed. Gates [1536f, 32b] across
            # three PSUM banks: rz [128, 8*32], hgn [128, 4*32],
            # an [128, 4*32]. h lives as [128, 4*32] (f32 carry + bf16).
            # =======================================================
            hbf = st.tile([128, 4 * BD], BF16, tag="hbf")
            nc.gpsimd.memset(hbf[:], 0.0)

            def dec_whh(rz, hgn, hbf_src, first):
                # h-side gate matmuls + n-gate hh-bias; pre-issued for the
                # NEXT step (data-ready as soon as hbf lands).
                for j in range(8):
                    for k in range(4):
                        nc.tensor.matmul(
                            rz[:, BD * j:BD * (j + 1)],
                            dwhh_b[k][:, 128 * j:128 * (j + 1)],
                            hbf_src[:, BD * k:BD * (k + 1)],
                            start=(j == 0 and k == 0), stop=False)
                for j in range(4):
                    for k in range(4):
                        nc.tensor.matmul(
                            hgn[:, BD * j:BD * (j + 1)],
                            dwhh_b[k][:, 128 * (8 + j):128 * (9 + j)],
                            hbf_src[:, BD * k:BD * (k + 1)],
                            start=(j == 0 and k == 0), stop=False)
                    nc.tensor.matmul(hgn[:, BD * j:BD * (j + 1)],
                                     dbhhn_s[0:1, 128 * j:128 * (j + 1)],
                                     ones_b[0:1, 0:BD],
                                     start=False, stop=(j == 3))

            def dec_m1bias(m1):
                for j in range(8):
                    nc.tensor.matmul(m1[:, BD * j:BD * (j + 1)],
                                     dm1b_s[0:1, 128 * j:128 * (j + 1)],
                                     ones_b[0:1, 0:BD],
                                     start=(j == 0), stop=False)

            def dec_m2bias(m2d):
                for j in range(4):
                    nc.tensor.matmul(m2d[:, BD * j:BD * (j + 1)],
                                     dm2b_s[0:1, 128 * j:128 * (j + 1)],
                                     ones_b[0:1, 0:BD],
                                     start=(j == 0), stop=False)

            rz = pA.tile([128, 512], F32, tag="A")
            hgn = pB.tile([128, 512], F32, tag="B")
            m1 = pD.tile([128, 512], F32, tag="D")
            m2d = pTR.tile([128, 512], F32, tag="TR")
            dec_m1bias(m1)
            dec_m2bias(m2d)
            # t=0: h=0, so no Whh matmuls; hgn(0) = bias only.
            for j in range(4):
                nc.tensor.matmul(hgn[:, BD * j:BD * (j + 1)],
                                 dbhhn_s[0:1, 128 * j:128 * (j + 1)],
                                 ones_b[0:1, 0:BD],
                                 start=(j == 0), stop=(j == 3))

            for t in range(hor):
                lastd = (t == hor - 1)
                # ---- y/const-side gate matmuls ----
                an = pC.tile([128, 512], F32, tag="C")
                for j in range(8):
                    nc.tensor.matmul(rz[:, BD * j:BD * (j + 1)],
                                     ycw[:, 128 * j:128 * (j + 1)], ypc[:],
                                     start=(t == 0 and j == 0),
                                     stop=(j == 7))
                for j in range(4):
                    nc.tensor.matmul(an[:, BD * j:BD * (j + 1)],
                                     ycw[:, 128 * (8 + j):128 * (9 + j)],
                                     ypc[:], start=(j == 0), stop=(j == 3))

                # ---- GRU eltwise, transposed layout [128, 4*32] ----
                r_t = tp.tile([128, 4 * BD], BF16, tag="dr")
                nc.scalar.activation(r_t[:], rz[:, 0:4 * BD], AF.Sigmoid)
                t2 = tp.tile([128, 4 * BD], BF16, tag="dt2")
                nc.vector.tensor_mul(t2[:], r_t[:], hgn[:, 0:4 * BD])
                npre = tp.tile([128, 4 * BD], BF16, tag="dnp")
                nc.vector.tensor_add(npre[:], t2[:], an[:, 0:4 * BD])
                z_t = tp.tile([128, 4 * BD], BF16, tag="dz")
                nc.scalar.activation(z_t[:], rz[:, 4 * BD:8 * BD], AF.Sigmoid)
                omz = tp.tile([128, 4 * BD], BF16, tag="domz")
                nc.vector.tensor_scalar(omz[:], z_t[:], -1.0, 1.0,
                                        OP.mult, OP.add)
                u_t = tp.tile([128, 4 * BD], BF16, tag="du")
                nc.gpsimd.tensor_mul(u_t[:], z_t[:], hbf[:])
                n_t = tp.tile([128, 4 * BD], BF16, tag="dn")
                nc.scalar.activation(n_t[:], npre[:], AF.Tanh)
                a_t = tp.tile([128, 4 * BD], BF16, tag="da")
                nc.vector.tensor_mul(a_t[:], omz[:], n_t[:])
                hbf_n = st.tile([128, 4 * BD], BF16, tag="hbf")
                nc.vector.tensor_add(hbf_n[:], a_t[:], u_t[:])
                hbf = hbf_n

                # ---- M1 (k-outer so chunks can start as h lands) ----
                for k in range(4):
                    for j in range(8):
                        nc.tensor.matmul(m1[:, BD * j:BD * (j + 1)],
                                         dm1_b[k][:, 128 * j:128 * (j + 1)],
                                         hbf[:, BD * k:BD * (k + 1)],
                                         start=False,
                                         stop=(k == 3 and j == 7))
                hm1_s = tp.tile([128, 8 * BD], BF16, tag="dhm1")
                nc.scalar.activation(hm1_s[:], m1[:, 0:8 * BD], AF.Relu)

                # ---- M2 ----
                for k in range(8):
                    for j in range(4):
                        nc.tensor.matmul(m2d[:, BD * j:BD * (j + 1)],
                                         dm2_b[k][:, 128 * j:128 * (j + 1)],
                                         hm1_s[:, BD * k:BD * (k + 1)],
                                         start=False,
                                         stop=(k == 7 and j == 3))
                hm2_s = tp.tile([128, 4 * BD], BF16, tag="dhm2")
                nc.scalar.activation(hm2_s[:], m2d[:, 0:4 * BD], AF.Relu)

                # bias pre-issue for t+1 (fills the out-matmul wait)
                if not lastd:
                    m1_n = pD.tile([128, 512], F32, tag="D")
                    dec_m1bias(m1_n)
                    m2_n = pTR.tile([128, 512], F32, tag="TR")
                    dec_m2bias(m2_n)

                # ---- output head: y [64, 32] ----
                yb = pC.tile([128, 512], F32, tag="C")
                for k in range(4):
                    nc.tensor.matmul(yb[0:NY, 0:BD],
                                     dow_b[k][:, 0:NY],
                                     hm2_s[:, BD * k:BD * (k + 1)],
                                     start=(k == 0), stop=(k == 3))
                if not lastd:
                    # critical path: feed y back (bf16) before the f32 copy
                    nc.scalar.activation(ypc[0:NY, :], yb[0:NY, 0:BD],
                                         AF.Identity, bias=dob_c[:])
                y_f = tp.tile([NY, BD], F32, tag="dy")
                nc.scalar.activation(y_f[:], yb[0:NY, 0:BD], AF.Identity,
                                     bias=dob_c[:])
                nc.sync.dma_start(d_out[NY * t:NY * (t + 1), :], y_f[:])
                if not lastd:
                    # pre-issue next step's h-side matmuls
                    rz_n = pA.tile([128, 512], F32, tag="A")
                    hgn_n = pB.tile([128, 512], F32, tag="B")
                    dec_whh(rz_n, hgn_n, hbf, False)
                    rz, hgn, m1, m2d = rz_n, hgn_n, m1_n, m2_n

    nc.compile()
    return nc


# ---------------------------------------------------------------------------
# Host-side sharding
# ---------------------------------------------------------------------------

def shard_inputs(inp, et=100, hor=60):
    f32 = np.float32
    x, y = np.asarray(inp["x"], f32), np.asarray(inp["y"], f32)
    chains = [("xf", False, x), ("xb", True, x),
              ("ef", False, y), ("eb", True, y)]
    in_maps = []
    shared = {}

    def wih_aug(pre):
        wih = np.asarray(inp[pre + "_Wih"], f32)
        bih = np.asarray(inp[pre + "_bih"], f32)
        bhh = np.asarray(inp[pre + "_bhh"], f32)
        aug = np.zeros((66, G), f32)
        aug[0:64, :] = wih.T
        bias = bih.copy()
        bias[0:2 * H] += bhh[0:2 * H]
        aug[64, :] = bias
        aug[65, H:2 * H] = BIG
        return aug

    d_Wih = np.asarray(inp["d_Wih"], f32)
    d_bih = np.asarray(inp["d_bih"], f32)
    d_bhh = np.asarray(inp["d_bhh"], f32)
    dc_b = d_bih.copy()
    dc_b[0:2 * H] += d_bhh[0:2 * H]

    shared["em_w1t"] = np.ascontiguousarray(np.asarray(inp["em_W1"], f32).T)
    shared["em_b1row"] = np.asarray(inp["em_b1"], f32)[None, :]
    shared["em_w2t"] = np.ascontiguousarray(np.asarray(inp["em_W2"], f32).T)
    shared["em_b2row"] = np.asarray(inp["em_b2"], f32)[None, :]
    shared["eo_wt"] = np.ascontiguousarray(np.asarray(inp["eo_W"], f32).T)
    shared["eo_brow"] = np.asarray(inp["eo_b"], f32)[None, :]
    shared["dc_wt"] = np.ascontiguousarray(d_Wih[:, 0:2 * H].T)
    shared["dc_brow"] = dc_b[None, :]
    shared["dwy_t"] = np.ascontiguousarray(d_Wih[:, 2 * H:].T)
    shared["dwhh_t"] = np.ascontiguousarray(np.asarray(inp["d_Whh"], f32).T)
    shared["dbhhn_row"] = np.ascontiguousarray(d_bhh[None, 2 * H:])
    shared["dm_w1t"] = np.ascontiguousarray(np.asarray(inp["dm_W1"], f32).T)
    shared["dm_b1row"] = np.asarray(inp["dm_b1"], f32)[None, :]
    shared["dm_w2t"] = np.ascontiguousarray(np.asarray(inp["dm_W2"], f32).T)
    shared["dm_b2row"] = np.asarray(inp["dm_b2"], f32)[None, :]
    shared["do_wt"] = np.ascontiguousarray(np.asarray(inp["do_W"], f32).T)
    shared["do_bcol"] = np.ascontiguousarray(
        np.asarray(inp["do_b"], f32)[:, None])

    for j in range(NCORE):
        chain, half = j // 2, j % 2
        pre, rev, seq = chains[chain]
        T = seq.shape[1]
        s = seq[128 * half:128 * (half + 1)]          # [128, T, 64]
        xin = np.zeros((66, et, BE), f32)
        xin[64, :, :] = 1.0
        pad = et - T
        if pad:
            xin[65, 0:pad, :] = 1.0
        order = np.arange(T)[::-1] if rev else np.arange(T)
        xin[0:64, pad:, :] = s[:, order, :].transpose(2, 1, 0)
        m = dict(shared)
        m["xin"] = np.ascontiguousarray(xin.reshape(66, et * BE))
        m["wih_aug"] = wih_aug(pre)
        m["whh_t"] = np.ascontiguousarray(np.asarray(inp[pre + "_Whh"],
                                                     f32).T)
        m["bhhn_row"] = np.ascontiguousarray(
            np.asarray(inp[pre + "_bhh"], f32)[None, 2 * H:])
        xl = np.concatenate([x[16 * j:16 * j + 16, -1, :],
                             x[128 + 16 * j:128 + 16 * j + 16, -1, :]])
        m["xlast_t"] = np.ascontiguousarray(xl.T)
        in_maps.append(m)
    return in_maps


def unshard(results, hor=60):
    out = np.zeros((B, hor, NY), np.float32)
    for j in range(NCORE):
        o = results[j]["out"].reshape(hor, NY, BD).transpose(2, 0, 1)
        out[16 * j:16 * j + 16] = o[0:16]
        out[128 + 16 * j:128 + 16 * j + 16] = o[16:32]
    return out


_NC = None


def kernel(**inputs):
    global _NC
    from concourse.bass_utils import run_bass_kernel_spmd
    if _NC is None:
        _NC = build_nc()
    in_maps = shard_inputs(inputs)
    res = run_bass_kernel_spmd(_NC, in_maps, core_ids=list(range(NCORE)))
    return unshard(res.results)


# revision 18
# speedup vs baseline: 1.8069x; 1.0265x over previous
"""Trainium2 Bass kernel for the GRU autoencoder (v4).

Distribution (8 NeuronCores):
  Encode : chain-parallel x batch-parallel. Core j handles GRU chain j//2
           (xf, xb, ef, eb) on batch half j%2 (128 rows), uniform 100-step
           loop; the 50-step x-chains get 50 exact identity steps (z forced
           to 1 via a +BIG flag row). AllToAll reshards 16-row slices so each
           core decodes global rows [16j:16j+16] u [128+16j:+16].

v4: everything bf16 end-to-end (weights shipped as bf16 from the host — no
device-side casting), encoder gate matmuls split into N=256 halves so the
eltwise/transpose tail software-pipelines across halves, decoder fully
transposed (weights-stationary, [feature, batch] layout, zero transposes).
"""

import sys

sys.path.insert(0, "/opt/trn_rl_repo")

import ml_dtypes
import numpy as np

import concourse.bass as bass
import concourse.mybir as mybir
import concourse.tile as tile
from concourse import bacc
from concourse.masks import make_identity

dt = mybir.dt
AF = mybir.ActivationFunctionType
OP = mybir.AluOpType

B, TX, TY, NX, NY, H, HOR = 256, 50, 100, 64, 64, 512, 60
M1, M2 = 1024, 512
G = 3 * H
NCORE = 8
BE = 128   # encoder batch rows per core
BD = 32    # decoder batch rows per core
BIG = 30000.0

F32, BF16 = dt.float32, dt.bfloat16
BF = ml_dtypes.bfloat16


def build_nc(et=100, hor=60):
    nc = bacc.Bacc("TRN2", target_bir_lowering=False, debug=False,
                   num_devices=NCORE)

    # ---- DRAM parameters (bf16 except the ACT bias column) ----
    d_xin = nc.dram_tensor("xin", [66, et * BE], BF16, kind="ExternalInput")
    d_wih = nc.dram_tensor("wih_aug", [66, G], BF16, kind="ExternalInput")
    d_whh = nc.dram_tensor("whh_t", [H, G], BF16, kind="ExternalInput")
    d_bhhn = nc.dram_tensor("bhhn_row", [1, H], BF16, kind="ExternalInput")

    d_em1 = nc.dram_tensor("em_w1t", [2 * H, M1], BF16, kind="ExternalInput")
    d_em2 = nc.dram_tensor("em_w2t", [M1, M2], BF16, kind="ExternalInput")
    d_eow = nc.dram_tensor("eo_wt", [M2, H], BF16, kind="ExternalInput")
    d_dcw = nc.dram_tensor("dc_wt", [2 * H, G], BF16, kind="ExternalInput")
    d_midb = nc.dram_tensor("mid_bias", [1, 3584], BF16,
                            kind="ExternalInput")

    d_dwy = nc.dram_tensor("dwy_t", [NY, G], BF16, kind="ExternalInput")
    d_dwhh = nc.dram_tensor("dwhh_t", [H, G], BF16, kind="ExternalInput")
    d_dbhhn = nc.dram_tensor("dbhhn_row", [1, H], BF16, kind="ExternalInput")
    d_dm1 = nc.dram_tensor("dm_w1t", [H, M1], BF16, kind="ExternalInput")
    d_dm1b = nc.dram_tensor("dm_b1row", [1, M1], BF16, kind="ExternalInput")
    d_dm2 = nc.dram_tensor("dm_w2t", [M1, M2], BF16, kind="ExternalInput")
    d_dm2b = nc.dram_tensor("dm_b2row", [1, M2], BF16, kind="ExternalInput")
    d_dow = nc.dram_tensor("do_wt", [M2, NY], BF16, kind="ExternalInput")
    d_dobc = nc.dram_tensor("do_bcol", [NY, 1], F32, kind="ExternalInput")
    d_xlast = nc.dram_tensor("xlast_t", [NX, BD], BF16, kind="ExternalInput")

    d_out = nc.dram_tensor("out", [hor * NY, BD], F32, kind="ExternalOutput")

    cc_in = nc.dram_tensor("cc_in", [BE, H], BF16)
    cc_out = nc.dram_tensor("cc_out", [NCORE, 16, H], BF16)

    with tile.TileContext(nc) as tc:
        with tc.tile_pool(name="pe", bufs=1) as pe, \
             tc.tile_pool(name="wts", bufs=1) as wts, \
             tc.tile_pool(name="xsp", bufs=2) as xsp, \
             tc.tile_pool(name="st", bufs=2) as st, \
             tc.tile_pool(name="tp", bufs=2) as tp, \
             tc.tile_pool(name="md", bufs=1) as md, \
             tc.tile_pool(name="pA", bufs=2, space="PSUM") as pA, \
             tc.tile_pool(name="pB", bufs=2, space="PSUM") as pB, \
             tc.tile_pool(name="pC", bufs=2, space="PSUM") as pC, \
             tc.tile_pool(name="pD", bufs=1, space="PSUM") as pD, \
             tc.tile_pool(name="pTR", bufs=1, space="PSUM") as pTR:

            # ---------- constants ----------
            idf = pe.tile([128, 128], F32, tag="idf")
            make_identity(nc, idf[:])
            idb = pe.tile([128, 128], BF16, tag="idb")
            nc.gpsimd.tensor_copy(idb[:], idf[:])
            ones_b = pe.tile([1, 128], BF16, tag="ones_b")
            nc.gpsimd.memset(ones_b[:], 1.0)
            zero_b = pe.tile([128, 512], BF16, tag="zero_b")
            nc.gpsimd.memset(zero_b[:], 0.0)

            def load_direct(pool, dram_ap, rows, cols, tag):
                r = pool.tile([rows, cols], BF16, tag=tag)
                nc.sync.dma_start(r[:], dram_ap)
                return r

            # ---------- encoder weights ----------
            wih_b = load_direct(wts, d_wih[:], 66, G, "wih")
            whh_b = [load_direct(wts, d_whh[128 * c:128 * (c + 1), :],
                                 128, G, f"whh{c}") for c in range(4)]
            ebhhn = load_direct(wts, d_bhhn[:], 1, H, "ebhhn")

            # ---------- encoder state ----------
            hT = pe.tile([128, H], BF16, tag="hT0")       # [feat%128, 4x128b]
            nc.vector.tensor_copy(hT[:], zero_b[:])
            h_bh = pe.tile([BE, H], BF16, tag="h0")       # [batch, feat]
            nc.gpsimd.memset(h_bh[:], 0.0)

            # ---------- middle + decoder weights (direct bf16 DMA) ----------
            em1_b = [load_direct(wts, d_em1[128 * c:128 * (c + 1), :],
                                 128, M1, f"em1_{c}") for c in range(8)]
            em2_b = [load_direct(wts, d_em2[128 * c:128 * (c + 1), :],
                                 128, M2, f"em2_{c}") for c in range(8)]
            eo_b = [load_direct(wts, d_eow[128 * c:128 * (c + 1), :],
                                128, H, f"eo_{c}") for c in range(4)]
            dcw_b = [load_direct(wts, d_dcw[128 * c:128 * (c + 1), :],
                                 128, G, f"dcw_{c}") for c in range(8)]
            bias_b = load_direct(pe, d_midb[:], 1, 3584, "bias_b")

            dwhh_b = [load_direct(wts, d_dwhh[128 * c:128 * (c + 1), :],
                                  128, G, f"dwhh{c}") for c in range(4)]
            dm1_b = [load_direct(wts, d_dm1[128 * c:128 * (c + 1), :],
                                 128, M1, f"dm1_{c}") for c in range(4)]
            dm2_b = [load_direct(wts, d_dm2[128 * c:128 * (c + 1), :],
                                 128, M2, f"dm2_{c}") for c in range(8)]
            dow_b = [load_direct(wts, d_dow[128 * c:128 * (c + 1), :],
                                 128, NY, f"dow_{c}") for c in range(4)]
            dbhhn_s = load_direct(wts, d_dbhhn[:], 1, H, "dbhhn")
            dm1b_s = load_direct(wts, d_dm1b[:], 1, M1, "dm1b")
            dm2b_s = load_direct(wts, d_dm2b[:], 1, M2, "dm2b")
            dob_c = wts.tile([NY, 1], F32, tag="dobc")
            nc.sync.dma_start(dob_c[:], d_dobc[:])
            # ycw: rows 0:64 = Wy^T, rows 64:96 = const (filled post-middle).
            ycw = pe.tile([96, G], BF16, tag="ycw")
            nc.sync.dma_start(ycw[0:NY, :], d_dwy[:])
            # ypc: rows 0:64 = y_t, rows 64:96 = I32 (selects const rows).
            ypc = pe.tile([96, BD], BF16, tag="ypc")
            nc.sync.dma_start(ypc[0:NX, :], d_xlast[:])
            nc.gpsimd.tensor_copy(ypc[64:96, :], idb[0:32, 0:32])

            # =======================================================
            # Encode loop, software-pipelined in feature halves.
            # =======================================================
            def enc_alloc():
                ga = pA.tile([BE, 512], F32, tag="A")
                gb = pB.tile([BE, 512], F32, tag="B")
                gc = pC.tile([BE, 512], F32, tag="C")
                gd = pD.tile([BE, 512], F32, tag="D")
                return ga, gb, gc, gd

            def enc_xs_mms(xs, ga, gb, gc, gd):
                nc.tensor.matmul(ga[:], xs[:], wih_b[:, 0:512],
                                 start=True, stop=False)
                nc.tensor.matmul(gb[:], xs[:], wih_b[:, 512:1024],
                                 start=True, stop=False)
                nc.tensor.matmul(gc[:], xs[:], wih_b[:, 1024:1536],
                                 start=True, stop=True)
                nc.tensor.matmul(gd[:], ones_b[0:1, 0:BE], ebhhn[:],
                                 start=True, stop=False)

            def load_xs(t):
                xb = xsp.tile([66, 128], BF16, tag="xs_b")
                nc.sync.dma_start(xb[:], d_xin[:, t * BE:(t + 1) * BE])
                return xb

            xs = load_xs(0)
            ga, gb, gc, gd = enc_alloc()
            enc_xs_mms(xs, ga, gb, gc, gd)

            for t in range(et):
                last = (t == et - 1)
                # h-side matmuls, half-major: r/n/z for cols 0:256 first.
                for s in range(2):
                    lo = 256 * s
                    for c in range(4):
                        nc.tensor.matmul(ga[:, lo:lo + 256],
                                         hT[:, 128 * c:128 * (c + 1)],
                                         whh_b[c][:, lo:lo + 256],
                                         start=False,
                                         stop=(s == 1 and c == 3))
                    for c in range(4):
                        nc.tensor.matmul(gd[:, lo:lo + 256],
                                         hT[:, 128 * c:128 * (c + 1)],
                                         whh_b[c][:, 1024 + lo:1280 + lo],
                                         start=False,
                                         stop=(s == 1 and c == 3))
                    for c in range(4):
                        nc.tensor.matmul(gb[:, lo:lo + 256],
                                         hT[:, 128 * c:128 * (c + 1)],
                                         whh_b[c][:, 512 + lo:768 + lo],
                                         start=False,
                                         stop=(s == 1 and c == 3))
                if not last:
                    xs_n = load_xs(t + 1)
                    ga_n, gb_n, gc_n, gd_n = enc_alloc()
                    enc_xs_mms(xs_n, ga_n, gb_n, gc_n, gd_n)

                # ---- eltwise in halves: h' = (1-z)*n + z*h ----
                r_t = tp.tile([BE, 512], BF16, tag="r")
                z_t = tp.tile([BE, 512], BF16, tag="z")
                n_t = tp.tile([BE, 512], BF16, tag="n")
                rhn = tp.tile([BE, 512], BF16, tag="rhn")
                npre = tp.tile([BE, 512], BF16, tag="npre")
                omz = tp.tile([BE, 512], BF16, tag="omz")
                b_t = tp.tile([BE, 512], BF16, tag="b")
                a_t = tp.tile([BE, 512], BF16, tag="a")
                h_new = st.tile([BE, H], BF16, tag="h")
                ptr = pTR.tile([128, 512], BF16, tag="TR")
                hT_new = st.tile([128, H], BF16, tag="hT")

                sl = [slice(0, 256), slice(256, 512)]
                # ACT: r0, z0, r1, t0, z1, t1 (+cp0 later)
                # DVE: rhn0, npre0, omz0, rhn1, npre1, omz1, a0, h0, a1, h1
                nc.scalar.activation(r_t[:, sl[0]], ga[:, sl[0]], AF.Sigmoid)
                nc.scalar.activation(z_t[:, sl[0]], gb[:, sl[0]], AF.Sigmoid)
                nc.vector.tensor_mul(rhn[:, sl[0]], r_t[:, sl[0]],
                                     gd[:, sl[0]])
                nc.vector.tensor_add(npre[:, sl[0]], rhn[:, sl[0]],
                                     gc[:, sl[0]])
                nc.vector.tensor_scalar(omz[:, sl[0]], z_t[:, sl[0]],
                                        -1.0, 1.0, OP.mult, OP.add)
                nc.gpsimd.tensor_mul(b_t[:, sl[0]], z_t[:, sl[0]],
                                     h_bh[:, sl[0]])
                nc.scalar.activation(r_t[:, sl[1]], ga[:, sl[1]], AF.Sigmoid)
                nc.scalar.activation(n_t[:, sl[0]], npre[:, sl[0]], AF.Tanh)
                nc.vector.tensor_mul(rhn[:, sl[1]], r_t[:, sl[1]],
                                     gd[:, sl[1]])
                nc.vector.tensor_add(npre[:, sl[1]], rhn[:, sl[1]],
                                     gc[:, sl[1]])
                nc.scalar.activation(z_t[:, sl[1]], gb[:, sl[1]], AF.Sigmoid)
                nc.vector.tensor_scalar(omz[:, sl[1]], z_t[:, sl[1]],
                                        -1.0, 1.0, OP.mult, OP.add)
                nc.gpsimd.tensor_mul(b_t[:, sl[1]], z_t[:, sl[1]],
                                     h_bh[:, sl[1]])
                nc.scalar.activation(n_t[:, sl[1]], npre[:, sl[1]], AF.Tanh)
                nc.vector.tensor_mul(a_t[:, sl[0]], omz[:, sl[0]],
                                     n_t[:, sl[0]])
                nc.vector.tensor_add(h_new[:, sl[0]], a_t[:, sl[0]],
                                     b_t[:, sl[0]])
                if not last:
                    for c in (0, 1):
                        nc.tensor.transpose(ptr[:, 128 * c:128 * (c + 1)],
                                            h_new[:, 128 * c:128 * (c + 1)],
                                            idb[:])
                    nc.scalar.copy(hT_new[:, sl[0]], ptr[:, sl[0]])
                nc.vector.tensor_mul(a_t[:, sl[1]], omz[:, sl[1]],
                                     n_t[:, sl[1]])
                nc.vector.tensor_add(h_new[:, sl[1]], a_t[:, sl[1]],
                                     b_t[:, sl[1]])
                if not last:
                    for c in (2, 3):
                        nc.tensor.transpose(ptr[:, 128 * c:128 * (c + 1)],
                                            h_new[:, 128 * c:128 * (c + 1)],
                                            idb[:])
                    nc.vector.tensor_copy(hT_new[:, sl[1]], ptr[:, sl[1]])
                    hT = hT_new
                    ga, gb, gc, gd = ga_n, gb_n, gc_n, gd_n
                h_bh = h_new

            # ---------- reshard: AllToAll of 16-row slices (bf16) ----------
            nc.sync.dma_start(cc_in[:], h_bh[:])
            nc.gpsimd.collective_compute(
                "AllToAll", OP.bypass,
                replica_groups=[list(range(NCORE))],
                ins=[cc_in[:]], outs=[cc_out[:]])

            pxa = md.tile([BD, H], BF16, tag="pA")
            pxb = md.tile([BD, H], BF16, tag="pB")
            pya = md.tile([BD, H], BF16, tag="pA")
            pyb = md.tile([BD, H], BF16, tag="pB")
            nc.sync.dma_start(pxa[0:16, :], cc_out[0][:])
            nc.sync.dma_start(pxa[16:32, :], cc_out[1][:])
            nc.sync.dma_start(pxb[0:16, :], cc_out[2][:])
            nc.sync.dma_start(pxb[16:32, :], cc_out[3][:])
            nc.sync.dma_start(pya[0:16, :], cc_out[4][:])
            nc.sync.dma_start(pya[16:32, :], cc_out[5][:])
            nc.sync.dma_start(pyb[0:16, :], cc_out[6][:])
            nc.sync.dma_start(pyb[16:32, :], cc_out[7][:])
            hx = md.tile([BD, H], F32, tag="hx")
            hy = md.tile([BD, H], F32, tag="hy")
            nc.vector.tensor_add(hx[:], pxa[:], pxb[:])
            nc.vector.tensor_add(hy[:], pya[:], pyb[:])

            def trsp_b(src, cols, tag):
                """src [BD, cols] f32 -> bf16 [128, (cols//128)*BD] via PE."""
                nch = cols // 128
                p = pTR.tile([128, 512], F32, tag="TR")
                for c in range(nch):
                    nc.tensor.transpose(p[:, BD * c:BD * (c + 1)],
                                        src[:, 128 * c:128 * (c + 1)],
                                        idf[0:32, 0:32])
                o = md.tile([128, nch * BD], BF16, tag=tag)
                nc.scalar.copy(o[:], p[:, 0:nch * BD])
                return o

            hxT = trsp_b(hx, H, "hxT")
            hyT = trsp_b(hy, H, "hyT")

            # ---- middle MLP (batch-major, activations stationary) ----
            m1a = pA.tile([BD, 512], F32, tag="A")
            m1b = pB.tile([BD, 512], F32, tag="B")
            for c in range(8):
                wt = em1_b[c]
                s = (hxT if c < 4 else hyT)[:, BD * (c % 4):BD * (c % 4 + 1)]
                nc.tensor.matmul(m1a[:], s, wt[:, 0:512],
                                 start=(c == 0), stop=False)
                nc.tensor.matmul(m1b[:], s, wt[:, 512:1024],
                                 start=(c == 0), stop=False)
            nc.tensor.matmul(m1a[:], ones_b[0:1, 0:BD],
                             bias_b[0:1, 1536:2048], start=False, stop=True)
            nc.tensor.matmul(m1b[:], ones_b[0:1, 0:BD],
                             bias_b[0:1, 2048:2560], start=False, stop=True)
            hm1 = md.tile([BD, M1], F32, tag="hm1")
            nc.scalar.activation(hm1[:, 0:512], m1a[:], AF.Relu)
            nc.scalar.activation(hm1[:, 512:1024], m1b[:], AF.Relu)
            hm1T = trsp_b(hm1, M1, "hm1T_m")

            m2 = pC.tile([BD, M2], F32, tag="C")
            for c in range(8):
                nc.tensor.matmul(m2[:], hm1T[:, BD * c:BD * (c + 1)],
                                 em2_b[c][:], start=(c == 0), stop=False)
            nc.tensor.matmul(m2[:], ones_b[0:1, 0:BD], bias_b[0:1, 2560:3072],
                             start=False, stop=True)
            hm2 = md.tile([BD, M2], F32, tag="hm2")
            nc.scalar.activation(hm2[:], m2[:], AF.Relu)
            hm2T = trsp_b(hm2, M2, "hm2T_m")

            zp = pD.tile([BD, 512], F32, tag="D")
            for c in range(4):
                nc.tensor.matmul(zp[:, 0:H], hm2T[:, BD * c:BD * (c + 1)],
                                 eo_b[c][:], start=(c == 0), stop=False)
            nc.tensor.matmul(zp[:, 0:H], ones_b[0:1, 0:BD],
                             bias_b[0:1, 3072:3584], start=False, stop=True)
            z_sb = md.tile([BD, H], F32, tag="z_sb")
            nc.scalar.copy(z_sb[:], zp[:, 0:H])
            zT = trsp_b(z_sb, H, "zT")

            # const = cat(h_x, z) @ d_Wih[:, :2H].T + d_bih + d_bhh(r,z)
            cpa = pA.tile([BD, 512], F32, tag="A")
            cpb = pB.tile([BD, 512], F32, tag="B")
            cpn = pC.tile([BD, 512], F32, tag="C")
            for c in range(8):
                wt = dcw_b[c]
                s = (hxT if c < 4 else zT)[:, BD * (c % 4):BD * (c % 4 + 1)]
                nc.tensor.matmul(cpa[:], s, wt[:, 0:512],
                                 start=(c == 0), stop=False)
                nc.tensor.matmul(cpb[:], s, wt[:, 512:1024],
                                 start=(c == 0), stop=False)
                nc.tensor.matmul(cpn[:], s, wt[:, 1024:1536],
                                 start=(c == 0), stop=False)
            nc.tensor.matmul(cpa[:], ones_b[0:1, 0:BD],
                             bias_b[0:1, 0:512], start=False, stop=True)
            nc.tensor.matmul(cpb[:], ones_b[0:1, 0:BD],
                             bias_b[0:1, 512:1024], start=False, stop=True)
            nc.tensor.matmul(cpn[:], ones_b[0:1, 0:BD],
                             bias_b[0:1, 1024:1536], start=False, stop=True)
            nc.vector.tensor_copy(ycw[64:96, 0:512], cpa[:])
            nc.vector.tensor_copy(ycw[64:96, 512:1024], cpb[:])
            nc.vector.tensor_copy(ycw[64:96, 1024:1536], cpn[:])

            # =======================================================
            # Decode loop: fully transposed, h as [128, 4*32] bf16.
            # =======================================================
            hbf = st.tile([128, 4 * BD], BF16, tag="hbf")
            nc.gpsimd.memset(hbf[:], 0.0)

            def dec_whh(rz, hgn, hbf_src):
                for j in range(8):
                    for k in range(4):
                        nc.tensor.matmul(
                            rz[:, BD * j:BD * (j + 1)],
                            dwhh_b[k][:, 128 * j:128 * (j + 1)],
                            hbf_src[:, BD * k:BD * (k + 1)],
                            start=(j == 0 and k == 0), stop=False)
                for j in range(4):
                    for k in range(4):
                        nc.tensor.matmul(
                            hgn[:, BD * j:BD * (j + 1)],
                            dwhh_b[k][:, 128 * (8 + j):128 * (9 + j)],
                            hbf_src[:, BD * k:BD * (k + 1)],
                            start=(j == 0 and k == 0), stop=False)
                    nc.tensor.matmul(hgn[:, BD * j:BD * (j + 1)],
                                     dbhhn_s[0:1, 128 * j:128 * (j + 1)],
                                     ones_b[0:1, 0:BD],
                                     start=False, stop=(j == 3))

            def dec_m1bias(m1):
                for j in range(8):
                    nc.tensor.matmul(m1[:, BD * j:BD * (j + 1)],
                                     dm1b_s[0:1, 128 * j:128 * (j + 1)],
                                     ones_b[0:1, 0:BD],
                                     start=(j == 0), stop=False)

            def dec_m2bias(m2d):
                for j in range(4):
                    nc.tensor.matmul(m2d[:, BD * j:BD * (j + 1)],
                                     dm2b_s[0:1, 128 * j:128 * (j + 1)],
                                     ones_b[0:1, 0:BD],
                                     start=(j == 0), stop=False)

            rz = pA.tile([128, 512], F32, tag="A")
            hgn = pB.tile([128, 512], F32, tag="B")
            m1 = pD.tile([128, 512], F32, tag="D")
            m2d = pTR.tile([128, 512], F32, tag="TR")
            dec_m1bias(m1)
            dec_m2bias(m2d)
            # t=0: h=0, so no Whh matmuls; hgn(0) = bias only.
            for j in range(4):
                nc.tensor.matmul(hgn[:, BD * j:BD * (j + 1)],
                                 dbhhn_s[0:1, 128 * j:128 * (j + 1)],
                                 ones_b[0:1, 0:BD],
                                 start=(j == 0), stop=(j == 3))

            for t in range(hor):
                lastd = (t == hor - 1)
                # ---- y/const-side gate matmuls ----
                an = pC.tile([128, 512], F32, tag="C")
                for j in range(8):
                    nc.tensor.matmul(rz[:, BD * j:BD * (j + 1)],
                                     ycw[:, 128 * j:128 * (j + 1)], ypc[:],
                                     start=(t == 0 and j == 0),
                                     stop=(j == 7))
                for j in range(4):
                    nc.tensor.matmul(an[:, BD * j:BD * (j + 1)],
                                     ycw[:, 128 * (8 + j):128 * (9 + j)],
                                     ypc[:], start=(j == 0), stop=(j == 3))

                # ---- GRU eltwise, transposed layout [128, 4*32] ----
                r_t = tp.tile([128, 4 * BD], BF16, tag="dr")
                nc.scalar.activation(r_t[:], rz[:, 0:4 * BD], AF.Sigmoid)
                t2 = tp.tile([128, 4 * BD], BF16, tag="dt2")
                nc.vector.tensor_mul(t2[:], r_t[:], hgn[:, 0:4 * BD])
                npre = tp.tile([128, 4 * BD], BF16, tag="dnp")
                nc.vector.tensor_add(npre[:], t2[:], an[:, 0:4 * BD])
                z_t = tp.tile([128, 4 * BD], BF16, tag="dz")
                nc.scalar.activation(z_t[:], rz[:, 4 * BD:8 * BD], AF.Sigmoid)
                omz = tp.tile([128, 4 * BD], BF16, tag="domz")
                nc.vector.tensor_scalar(omz[:], z_t[:], -1.0, 1.0,
                                        OP.mult, OP.add)
                u_t = tp.tile([128, 4 * BD], BF16, tag="du")
                nc.gpsimd.tensor_mul(u_t[:], z_t[:], hbf[:])
                n_t = tp.tile([128, 4 * BD], BF16, tag="dn")
                nc.scalar.activation(n_t[:], npre[:], AF.Tanh)
                a_t = tp.tile([128, 4 * BD], BF16, tag="da")
                nc.vector.tensor_mul(a_t[:], omz[:], n_t[:])
                hbf_n = st.tile([128, 4 * BD], BF16, tag="hbf")
                nc.vector.tensor_add(hbf_n[:], a_t[:], u_t[:])
                hbf = hbf_n

                # ---- M1 (k-outer so chunks start as h lands) ----
                for k in range(4):
                    for j in range(8):
                        nc.tensor.matmul(m1[:, BD * j:BD * (j + 1)],
                                         dm1_b[k][:, 128 * j:128 * (j + 1)],
                                         hbf[:, BD * k:BD * (k + 1)],
                                         start=False,
                                         stop=(k == 3 and j == 7))
                hm1_s = tp.tile([128, 8 * BD], BF16, tag="dhm1")
                nc.scalar.activation(hm1_s[:], m1[:, 0:8 * BD], AF.Relu)

                # ---- M2 ----
                for k in range(8):
                    for j in range(4):
                        nc.tensor.matmul(m2d[:, BD * j:BD * (j + 1)],
                                         dm2_b[k][:, 128 * j:128 * (j + 1)],
                                         hm1_s[:, BD * k:BD * (k + 1)],
                                         start=False,
                                         stop=(k == 7 and j == 3))
                hm2_s = tp.tile([128, 4 * BD], BF16, tag="dhm2")
                nc.scalar.activation(hm2_s[:], m2d[:, 0:4 * BD], AF.Relu)

                # bias pre-issue for t+1 (fills the out-matmul wait)
                if not lastd:
                    m1_n = pD.tile([128, 512], F32, tag="D")
                    dec_m1bias(m1_n)
                    m2_n = pTR.tile([128, 512], F32, tag="TR")
                    dec_m2bias(m2_n)

                # ---- output head: y [64, 32] ----
                yb = pC.tile([128, 512], F32, tag="C")
                for k in range(4):
                    nc.tensor.matmul(yb[0:NY, 0:BD],
                                     dow_b[k][:, 0:NY],
                                     hm2_s[:, BD * k:BD * (k + 1)],
                                     start=(k == 0), stop=(k == 3))
                if not lastd:
                    # critical path: feed y back (bf16) before the f32 copy
                    nc.scalar.activation(ypc[0:NY, :], yb[0:NY, 0:BD],
                                         AF.Identity, bias=dob_c[:])
                y_f = tp.tile([NY, BD], F32, tag="dy")
                nc.scalar.activation(y_f[:], yb[0:NY, 0:BD], AF.Identity,
                                     bias=dob_c[:])
                nc.sync.dma_start(d_out[NY * t:NY * (t + 1), :], y_f[:])
                if not lastd:
                    # pre-issue next step's h-side matmuls
                    rz_n = pA.tile([128, 512], F32, tag="A")
                    hgn_n = pB.tile([128, 512], F32, tag="B")
                    dec_whh(rz_n, hgn_n, hbf)
                    rz, hgn, m1, m2d = rz_n, hgn_n, m1_n, m2_n

    nc.compile()
    return nc


# ---------------------------------------------------------------------------
# Host-side sharding
# ---------------------------------------------------------------------------

def shard_inputs(inp, et=100, hor=60):
    f32 = np.float32

    def bf(a):
        return np.ascontiguousarray(np.asarray(a, f32).astype(BF))

    x, y = np.asarray(inp["x"], f32), np.asarray(inp["y"], f32)
    chains = [("xf", False, x), ("xb", True, x),
              ("ef", False, y), ("eb", True, y)]
    in_maps = []
    shared = {}

    def wih_aug(pre):
        wih = np.asarray(inp[pre + "_Wih"], f32)
        bih = np.asarray(inp[pre + "_bih"], f32)
        bhh = np.asarray(inp[pre + "_bhh"], f32)
        aug = np.zeros((66, G), f32)
        aug[0:64, :] = wih.T
        bias = bih.copy()
        bias[0:2 * H] += bhh[0:2 * H]
        aug[64, :] = bias
        aug[65, H:2 * H] = BIG
        return bf(aug)

    d_Wih = np.asarray(inp["d_Wih"], f32)
    d_bih = np.asarray(inp["d_bih"], f32)
    d_bhh = np.asarray(inp["d_bhh"], f32)
    dc_b = d_bih.copy()
    dc_b[0:2 * H] += d_bhh[0:2 * H]

    shared["em_w1t"] = bf(np.asarray(inp["em_W1"], f32).T)
    shared["em_w2t"] = bf(np.asarray(inp["em_W2"], f32).T)
    shared["eo_wt"] = bf(np.asarray(inp["eo_W"], f32).T)
    shared["dc_wt"] = bf(d_Wih[:, 0:2 * H].T)
    midb = np.concatenate([dc_b, np.asarray(inp["em_b1"], f32),
                           np.asarray(inp["em_b2"], f32),
                           np.asarray(inp["eo_b"], f32)])[None, :]
    shared["mid_bias"] = bf(midb)
    shared["dwy_t"] = bf(d_Wih[:, 2 * H:].T)
    shared["dwhh_t"] = bf(np.asarray(inp["d_Whh"], f32).T)
    shared["dbhhn_row"] = bf(d_bhh[None, 2 * H:])
    shared["dm_w1t"] = bf(np.asarray(inp["dm_W1"], f32).T)
    shared["dm_b1row"] = bf(np.asarray(inp["dm_b1"], f32)[None, :])
    shared["dm_w2t"] = bf(np.asarray(inp["dm_W2"], f32).T)
    shared["dm_b2row"] = bf(np.asarray(inp["dm_b2"], f32)[None, :])
    shared["do_wt"] = bf(np.asarray(inp["do_W"], f32).T)
    shared["do_bcol"] = np.ascontiguousarray(
        np.asarray(inp["do_b"], f32)[:, None])

    for j in range(NCORE):
        chain, half = j // 2, j % 2
        pre, rev, seq = chains[chain]
        T = seq.shape[1]
        s = seq[128 * half:128 * (half + 1)]          # [128, T, 64]
        xin = np.zeros((66, et, BE), f32)
        xin[64, :, :] = 1.0
        pad = et - T
        if pad:
            xin[65, 0:pad, :] = 1.0
        order = np.arange(T)[::-1] if rev else np.arange(T)
        xin[0:64, pad:, :] = s[:, order, :].transpose(2, 1, 0)
        m = dict(shared)
        m["xin"] = bf(xin.reshape(66, et * BE))
        m["wih_aug"] = wih_aug(pre)
        m["whh_t"] = bf(np.asarray(inp[pre + "_Whh"], f32).T)
        m["bhhn_row"] = bf(np.asarray(inp[pre + "_bhh"], f32)[None, 2 * H:])
        xl = np.concatenate([x[16 * j:16 * j + 16, -1, :],
                             x[128 + 16 * j:128 + 16 * j + 16, -1, :]])
        m["xlast_t"] = bf(xl.T)
        in_maps.append(m)
    return in_maps


def unshard(results, hor=60):
    out = np.zeros((B, hor, NY), np.float32)
    for j in range(NCORE):
        o = results[j]["out"].reshape(hor, NY, BD).transpose(2, 0, 1)
        out[16 * j:16 * j + 16] = o[0:16]
        out[128 + 16 * j:128 + 16 * j + 16] = o[16:32]
    return out


_NC = None


def kernel(**inputs):
    global _NC
    from concourse.bass_utils import run_bass_kernel_spmd
    if _NC is None:
        _NC = build_nc()
    in_maps = shard_inputs(inputs)
    res = run_bass_kernel_spmd(_NC, in_maps, core_ids=list(range(NCORE)))
    return unshard(res.results)


# revision 20
# speedup vs baseline: 2.0322x; 1.1247x over previous
"""Trainium2 Bass kernel for the GRU autoencoder (v4).

Distribution (8 NeuronCores):
  Encode : chain-parallel x batch-parallel. Core j handles GRU chain j//2
           (xf, xb, ef, eb) on batch half j%2 (128 rows), uniform 100-step
           loop; the 50-step x-chains get 50 exact identity steps (z forced
           to 1 via a +BIG flag row). AllToAll reshards 16-row slices so each
           core decodes global rows [16j:16j+16] u [128+16j:+16].

v4: everything bf16 end-to-end (weights shipped as bf16 from the host — no
device-side casting), encoder gate matmuls split into N=256 halves so the
eltwise/transpose tail software-pipelines across halves, decoder fully
transposed (weights-stationary, [feature, batch] layout, zero transposes).
"""

import sys

sys.path.insert(0, "/opt/trn_rl_repo")

import ml_dtypes
import numpy as np

import concourse.bass as bass
import concourse.mybir as mybir
import concourse.tile as tile
from concourse import bacc
from concourse.masks import make_identity

dt = mybir.dt
AF = mybir.ActivationFunctionType
OP = mybir.AluOpType

B, TX, TY, NX, NY, H, HOR = 256, 50, 100, 64, 64, 512, 60
M1, M2 = 1024, 512
G = 3 * H
NCORE = 8
BE = 128   # encoder batch rows per core
BD = 32    # decoder batch rows per core
BIG = 30000.0

F32, BF16 = dt.float32, dt.bfloat16
BF = ml_dtypes.bfloat16


def build_nc(et=100, hor=60):
    nc = bacc.Bacc("TRN2", target_bir_lowering=False, debug=False,
                   num_devices=NCORE)

    # ---- DRAM parameters (bf16 except the ACT bias column) ----
    d_xin = nc.dram_tensor("xin", [66, et * BE], BF16, kind="ExternalInput")
    d_wih = nc.dram_tensor("wih_aug", [66, G], BF16, kind="ExternalInput")
    d_whh = nc.dram_tensor("whh_t", [H, G], BF16, kind="ExternalInput")
    d_bhhn = nc.dram_tensor("bhhn_row", [1, H], BF16, kind="ExternalInput")

    d_em1 = nc.dram_tensor("em_w1t", [2 * H, M1], BF16, kind="ExternalInput")
    d_em2 = nc.dram_tensor("em_w2t", [M1, M2], BF16, kind="ExternalInput")
    d_eow = nc.dram_tensor("eo_wt", [M2, H], BF16, kind="ExternalInput")
    d_dcw = nc.dram_tensor("dc_wt", [2 * H, G], BF16, kind="ExternalInput")
    d_midb = nc.dram_tensor("mid_bias", [1, 3584], BF16,
                            kind="ExternalInput")

    d_dwy = nc.dram_tensor("dwy_t", [NY, G], BF16, kind="ExternalInput")
    d_dwhh = nc.dram_tensor("dwhh_t", [H, G], BF16, kind="ExternalInput")
    d_dbhhn = nc.dram_tensor("dbhhn_row", [1, H], BF16, kind="ExternalInput")
    d_dm1 = nc.dram_tensor("dm_w1t", [H, M1], BF16, kind="ExternalInput")
    d_dm1b = nc.dram_tensor("dm_b1row", [1, M1], BF16, kind="ExternalInput")
    d_dm2 = nc.dram_tensor("dm_w2t", [M1, M2], BF16, kind="ExternalInput")
    d_dm2b = nc.dram_tensor("dm_b2row", [1, M2], BF16, kind="ExternalInput")
    d_dow = nc.dram_tensor("do_wt", [M2, NY], BF16, kind="ExternalInput")
    d_dobc = nc.dram_tensor("do_bcol", [NY, 1], F32, kind="ExternalInput")
    d_xlast = nc.dram_tensor("xlast_t", [NX, BD], BF16, kind="ExternalInput")

    d_out = nc.dram_tensor("out", [hor * NY, BD], F32, kind="ExternalOutput")

    cc_in = nc.dram_tensor("cc_in", [BE, H], BF16)
    cc_out = nc.dram_tensor("cc_out", [NCORE, 16, H], BF16)

    with tile.TileContext(nc) as tc:
        with tc.tile_pool(name="pe", bufs=1) as pe, \
             tc.tile_pool(name="wts", bufs=1) as wts, \
             tc.tile_pool(name="xsp", bufs=2) as xsp, \
             tc.tile_pool(name="st", bufs=2) as st, \
             tc.tile_pool(name="tp", bufs=2) as tp, \
             tc.tile_pool(name="md", bufs=1) as md, \
             tc.tile_pool(name="pA", bufs=2, space="PSUM") as pA, \
             tc.tile_pool(name="pB", bufs=2, space="PSUM") as pB, \
             tc.tile_pool(name="pC", bufs=2, space="PSUM") as pC, \
             tc.tile_pool(name="pD", bufs=1, space="PSUM") as pD, \
             tc.tile_pool(name="pTR", bufs=1, space="PSUM") as pTR:

            # ---------- constants ----------
            idf = pe.tile([128, 128], F32, tag="idf")
            make_identity(nc, idf[:])
            idb = pe.tile([128, 128], BF16, tag="idb")
            nc.gpsimd.tensor_copy(idb[:], idf[:])
            ones_b = pe.tile([1, 128], BF16, tag="ones_b")
            nc.gpsimd.memset(ones_b[:], 1.0)
            zero_b = pe.tile([128, 512], BF16, tag="zero_b")
            nc.gpsimd.memset(zero_b[:], 0.0)

            def load_direct(pool, dram_ap, rows, cols, tag):
                r = pool.tile([rows, cols], BF16, tag=tag)
                nc.sync.dma_start(r[:], dram_ap)
                return r

            # Middle/decoder weights: allocate now, DMA lazily inside the
            # encode loop (one tile per step) so the startup xin load isn't
            # queued behind ~6MB of weight traffic.
            wload = []

            def load_lazy(dram_ap, rows, cols, tag):
                r = wts.tile([rows, cols], BF16, tag=tag)
                wload.append((r, dram_ap))
                return r

            # ---------- encoder weights (needed immediately) ----------
            wih_b = load_direct(wts, d_wih[:], 66, G, "wih")
            whh_b = [load_direct(wts, d_whh[128 * c:128 * (c + 1), :],
                                 128, G, f"whh{c}") for c in range(4)]
            ebhhn = load_direct(wts, d_bhhn[:], 1, H, "ebhhn")

            # ---------- encoder state ----------
            hT = pe.tile([128, H], BF16, tag="hT0")       # [feat%128, 4x128b]
            nc.vector.tensor_copy(hT[:], zero_b[:])
            h_bh = pe.tile([BE, H], BF16, tag="h0")       # [batch, feat]
            nc.gpsimd.memset(h_bh[:], 0.0)

            # ---------- middle + decoder weights (lazy bf16 DMA) ----------
            em1_b = [load_lazy(d_em1[128 * c:128 * (c + 1), :],
                               128, M1, f"em1_{c}") for c in range(8)]
            em2_b = [load_lazy(d_em2[128 * c:128 * (c + 1), :],
                               128, M2, f"em2_{c}") for c in range(8)]
            eo_b = [load_lazy(d_eow[128 * c:128 * (c + 1), :],
                              128, H, f"eo_{c}") for c in range(4)]
            dcw_b = [load_lazy(d_dcw[128 * c:128 * (c + 1), :],
                               128, G, f"dcw_{c}") for c in range(8)]
            bias_b = pe.tile([1, 3584], BF16, tag="bias_b")
            wload.append((bias_b, d_midb[:]))

            dwhh_b = [load_lazy(d_dwhh[128 * c:128 * (c + 1), :],
                                128, G, f"dwhh{c}") for c in range(4)]
            dm1_b = [load_lazy(d_dm1[128 * c:128 * (c + 1), :],
                               128, M1, f"dm1_{c}") for c in range(4)]
            dm2_b = [load_lazy(d_dm2[128 * c:128 * (c + 1), :],
                               128, M2, f"dm2_{c}") for c in range(8)]
            dow_b = [load_lazy(d_dow[128 * c:128 * (c + 1), :],
                               128, NY, f"dow_{c}") for c in range(4)]
            dbhhn_s = load_lazy(d_dbhhn[:], 1, H, "dbhhn")
            dm1b_s = load_lazy(d_dm1b[:], 1, M1, "dm1b")
            dm2b_s = load_lazy(d_dm2b[:], 1, M2, "dm2b")
            dob_c = wts.tile([NY, 1], F32, tag="dobc")
            nc.sync.dma_start(dob_c[:], d_dobc[:])
            # ycw: rows 0:64 = Wy^T, rows 64:96 = const (filled post-middle).
            ycw = pe.tile([96, G], BF16, tag="ycw")
            wload.append((ycw[0:NY, :], d_dwy[:]))
            # ypc: rows 0:64 = y_t, rows 64:96 = I32 (selects const rows).
            ypc = pe.tile([96, BD], BF16, tag="ypc")
            nc.sync.dma_start(ypc[0:NX, :], d_xlast[:])
            nc.gpsimd.tensor_copy(ypc[64:96, :], idb[0:32, 0:32])

            # =======================================================
            # Encode loop, software-pipelined in feature halves.
            # =======================================================
            def enc_alloc():
                ga = pA.tile([BE, 512], F32, tag="A")
                gb = pB.tile([BE, 512], F32, tag="B")
                gc = pC.tile([BE, 512], F32, tag="C")
                gd = pD.tile([BE, 512], F32, tag="D")
                return ga, gb, gc, gd

            def enc_xs_mms(xs, ga, gb, gc, gd):
                nc.tensor.matmul(ga[:], xs[:], wih_b[:, 0:512],
                                 start=True, stop=False)
                nc.tensor.matmul(gb[:], xs[:], wih_b[:, 512:1024],
                                 start=True, stop=False)
                nc.tensor.matmul(gc[:], xs[:], wih_b[:, 1024:1536],
                                 start=True, stop=True)
                nc.tensor.matmul(gd[:], ones_b[0:1, 0:BE], ebhhn[:],
                                 start=True, stop=False)

            def load_xs(t):
                xb = xsp.tile([66, 128], BF16, tag="xs_b")
                nc.sync.dma_start(xb[:], d_xin[:, t * BE:(t + 1) * BE])
                return xb

            xs = load_xs(0)
            ga, gb, gc, gd = enc_alloc()
            enc_xs_mms(xs, ga, gb, gc, gd)

            for t in range(et):
                last = (t == et - 1)
                # h-side matmuls, bank-major: r-gates, n-h-gates, z-gates.
                for c in range(4):
                    nc.tensor.matmul(ga[:], hT[:, 128 * c:128 * (c + 1)],
                                     whh_b[c][:, 0:512],
                                     start=False, stop=(c == 3))
                for c in range(4):
                    nc.tensor.matmul(gd[:], hT[:, 128 * c:128 * (c + 1)],
                                     whh_b[c][:, 1024:1536],
                                     start=False, stop=(c == 3))
                for c in range(4):
                    nc.tensor.matmul(gb[:], hT[:, 128 * c:128 * (c + 1)],
                                     whh_b[c][:, 512:1024],
                                     start=False, stop=(c == 3))
                if not last:
                    xs_n = load_xs(t + 1)
                    if t < len(wload):
                        wa, wd = wload[t]
                        nc.sync.dma_start(wa[:], wd)
                    ga_n, gb_n, gc_n, gd_n = enc_alloc()
                    enc_xs_mms(xs_n, ga_n, gb_n, gc_n, gd_n)

                # ---- eltwise: h' = (1-z)*n + z*h; tanh/a/h'/transpose/copy
                # run in feature halves so hT chunks land early; dummy
                # transposes chained on eltwise temps keep the PE active
                # through the tail (HAM stays at K=8/8).
                r_t = tp.tile([BE, 512], BF16, tag="r")
                z_t = tp.tile([BE, 512], BF16, tag="z")
                n_t = tp.tile([BE, 512], BF16, tag="n")
                rhn = tp.tile([BE, 512], BF16, tag="rhn")
                npre = tp.tile([BE, 512], BF16, tag="npre")
                omz = tp.tile([BE, 512], BF16, tag="omz")
                b_t = tp.tile([BE, 512], BF16, tag="b")
                a_t = tp.tile([BE, 512], BF16, tag="a")
                h_new = st.tile([BE, H], BF16, tag="h")
                ptr = pTR.tile([128, 512], BF16, tag="TR")
                hT_new = st.tile([128, H], BF16, tag="hT")

                sl = [slice(0, 256), slice(256, 512)]
                nc.scalar.activation(r_t[:], ga[:], AF.Sigmoid)
                nc.vector.tensor_mul(rhn[:], r_t[:], gd[:])
                nc.vector.tensor_add(npre[:], rhn[:], gc[:])
                nc.scalar.activation(z_t[:], gb[:], AF.Sigmoid)
                nc.vector.tensor_scalar(omz[:], z_t[:], -1.0, 1.0,
                                        OP.mult, OP.add)
                nc.gpsimd.tensor_mul(b_t[:, sl[0]], z_t[:, sl[0]],
                                     h_bh[:, sl[0]])
                nc.gpsimd.tensor_mul(b_t[:, sl[1]], z_t[:, sl[1]],
                                     h_bh[:, sl[1]])
                nc.scalar.activation(n_t[:, sl[0]], npre[:, sl[0]], AF.Tanh)
                nc.scalar.activation(n_t[:, sl[1]], npre[:, sl[1]], AF.Tanh)
                if not last:
                    # HAM fillers: cheap transposes gated on eltwise temps
                    nc.tensor.transpose(ptr[:, 0:128], r_t[:, 0:128],
                                        idb[:])
                    nc.tensor.transpose(ptr[:, 128:256], npre[:, 0:128],
                                        idb[:])
                    nc.tensor.transpose(ptr[:, 256:384], n_t[:, 0:128],
                                        idb[:])
                for s in range(2):
                    nc.vector.tensor_mul(a_t[:, sl[s]], omz[:, sl[s]],
                                         n_t[:, sl[s]])
                    nc.vector.tensor_add(h_new[:, sl[s]], a_t[:, sl[s]],
                                         b_t[:, sl[s]])
                    if not last:
                        for c in (2 * s, 2 * s + 1):
                            nc.tensor.transpose(
                                ptr[:, 128 * c:128 * (c + 1)],
                                h_new[:, 128 * c:128 * (c + 1)], idb[:])
                        if s == 0:
                            nc.scalar.copy(hT_new[:, sl[0]], ptr[:, sl[0]])
                        else:
                            nc.vector.tensor_copy(hT_new[:, sl[1]],
                                                  ptr[:, sl[1]])
                if not last:
                    hT = hT_new
                    ga, gb, gc, gd = ga_n, gb_n, gc_n, gd_n
                h_bh = h_new

            # ---------- reshard: AllToAll of 16-row slices (bf16) ----------
            nc.sync.dma_start(cc_in[:], h_bh[:])
            nc.gpsimd.collective_compute(
                "AllToAll", OP.bypass,
                replica_groups=[list(range(NCORE))],
                ins=[cc_in[:]], outs=[cc_out[:]])

            pxa = md.tile([BD, H], BF16, tag="pA")
            pxb = md.tile([BD, H], BF16, tag="pB")
            pya = md.tile([BD, H], BF16, tag="pA")
            pyb = md.tile([BD, H], BF16, tag="pB")
            nc.sync.dma_start(pxa[0:16, :], cc_out[0][:])
            nc.sync.dma_start(pxa[16:32, :], cc_out[1][:])
            nc.sync.dma_start(pxb[0:16, :], cc_out[2][:])
            nc.sync.dma_start(pxb[16:32, :], cc_out[3][:])
            nc.sync.dma_start(pya[0:16, :], cc_out[4][:])
            nc.sync.dma_start(pya[16:32, :], cc_out[5][:])
            nc.sync.dma_start(pyb[0:16, :], cc_out[6][:])
            nc.sync.dma_start(pyb[16:32, :], cc_out[7][:])
            hx = md.tile([BD, H], F32, tag="hx")
            hy = md.tile([BD, H], F32, tag="hy")
            nc.vector.tensor_add(hx[:], pxa[:], pxb[:])
            nc.vector.tensor_add(hy[:], pya[:], pyb[:])

            def trsp_b(src, cols, tag):
                """src [BD, cols] f32 -> bf16 [128, (cols//128)*BD] via PE."""
                nch = cols // 128
                p = pTR.tile([128, 512], F32, tag="TR")
                for c in range(nch):
                    nc.tensor.transpose(p[:, BD * c:BD * (c + 1)],
                                        src[:, 128 * c:128 * (c + 1)],
                                        idf[0:32, 0:32])
                o = md.tile([128, nch * BD], BF16, tag=tag)
                nc.scalar.copy(o[:], p[:, 0:nch * BD])
                return o

            hxT = trsp_b(hx, H, "hxT")
            hyT = trsp_b(hy, H, "hyT")

            # ---- middle MLP (batch-major, activations stationary) ----
            m1a = pA.tile([BD, 512], F32, tag="A")
            m1b = pB.tile([BD, 512], F32, tag="B")
            for c in range(8):
                wt = em1_b[c]
                s = (hxT if c < 4 else hyT)[:, BD * (c % 4):BD * (c % 4 + 1)]
                nc.tensor.matmul(m1a[:], s, wt[:, 0:512],
                                 start=(c == 0), stop=False)
                nc.tensor.matmul(m1b[:], s, wt[:, 512:1024],
                                 start=(c == 0), stop=False)
            nc.tensor.matmul(m1a[:], ones_b[0:1, 0:BD],
                             bias_b[0:1, 1536:2048], start=False, stop=True)
            nc.tensor.matmul(m1b[:], ones_b[0:1, 0:BD],
                             bias_b[0:1, 2048:2560], start=False, stop=True)
            hm1 = md.tile([BD, M1], F32, tag="hm1")
            nc.scalar.activation(hm1[:, 0:512], m1a[:], AF.Relu)
            nc.scalar.activation(hm1[:, 512:1024], m1b[:], AF.Relu)
            hm1T = trsp_b(hm1, M1, "hm1T_m")

            m2 = pC.tile([BD, M2], F32, tag="C")
            for c in range(8):
                nc.tensor.matmul(m2[:], hm1T[:, BD * c:BD * (c + 1)],
                                 em2_b[c][:], start=(c == 0), stop=False)
            nc.tensor.matmul(m2[:], ones_b[0:1, 0:BD], bias_b[0:1, 2560:3072],
                             start=False, stop=True)
            hm2 = md.tile([BD, M2], F32, tag="hm2")
            nc.scalar.activation(hm2[:], m2[:], AF.Relu)
            hm2T = trsp_b(hm2, M2, "hm2T_m")

            zp = pD.tile([BD, 512], F32, tag="D")
            for c in range(4):
                nc.tensor.matmul(zp[:, 0:H], hm2T[:, BD * c:BD * (c + 1)],
                                 eo_b[c][:], start=(c == 0), stop=False)
            nc.tensor.matmul(zp[:, 0:H], ones_b[0:1, 0:BD],
                             bias_b[0:1, 3072:3584], start=False, stop=True)
            z_sb = md.tile([BD, H], F32, tag="z_sb")
            nc.scalar.copy(z_sb[:], zp[:, 0:H])
            zT = trsp_b(z_sb, H, "zT")

            # const = cat(h_x, z) @ d_Wih[:, :2H].T + d_bih + d_bhh(r,z)
            cpa = pA.tile([BD, 512], F32, tag="A")
            cpb = pB.tile([BD, 512], F32, tag="B")
            cpn = pC.tile([BD, 512], F32, tag="C")
            for c in range(8):
                wt = dcw_b[c]
                s = (hxT if c < 4 else zT)[:, BD * (c % 4):BD * (c % 4 + 1)]
                nc.tensor.matmul(cpa[:], s, wt[:, 0:512],
                                 start=(c == 0), stop=False)
                nc.tensor.matmul(cpb[:], s, wt[:, 512:1024],
                                 start=(c == 0), stop=False)
                nc.tensor.matmul(cpn[:], s, wt[:, 1024:1536],
                                 start=(c == 0), stop=False)
            nc.tensor.matmul(cpa[:], ones_b[0:1, 0:BD],
                             bias_b[0:1, 0:512], start=False, stop=True)
            nc.tensor.matmul(cpb[:], ones_b[0:1, 0:BD],
                             bias_b[0:1, 512:1024], start=False, stop=True)
            nc.tensor.matmul(cpn[:], ones_b[0:1, 0:BD],
                             bias_b[0:1, 1024:1536], start=False, stop=True)
            nc.vector.tensor_copy(ycw[64:96, 0:512], cpa[:])
            nc.vector.tensor_copy(ycw[64:96, 512:1024], cpb[:])
            nc.vector.tensor_copy(ycw[64:96, 1024:1536], cpn[:])

            # =======================================================
            # Decode loop: fully transposed, h as [128, 4*32] bf16.
            # =======================================================
            hbf = st.tile([128, 4 * BD], BF16, tag="hbf")
            nc.gpsimd.memset(hbf[:], 0.0)

            def dec_whh(rz, hgn, hbf_src):
                for j in range(8):
                    for k in range(4):
                        nc.tensor.matmul(
                            rz[:, BD * j:BD * (j + 1)],
                            dwhh_b[k][:, 128 * j:128 * (j + 1)],
                            hbf_src[:, BD * k:BD * (k + 1)],
                            start=(j == 0 and k == 0), stop=False)
                for j in range(4):
                    for k in range(4):
                        nc.tensor.matmul(
                            hgn[:, BD * j:BD * (j + 1)],
                            dwhh_b[k][:, 128 * (8 + j):128 * (9 + j)],
                            hbf_src[:, BD * k:BD * (k + 1)],
                            start=(j == 0 and k == 0), stop=False)
                    nc.tensor.matmul(hgn[:, BD * j:BD * (j + 1)],
                                     dbhhn_s[0:1, 128 * j:128 * (j + 1)],
                                     ones_b[0:1, 0:BD],
                                     start=False, stop=(j == 3))

            def dec_m1bias(m1):
                for j in range(8):
                    nc.tensor.matmul(m1[:, BD * j:BD * (j + 1)],
                                     dm1b_s[0:1, 128 * j:128 * (j + 1)],
                                     ones_b[0:1, 0:BD],
                                     start=(j == 0), stop=False)

            def dec_m2bias(m2d):
                for j in range(4):
                    nc.tensor.matmul(m2d[:, BD * j:BD * (j + 1)],
                                     dm2b_s[0:1, 128 * j:128 * (j + 1)],
                                     ones_b[0:1, 0:BD],
                                     start=(j == 0), stop=False)

            rz = pA.tile([128, 512], F32, tag="A")
            hgn = pB.tile([128, 512], F32, tag="B")
            m1 = pD.tile([128, 512], F32, tag="D")
            m2d = pTR.tile([128, 512], F32, tag="TR")
            dec_m1bias(m1)
            dec_m2bias(m2d)
            # t=0: h=0, so no Whh matmuls; hgn(0) = bias only.
            for j in range(4):
                nc.tensor.matmul(hgn[:, BD * j:BD * (j + 1)],
                                 dbhhn_s[0:1, 128 * j:128 * (j + 1)],
                                 ones_b[0:1, 0:BD],
                                 start=(j == 0), stop=(j == 3))

            for t in range(hor):
                lastd = (t == hor - 1)
                # ---- y/const-side gate matmuls ----
                an = pC.tile([128, 512], F32, tag="C")
                for j in range(8):
                    nc.tensor.matmul(rz[:, BD * j:BD * (j + 1)],
                                     ycw[:, 128 * j:128 * (j + 1)], ypc[:],
                                     start=(t == 0 and j == 0),
                                     stop=(j == 7))
                for j in range(4):
                    nc.tensor.matmul(an[:, BD * j:BD * (j + 1)],
                                     ycw[:, 128 * (8 + j):128 * (9 + j)],
                                     ypc[:], start=(j == 0), stop=(j == 3))

                # ---- GRU eltwise, transposed layout [128, 4*32] ----
                r_t = tp.tile([128, 4 * BD], BF16, tag="dr")
                nc.scalar.activation(r_t[:], rz[:, 0:4 * BD], AF.Sigmoid)
                t2 = tp.tile([128, 4 * BD], BF16, tag="dt2")
                nc.vector.tensor_mul(t2[:], r_t[:], hgn[:, 0:4 * BD])
                npre = tp.tile([128, 4 * BD], BF16, tag="dnp")
                nc.vector.tensor_add(npre[:], t2[:], an[:, 0:4 * BD])
                z_t = tp.tile([128, 4 * BD], BF16, tag="dz")
                nc.scalar.activation(z_t[:], rz[:, 4 * BD:8 * BD], AF.Sigmoid)
                omz = tp.tile([128, 4 * BD], BF16, tag="domz")
                nc.vector.tensor_scalar(omz[:], z_t[:], -1.0, 1.0,
                                        OP.mult, OP.add)
                u_t = tp.tile([128, 4 * BD], BF16, tag="du")
                nc.gpsimd.tensor_mul(u_t[:], z_t[:], hbf[:])
                n_t = tp.tile([128, 4 * BD], BF16, tag="dn")
                nc.scalar.activation(n_t[:], npre[:], AF.Tanh)
                a_t = tp.tile([128, 4 * BD], BF16, tag="da")
                nc.vector.tensor_mul(a_t[:], omz[:], n_t[:])
                hbf_n = st.tile([128, 4 * BD], BF16, tag="hbf")
                nc.vector.tensor_add(hbf_n[:], a_t[:], u_t[:])
                hbf = hbf_n

                # ---- M1 (k-outer so chunks start as h lands) ----
                for k in range(4):
                    for j in range(8):
                        nc.tensor.matmul(m1[:, BD * j:BD * (j + 1)],
                                         dm1_b[k][:, 128 * j:128 * (j + 1)],
                                         hbf[:, BD * k:BD * (k + 1)],
                                         start=False,
                                         stop=(k == 3 and j == 7))
                hm1_s = tp.tile([128, 8 * BD], BF16, tag="dhm1")
                nc.scalar.activation(hm1_s[:], m1[:, 0:8 * BD], AF.Relu)

                # ---- M2 ----
                for k in range(8):
                    for j in range(4):
                        nc.tensor.matmul(m2d[:, BD * j:BD * (j + 1)],
                                         dm2_b[k][:, 128 * j:128 * (j + 1)],
                                         hm1_s[:, BD * k:BD * (k + 1)],
                                         start=False,
                                         stop=(k == 7 and j == 3))
                hm2_s = tp.tile([128, 4 * BD], BF16, tag="dhm2")
                nc.scalar.activation(hm2_s[:], m2d[:, 0:4 * BD], AF.Relu)

                # bias pre-issue for t+1 (fills the out-matmul wait)
                if not lastd:
                    m1_n = pD.tile([128, 512], F32, tag="D")
                    dec_m1bias(m1_n)
                    m2_n = pTR.tile([128, 512], F32, tag="TR")
                    dec_m2bias(m2_n)

                # ---- output head: y [64, 32] ----
                yb = pC.tile([128, 512], F32, tag="C")
                for k in range(4):
                    nc.tensor.matmul(yb[0:NY, 0:BD],
                                     dow_b[k][:, 0:NY],
                                     hm2_s[:, BD * k:BD * (k + 1)],
                                     start=(k == 0), stop=(k == 3))
                if not lastd:
                    # critical path: feed y back (bf16) before the f32 copy
                    nc.scalar.activation(ypc[0:NY, :], yb[0:NY, 0:BD],
                                         AF.Identity, bias=dob_c[:])
                y_f = tp.tile([NY, BD], F32, tag="dy")
                nc.scalar.activation(y_f[:], yb[0:NY, 0:BD], AF.Identity,
                                     bias=dob_c[:])
                nc.sync.dma_start(d_out[NY * t:NY * (t + 1), :], y_f[:])
                if not lastd:
                    # pre-issue next step's h-side matmuls
                    rz_n = pA.tile([128, 512], F32, tag="A")
                    hgn_n = pB.tile([128, 512], F32, tag="B")
                    dec_whh(rz_n, hgn_n, hbf)
                    rz, hgn, m1, m2d = rz_n, hgn_n, m1_n, m2_n

    nc.compile()
    return nc


# ---------------------------------------------------------------------------
# Host-side sharding
# ---------------------------------------------------------------------------

def shard_inputs(inp, et=100, hor=60):
    f32 = np.float32

    def bf(a):
        return np.ascontiguousarray(np.asarray(a, f32).astype(BF))

    x, y = np.asarray(inp["x"], f32), np.asarray(inp["y"], f32)
    chains = [("xf", False, x), ("xb", True, x),
              ("ef", False, y), ("eb", True, y)]
    in_maps = []
    shared = {}

    def wih_aug(pre):
        wih = np.asarray(inp[pre + "_Wih"], f32)
        bih = np.asarray(inp[pre + "_bih"], f32)
        bhh = np.asarray(inp[pre + "_bhh"], f32)
        aug = np.zeros((66, G), f32)
        aug[0:64, :] = wih.T
        bias = bih.copy()
        bias[0:2 * H] += bhh[0:2 * H]
        aug[64, :] = bias
        aug[65, H:2 * H] = BIG
        return bf(aug)

    d_Wih = np.asarray(inp["d_Wih"], f32)
    d_bih = np.asarray(inp["d_bih"], f32)
    d_bhh = np.asarray(inp["d_bhh"], f32)
    dc_b = d_bih.copy()
    dc_b[0:2 * H] += d_bhh[0:2 * H]

    shared["em_w1t"] = bf(np.asarray(inp["em_W1"], f32).T)
    shared["em_w2t"] = bf(np.asarray(inp["em_W2"], f32).T)
    shared["eo_wt"] = bf(np.asarray(inp["eo_W"], f32).T)
    shared["dc_wt"] = bf(d_Wih[:, 0:2 * H].T)
    midb = np.concatenate([dc_b, np.asarray(inp["em_b1"], f32),
                           np.asarray(inp["em_b2"], f32),
                           np.asarray(inp["eo_b"], f32)])[None, :]
    shared["mid_bias"] = bf(midb)
    shared["dwy_t"] = bf(d_Wih[:, 2 * H:].T)
    shared["dwhh_t"] = bf(np.asarray(inp["d_Whh"], f32).T)
    shared["dbhhn_row"] = bf(d_bhh[None, 2 * H:])
    shared["dm_w1t"] = bf(np.asarray(inp["dm_W1"], f32).T)
    shared["dm_b1row"] = bf(np.asarray(inp["dm_b1"], f32)[None, :])
    shared["dm_w2t"] = bf(np.asarray(inp["dm_W2"], f32).T)
    shared["dm_b2row"] = bf(np.asarray(inp["dm_b2"], f32)[None, :])
    shared["do_wt"] = bf(np.asarray(inp["do_W"], f32).T)
    shared["do_bcol"] = np.ascontiguousarray(
        np.asarray(inp["do_b"], f32)[:, None])

    for j in range(NCORE):
        chain, half = j // 2, j % 2
        pre, rev, seq = chains[chain]
        T = seq.shape[1]
        s = seq[128 * half:128 * (half + 1)]          # [128, T, 64]
        xin = np.zeros((66, et, BE), f32)
        xin[64, :, :] = 1.0
        pad = et - T
        if pad:
            xin[65, 0:pad, :] = 1.0
        order = np.arange(T)[::-1] if rev else np.arange(T)
        xin[0:64, pad:, :] = s[:, order, :].transpose(2, 1, 0)
        m = dict(shared)
        m["xin"] = bf(xin.reshape(66, et * BE))
        m["wih_aug"] = wih_aug(pre)
        m["whh_t"] = bf(np.asarray(inp[pre + "_Whh"], f32).T)
        m["bhhn_row"] = bf(np.asarray(inp[pre + "_bhh"], f32)[None, 2 * H:])
        xl = np.concatenate([x[16 * j:16 * j + 16, -1, :],
                             x[128 + 16 * j:128 + 16 * j + 16, -1, :]])
        m["xlast_t"] = bf(xl.T)
        in_maps.append(m)
    return in_maps


def unshard(results, hor=60):
    out = np.zeros((B, hor, NY), np.float32)
    for j in range(NCORE):
        o = results[j]["out"].reshape(hor, NY, BD).transpose(2, 0, 1)
        out[16 * j:16 * j + 16] = o[0:16]
        out[128 + 16 * j:128 + 16 * j + 16] = o[16:32]
    return out


_NC = None


def kernel(**inputs):
    global _NC
    from concourse.bass_utils import run_bass_kernel_spmd
    if _NC is None:
        _NC = build_nc()
    in_maps = shard_inputs(inputs)
    res = run_bass_kernel_spmd(_NC, in_maps, core_ids=list(range(NCORE)))
    return unshard(res.results)


# revision 28
# speedup vs baseline: 2.0354x; 1.0016x over previous
"""Trainium2 Bass kernel for the GRU autoencoder (v4).

Distribution (8 NeuronCores):
  Encode : chain-parallel x batch-parallel. Core j handles GRU chain j//2
           (xf, xb, ef, eb) on batch half j%2 (128 rows), uniform 100-step
           loop; the 50-step x-chains get 50 exact identity steps (z forced
           to 1 via a +BIG flag row). AllToAll reshards 16-row slices so each
           core decodes global rows [16j:16j+16] u [128+16j:+16].

v4: everything bf16 end-to-end (weights shipped as bf16 from the host — no
device-side casting), encoder gate matmuls split into N=256 halves so the
eltwise/transpose tail software-pipelines across halves, decoder fully
transposed (weights-stationary, [feature, batch] layout, zero transposes).
"""

import sys

sys.path.insert(0, "/opt/trn_rl_repo")

import ml_dtypes
import numpy as np

import concourse.bass as bass
import concourse.mybir as mybir
import concourse.tile as tile
from concourse import bacc
from concourse.masks import make_identity

dt = mybir.dt
AF = mybir.ActivationFunctionType
OP = mybir.AluOpType

B, TX, TY, NX, NY, H, HOR = 256, 50, 100, 64, 64, 512, 60
M1, M2 = 1024, 512
G = 3 * H
NCORE = 8
BE = 128   # encoder batch rows per core
BD = 32    # decoder batch rows per core
BIG = 30000.0

F32, BF16 = dt.float32, dt.bfloat16
BF = ml_dtypes.bfloat16


def build_nc(et=100, hor=60):
    nc = bacc.Bacc("TRN2", target_bir_lowering=False, debug=False,
                   num_devices=NCORE)

    # ---- DRAM parameters (bf16 except the ACT bias column) ----
    d_xin = nc.dram_tensor("xin", [66, et * BE], BF16, kind="ExternalInput")
    d_wih = nc.dram_tensor("wih_aug", [66, G], BF16, kind="ExternalInput")
    d_whh = nc.dram_tensor("whh_t", [H, G], BF16, kind="ExternalInput")
    d_bhhn = nc.dram_tensor("bhhn_row", [1, H], BF16, kind="ExternalInput")

    d_em1 = nc.dram_tensor("em_w1t", [2 * H, M1], BF16, kind="ExternalInput")
    d_em2 = nc.dram_tensor("em_w2t", [M1, M2], BF16, kind="ExternalInput")
    d_eow = nc.dram_tensor("eo_wt", [M2, H], BF16, kind="ExternalInput")
    d_dcw = nc.dram_tensor("dc_wt", [2 * H, G], BF16, kind="ExternalInput")
    d_midb = nc.dram_tensor("mid_bias", [1, 3584], BF16,
                            kind="ExternalInput")

    d_dwy = nc.dram_tensor("dwy_t", [NY, G], BF16, kind="ExternalInput")
    d_dwhh = nc.dram_tensor("dwhh_t", [H, G], BF16, kind="ExternalInput")
    d_dbhhn = nc.dram_tensor("dbhhn_row", [1, H], BF16, kind="ExternalInput")
    d_dm1 = nc.dram_tensor("dm_w1t", [H, M1], BF16, kind="ExternalInput")
    d_dm1b = nc.dram_tensor("dm_b1row", [1, M1], BF16, kind="ExternalInput")
    d_dm2 = nc.dram_tensor("dm_w2t", [M1, M2], BF16, kind="ExternalInput")
    d_dm2b = nc.dram_tensor("dm_b2row", [1, M2], BF16, kind="ExternalInput")
    d_dow = nc.dram_tensor("do_wt", [M2, NY], BF16, kind="ExternalInput")
    d_dobc = nc.dram_tensor("do_bcol", [NY, 1], F32, kind="ExternalInput")
    d_xlast = nc.dram_tensor("xlast_t", [NX, BD], BF16, kind="ExternalInput")

    d_out = nc.dram_tensor("out", [hor * NY, BD], F32, kind="ExternalOutput")

    cc_in = nc.dram_tensor("cc_in", [BE, H], BF16)
    cc_out = nc.dram_tensor("cc_out", [NCORE, 16, H], BF16)

    with tile.TileContext(nc) as tc:
        with tc.tile_pool(name="pe", bufs=1) as pe, \
             tc.tile_pool(name="wts", bufs=1) as wts, \
             tc.tile_pool(name="xsp", bufs=2) as xsp, \
             tc.tile_pool(name="st", bufs=2) as st, \
             tc.tile_pool(name="tp", bufs=2) as tp, \
             tc.tile_pool(name="md", bufs=1) as md, \
             tc.tile_pool(name="pA", bufs=2, space="PSUM") as pA, \
             tc.tile_pool(name="pB", bufs=2, space="PSUM") as pB, \
             tc.tile_pool(name="pC", bufs=2, space="PSUM") as pC, \
             tc.tile_pool(name="pD", bufs=1, space="PSUM") as pD, \
             tc.tile_pool(name="pTR", bufs=1, space="PSUM") as pTR:

            # ---------- constants ----------
            idf = pe.tile([128, 128], F32, tag="idf")
            make_identity(nc, idf[:])
            idb = pe.tile([128, 128], BF16, tag="idb")
            nc.gpsimd.tensor_copy(idb[:], idf[:])
            ones_b = pe.tile([1, 128], BF16, tag="ones_b")
            nc.gpsimd.memset(ones_b[:], 1.0)
            zero_b = pe.tile([128, 512], BF16, tag="zero_b")
            nc.gpsimd.memset(zero_b[:], 0.0)

            def load_direct(pool, dram_ap, rows, cols, tag):
                r = pool.tile([rows, cols], BF16, tag=tag)
                nc.sync.dma_start(r[:], dram_ap)
                return r

            # Middle/decoder weights: allocate now, DMA lazily inside the
            # encode loop (one tile per step) so the startup xin load isn't
            # queued behind ~6MB of weight traffic.
            wload = []

            def load_lazy(dram_ap, rows, cols, tag, rdt=BF16):
                r = wts.tile([rows, cols], rdt, tag=tag)
                wload.append((r, dram_ap))
                return r

            # ---------- encoder weights (needed immediately) ----------
            wih_b = load_direct(wts, d_wih[:], 66, G, "wih")
            whh_b = [load_direct(wts, d_whh[128 * c:128 * (c + 1), :],
                                 128, G, f"whh{c}") for c in range(4)]
            ebhhn = load_direct(wts, d_bhhn[:], 1, H, "ebhhn")

            # ---------- encoder state ----------
            hT = pe.tile([128, H], BF16, tag="hT0")       # [feat%128, 4x128b]
            nc.vector.tensor_copy(hT[:], zero_b[:])
            h_bh = pe.tile([BE, H], BF16, tag="h0")       # [batch, feat]
            nc.gpsimd.memset(h_bh[:], 0.0)

            # ---------- middle + decoder weights (lazy bf16 DMA) ----------
            em1_b = [load_lazy(d_em1[128 * c:128 * (c + 1), :],
                               128, M1, f"em1_{c}") for c in range(8)]
            em2_b = [load_lazy(d_em2[128 * c:128 * (c + 1), :],
                               128, M2, f"em2_{c}") for c in range(8)]
            eo_b = [load_lazy(d_eow[128 * c:128 * (c + 1), :],
                              128, H, f"eo_{c}") for c in range(4)]
            dcw_b = [load_lazy(d_dcw[128 * c:128 * (c + 1), :],
                               128, G, f"dcw_{c}") for c in range(8)]
            bias_b = pe.tile([1, 3584], BF16, tag="bias_b")
            wload.append((bias_b, d_midb[:]))

            dwhh_b = [load_lazy(d_dwhh[128 * c:128 * (c + 1), :],
                                128, G, f"dwhh{c}") for c in range(4)]
            dm1_b = [load_lazy(d_dm1[128 * c:128 * (c + 1), :],
                               128, M1, f"dm1_{c}") for c in range(4)]
            dm2_b = [load_lazy(d_dm2[128 * c:128 * (c + 1), :],
                               128, M2, f"dm2_{c}") for c in range(8)]
            dow_b = [load_lazy(d_dow[128 * c:128 * (c + 1), :],
                               128, NY, f"dow_{c}") for c in range(4)]
            dbhhn_s = load_lazy(d_dbhhn[:], 1, H, "dbhhn")
            dm1b_s = load_lazy(d_dm1b[:], 1, M1, "dm1b")
            dm2b_s = load_lazy(d_dm2b[:], 1, M2, "dm2b")
            dob_c = wts.tile([NY, 1], F32, tag="dobc")
            nc.sync.dma_start(dob_c[:], d_dobc[:])
            # ycw: rows 0:64 = Wy^T, rows 64:96 = const (filled post-middle).
            ycw = pe.tile([96, G], BF16, tag="ycw")
            wload.append((ycw[0:NY, :], d_dwy[:]))
            # ypc: rows 0:64 = y_t, rows 64:96 = I32 (selects const rows).
            ypc = pe.tile([96, BD], BF16, tag="ypc")
            nc.sync.dma_start(ypc[0:NX, :], d_xlast[:])
            nc.gpsimd.tensor_copy(ypc[64:96, :], idb[0:32, 0:32])

            # =======================================================
            # Encode loop, software-pipelined in feature halves.
            # =======================================================
            def enc_alloc():
                ga = pA.tile([BE, 512], F32, tag="A")
                gb = pB.tile([BE, 512], F32, tag="B")
                gc = pC.tile([BE, 512], F32, tag="C")
                gd = pD.tile([BE, 512], F32, tag="D")
                return ga, gb, gc, gd

            def enc_xs_mms(xs, ga, gb, gc):
                nc.tensor.matmul(ga[:], xs[:], wih_b[:, 0:512],
                                 start=True, stop=False)
                nc.tensor.matmul(gb[:], xs[:], wih_b[:, 512:1024],
                                 start=True, stop=False)
                nc.tensor.matmul(gc[:], xs[:], wih_b[:, 1024:1536],
                                 start=True, stop=True)

            def enc_bias_mm(gd):
                nc.tensor.matmul(gd[:], ones_b[0:1, 0:BE], ebhhn[:],
                                 start=True, stop=False)

            def load_xs(t):
                xb = xsp.tile([66, 128], BF16, tag="xs_b")
                nc.sync.dma_start(xb[:], d_xin[:, t * BE:(t + 1) * BE])
                return xb

            xs = load_xs(0)
            ga, gb, gc, gd = enc_alloc()
            enc_xs_mms(xs, ga, gb, gc)
            enc_bias_mm(gd)

            for t in range(et):
                last = (t == et - 1)
                # h-side matmuls, bank-major: r-gates, z-gates, n-h-gates
                # (z early so b=z*h and omz are off the critical path).
                for c in range(4):
                    nc.tensor.matmul(ga[:], hT[:, 128 * c:128 * (c + 1)],
                                     whh_b[c][:, 0:512],
                                     start=False, stop=(c == 3))
                for c in range(4):
                    nc.tensor.matmul(gb[:], hT[:, 128 * c:128 * (c + 1)],
                                     whh_b[c][:, 512:1024],
                                     start=False, stop=(c == 3))
                for c in range(4):
                    nc.tensor.matmul(gd[:], hT[:, 128 * c:128 * (c + 1)],
                                     whh_b[c][:, 1024:1536],
                                     start=False, stop=(c == 3))
                if not last:
                    xs_n = load_xs(t + 1)
                    if t < len(wload):
                        wa, wd = wload[t]
                        nc.sync.dma_start(wa[:], wd)
                    ga_n, gb_n, gc_n, gd_n = enc_alloc()
                    enc_xs_mms(xs_n, ga_n, gb_n, gc_n)

                # ---- eltwise: h' = (1-z)*n + z*h; rhn/npre and the tail
                # run in feature halves so tanh/hT chunks land early; dummy
                # transposes chained on eltwise temps keep the PE active
                # through the tail (HAM stays at K=8/8).
                r_t = tp.tile([BE, 512], BF16, tag="r")
                z_t = tp.tile([BE, 512], BF16, tag="z")
                n_t = tp.tile([BE, 512], BF16, tag="n")
                rhn = tp.tile([BE, 512], BF16, tag="rhn")
                npre = tp.tile([BE, 512], BF16, tag="npre")
                omz = tp.tile([BE, 512], BF16, tag="omz")
                b_t = tp.tile([BE, 512], BF16, tag="b")
                a_t = tp.tile([BE, 512], BF16, tag="a")
                h_new = st.tile([BE, H], BF16, tag="h")
                ptr = pTR.tile([128, 512], BF16, tag="TR")
                hT_new = st.tile([128, H], BF16, tag="hT")

                sl = [slice(0, 256), slice(256, 512)]
                nc.scalar.activation(r_t[:], ga[:], AF.Sigmoid)
                nc.scalar.activation(z_t[:], gb[:], AF.Sigmoid)
                nc.vector.tensor_scalar(omz[:], z_t[:], -1.0, 1.0,
                                        OP.mult, OP.add)
                nc.gpsimd.tensor_mul(b_t[:, sl[0]], z_t[:, sl[0]],
                                     h_bh[:, sl[0]])
                nc.gpsimd.tensor_mul(b_t[:, sl[1]], z_t[:, sl[1]],
                                     h_bh[:, sl[1]])
                for s in range(2):
                    nc.vector.tensor_mul(rhn[:, sl[s]], r_t[:, sl[s]],
                                         gd[:, sl[s]])
                    nc.vector.tensor_add(npre[:, sl[s]], rhn[:, sl[s]],
                                         gc[:, sl[s]])
                    nc.scalar.activation(n_t[:, sl[s]], npre[:, sl[s]],
                                         AF.Tanh)
                if not last:
                    # HAM fillers: cheap transposes gated on eltwise temps
                    nc.tensor.transpose(ptr[:, 0:128], r_t[:, 0:128],
                                        idb[:])
                    nc.tensor.transpose(ptr[:, 128:256], npre[:, 0:128],
                                        idb[:])
                    nc.tensor.transpose(ptr[:, 256:384], n_t[:, 0:128],
                                        idb[:])
                for s in range(2):
                    nc.vector.tensor_mul(a_t[:, sl[s]], omz[:, sl[s]],
                                         n_t[:, sl[s]])
                    nc.vector.tensor_add(h_new[:, sl[s]], a_t[:, sl[s]],
                                         b_t[:, sl[s]])
                    if not last:
                        for c in (2 * s, 2 * s + 1):
                            nc.tensor.transpose(
                                ptr[:, 128 * c:128 * (c + 1)],
                                h_new[:, 128 * c:128 * (c + 1)], idb[:])
                        if s == 0:
                            nc.scalar.copy(hT_new[:, sl[0]], ptr[:, sl[0]])
                        else:
                            nc.vector.tensor_copy(hT_new[:, sl[1]],
                                                  ptr[:, sl[1]])
                if not last:
                    enc_bias_mm(gd_n)
                    hT = hT_new
                    ga, gb, gc, gd = ga_n, gb_n, gc_n, gd_n
                h_bh = h_new

            # ---------- reshard: AllToAll of 16-row slices (bf16) ----------
            nc.sync.dma_start(cc_in[:], h_bh[:])
            nc.gpsimd.collective_compute(
                "AllToAll", OP.bypass,
                replica_groups=[list(range(NCORE))],
                ins=[cc_in[:]], outs=[cc_out[:]])

            pxa = md.tile([BD, H], BF16, tag="pA")
            pxb = md.tile([BD, H], BF16, tag="pB")
            pya = md.tile([BD, H], BF16, tag="pA")
            pyb = md.tile([BD, H], BF16, tag="pB")
            nc.sync.dma_start(pxa[0:16, :], cc_out[0][:])
            nc.sync.dma_start(pxa[16:32, :], cc_out[1][:])
            nc.sync.dma_start(pxb[0:16, :], cc_out[2][:])
            nc.sync.dma_start(pxb[16:32, :], cc_out[3][:])
            nc.sync.dma_start(pya[0:16, :], cc_out[4][:])
            nc.sync.dma_start(pya[16:32, :], cc_out[5][:])
            nc.sync.dma_start(pyb[0:16, :], cc_out[6][:])
            nc.sync.dma_start(pyb[16:32, :], cc_out[7][:])
            hx = md.tile([BD, H], F32, tag="hx")
            hy = md.tile([BD, H], F32, tag="hy")
            nc.vector.tensor_add(hx[:], pxa[:], pxb[:])
            nc.vector.tensor_add(hy[:], pya[:], pyb[:])

            def trsp_b(src, cols, tag):
                """src [BD, cols] f32 -> bf16 [128, (cols//128)*BD] via PE."""
                nch = cols // 128
                p = pTR.tile([128, 512], F32, tag="TR")
                for c in range(nch):
                    nc.tensor.transpose(p[:, BD * c:BD * (c + 1)],
                                        src[:, 128 * c:128 * (c + 1)],
                                        idf[0:32, 0:32])
                o = md.tile([128, nch * BD], BF16, tag=tag)
                nc.scalar.copy(o[:], p[:, 0:nch * BD])
                return o

            hxT = trsp_b(hx, H, "hxT")
            hyT = trsp_b(hy, H, "hyT")

            # ---- middle MLP (batch-major, activations stationary) ----
            m1a = pA.tile([BD, 512], F32, tag="A")
            m1b = pB.tile([BD, 512], F32, tag="B")
            for c in range(8):
                wt = em1_b[c]
                s = (hxT if c < 4 else hyT)[:, BD * (c % 4):BD * (c % 4 + 1)]
                nc.tensor.matmul(m1a[:], s, wt[:, 0:512],
                                 start=(c == 0), stop=False)
                nc.tensor.matmul(m1b[:], s, wt[:, 512:1024],
                                 start=(c == 0), stop=False)
            nc.tensor.matmul(m1a[:], ones_b[0:1, 0:BD],
                             bias_b[0:1, 1536:2048], start=False, stop=True)
            nc.tensor.matmul(m1b[:], ones_b[0:1, 0:BD],
                             bias_b[0:1, 2048:2560], start=False, stop=True)
            hm1 = md.tile([BD, M1], F32, tag="hm1")
            nc.scalar.activation(hm1[:, 0:512], m1a[:], AF.Relu)
            nc.scalar.activation(hm1[:, 512:1024], m1b[:], AF.Relu)
            hm1T = trsp_b(hm1, M1, "hm1T_m")

            m2 = pC.tile([BD, M2], F32, tag="C")
            for c in range(8):
                nc.tensor.matmul(m2[:], hm1T[:, BD * c:BD * (c + 1)],
                                 em2_b[c][:], start=(c == 0), stop=False)
            nc.tensor.matmul(m2[:], ones_b[0:1, 0:BD], bias_b[0:1, 2560:3072],
                             start=False, stop=True)
            hm2 = md.tile([BD, M2], F32, tag="hm2")
            nc.scalar.activation(hm2[:], m2[:], AF.Relu)
            hm2T = trsp_b(hm2, M2, "hm2T_m")

            zp = pD.tile([BD, 512], F32, tag="D")
            for c in range(4):
                nc.tensor.matmul(zp[:, 0:H], hm2T[:, BD * c:BD * (c + 1)],
                                 eo_b[c][:], start=(c == 0), stop=False)
            nc.tensor.matmul(zp[:, 0:H], ones_b[0:1, 0:BD],
                             bias_b[0:1, 3072:3584], start=False, stop=True)
            z_sb = md.tile([BD, H], F32, tag="z_sb")
            nc.scalar.copy(z_sb[:], zp[:, 0:H])
            zT = trsp_b(z_sb, H, "zT")

            # const = cat(h_x, z) @ d_Wih[:, :2H].T + d_bih + d_bhh(r,z)
            cpa = pA.tile([BD, 512], F32, tag="A")
            cpb = pB.tile([BD, 512], F32, tag="B")
            cpn = pC.tile([BD, 512], F32, tag="C")
            for c in range(8):
                wt = dcw_b[c]
                s = (hxT if c < 4 else zT)[:, BD * (c % 4):BD * (c % 4 + 1)]
                nc.tensor.matmul(cpa[:], s, wt[:, 0:512],
                                 start=(c == 0), stop=False)
                nc.tensor.matmul(cpb[:], s, wt[:, 512:1024],
                                 start=(c == 0), stop=False)
                nc.tensor.matmul(cpn[:], s, wt[:, 1024:1536],
                                 start=(c == 0), stop=False)
            nc.tensor.matmul(cpa[:], ones_b[0:1, 0:BD],
                             bias_b[0:1, 0:512], start=False, stop=True)
            nc.tensor.matmul(cpb[:], ones_b[0:1, 0:BD],
                             bias_b[0:1, 512:1024], start=False, stop=True)
            nc.tensor.matmul(cpn[:], ones_b[0:1, 0:BD],
                             bias_b[0:1, 1024:1536], start=False, stop=True)
            nc.vector.tensor_copy(ycw[64:96, 0:512], cpa[:])
            nc.vector.tensor_copy(ycw[64:96, 512:1024], cpb[:])
            nc.vector.tensor_copy(ycw[64:96, 1024:1536], cpn[:])

            # =======================================================
            # Decode loop: fully transposed, h as [128, 4*32] bf16.
            # =======================================================
            hbf = st.tile([128, 4 * BD], BF16, tag="hbf")
            nc.gpsimd.memset(hbf[:], 0.0)

            def dec_whh(rz, hgn, hbf_src):
                for j in range(8):
                    for k in range(4):
                        nc.tensor.matmul(
                            rz[:, BD * j:BD * (j + 1)],
                            dwhh_b[k][:, 128 * j:128 * (j + 1)],
                            hbf_src[:, BD * k:BD * (k + 1)],
                            start=(j == 0 and k == 0), stop=False)
                for j in range(4):
                    for k in range(4):
                        nc.tensor.matmul(
                            hgn[:, BD * j:BD * (j + 1)],
                            dwhh_b[k][:, 128 * (8 + j):128 * (9 + j)],
                            hbf_src[:, BD * k:BD * (k + 1)],
                            start=(j == 0 and k == 0), stop=False)
                    nc.tensor.matmul(hgn[:, BD * j:BD * (j + 1)],
                                     dbhhn_s[0:1, 128 * j:128 * (j + 1)],
                                     ones_b[0:1, 0:BD],
                                     start=False, stop=(j == 3))

            def dec_m1bias(m1):
                for j in range(8):
                    nc.tensor.matmul(m1[:, BD * j:BD * (j + 1)],
                                     dm1b_s[0:1, 128 * j:128 * (j + 1)],
                                     ones_b[0:1, 0:BD],
                                     start=(j == 0), stop=False)

            def dec_m2bias(m2d):
                for j in range(4):
                    nc.tensor.matmul(m2d[:, BD * j:BD * (j + 1)],
                                     dm2b_s[0:1, 128 * j:128 * (j + 1)],
                                     ones_b[0:1, 0:BD],
                                     start=(j == 0), stop=False)

            rz = pA.tile([128, 512], F32, tag="A")
            hgn = pB.tile([128, 512], F32, tag="B")
            m1 = pD.tile([128, 512], F32, tag="D")
            m2d = pTR.tile([128, 512], F32, tag="TR")
            dec_m1bias(m1)
            dec_m2bias(m2d)
            # t=0: h=0, so no Whh matmuls; hgn(0) = bias only.
            for j in range(4):
                nc.tensor.matmul(hgn[:, BD * j:BD * (j + 1)],
                                 dbhhn_s[0:1, 128 * j:128 * (j + 1)],
                                 ones_b[0:1, 0:BD],
                                 start=(j == 0), stop=(j == 3))

            for t in range(hor):
                lastd = (t == hor - 1)
                # ---- y/const-side gate matmuls ----
                an = pC.tile([128, 512], F32, tag="C")
                for j in range(8):
                    nc.tensor.matmul(rz[:, BD * j:BD * (j + 1)],
                                     ycw[:, 128 * j:128 * (j + 1)], ypc[:],
                                     start=(t == 0 and j == 0),
                                     stop=(j == 7))
                for j in range(4):
                    nc.tensor.matmul(an[:, BD * j:BD * (j + 1)],
                                     ycw[:, 128 * (8 + j):128 * (9 + j)],
                                     ypc[:], start=(j == 0), stop=(j == 3))

                # ---- GRU eltwise, transposed layout [128, 4*32]; tail in
                # chunk-pair halves so M1 starts on half 0 early ----
                r_t = tp.tile([128, 4 * BD], BF16, tag="dr")
                nc.scalar.activation(r_t[:], rz[:, 0:4 * BD], AF.Sigmoid)
                z_t = tp.tile([128, 4 * BD], BF16, tag="dz")
                nc.scalar.activation(z_t[:], rz[:, 4 * BD:8 * BD], AF.Sigmoid)
                omz = tp.tile([128, 4 * BD], BF16, tag="domz")
                nc.vector.tensor_scalar(omz[:], z_t[:], -1.0, 1.0,
                                        OP.mult, OP.add)
                u_t = tp.tile([128, 4 * BD], BF16, tag="du")
                nc.gpsimd.tensor_mul(u_t[:], z_t[:], hbf[:])
                t2 = tp.tile([128, 4 * BD], BF16, tag="dt2")
                npre = tp.tile([128, 4 * BD], BF16, tag="dnp")
                n_t = tp.tile([128, 4 * BD], BF16, tag="dn")
                a_t = tp.tile([128, 4 * BD], BF16, tag="da")
                hbf_n = st.tile([128, 4 * BD], BF16, tag="hbf")
                for s in range(2):
                    dsl = slice(2 * BD * s, 2 * BD * (s + 1))
                    nc.vector.tensor_mul(t2[:, dsl], r_t[:, dsl],
                                         hgn[:, dsl])
                    nc.vector.tensor_add(npre[:, dsl], t2[:, dsl],
                                         an[:, dsl])
                    nc.scalar.activation(n_t[:, dsl], npre[:, dsl], AF.Tanh)
                    nc.vector.tensor_mul(a_t[:, dsl], omz[:, dsl],
                                         n_t[:, dsl])
                    nc.vector.tensor_add(hbf_n[:, dsl], a_t[:, dsl],
                                         u_t[:, dsl])
                    # M1 on the two ready h chunks
                    for k in (2 * s, 2 * s + 1):
                        for j in range(8):
                            nc.tensor.matmul(
                                m1[:, BD * j:BD * (j + 1)],
                                dm1_b[k][:, 128 * j:128 * (j + 1)],
                                hbf_n[:, BD * k:BD * (k + 1)],
                                start=False, stop=(k == 3 and j == 7))
                hbf = hbf_n
                hm1_s = tp.tile([128, 8 * BD], BF16, tag="dhm1")
                nc.scalar.activation(hm1_s[:], m1[:, 0:8 * BD], AF.Relu)

                # ---- M2 ----
                for k in range(8):
                    for j in range(4):
                        nc.tensor.matmul(m2d[:, BD * j:BD * (j + 1)],
                                         dm2_b[k][:, 128 * j:128 * (j + 1)],
                                         hm1_s[:, BD * k:BD * (k + 1)],
                                         start=False,
                                         stop=(k == 7 and j == 3))
                hm2_s = tp.tile([128, 4 * BD], BF16, tag="dhm2")
                nc.scalar.activation(hm2_s[:], m2d[:, 0:4 * BD], AF.Relu)

                # bias pre-issue for t+1 (fills the out-matmul wait)
                if not lastd:
                    m1_n = pD.tile([128, 512], F32, tag="D")
                    dec_m1bias(m1_n)
                    m2_n = pTR.tile([128, 512], F32, tag="TR")
                    dec_m2bias(m2_n)

                # ---- output head: y [64, 32] ----
                yb = pC.tile([128, 512], F32, tag="C")
                for k in range(4):
                    nc.tensor.matmul(yb[0:NY, 0:BD],
                                     dow_b[k][:, 0:NY],
                                     hm2_s[:, BD * k:BD * (k + 1)],
                                     start=(k == 0), stop=(k == 3))
                if not lastd:
                    # critical path: feed y back (bf16) before the f32 copy
                    nc.scalar.activation(ypc[0:NY, :], yb[0:NY, 0:BD],
                                         AF.Identity, bias=dob_c[:])
                y_f = tp.tile([NY, BD], F32, tag="dy")
                nc.scalar.activation(y_f[:], yb[0:NY, 0:BD], AF.Identity,
                                     bias=dob_c[:])
                nc.sync.dma_start(d_out[NY * t:NY * (t + 1), :], y_f[:])
                if not lastd:
                    # pre-issue next step's h-side matmuls
                    rz_n = pA.tile([128, 512], F32, tag="A")
                    hgn_n = pB.tile([128, 512], F32, tag="B")
                    dec_whh(rz_n, hgn_n, hbf)
                    rz, hgn, m1, m2d = rz_n, hgn_n, m1_n, m2_n

    nc.compile()
    return nc


# ---------------------------------------------------------------------------
# Host-side sharding
# ---------------------------------------------------------------------------

def shard_inputs(inp, et=100, hor=60):
    f32 = np.float32

    def bf(a):
        return np.ascontiguousarray(np.asarray(a, f32).astype(BF))

    x, y = np.asarray(inp["x"], f32), np.asarray(inp["y"], f32)
    chains = [("xf", False, x), ("xb", True, x),
              ("ef", False, y), ("eb", True, y)]
    in_maps = []
    shared = {}

    def wih_aug(pre):
        wih = np.asarray(inp[pre + "_Wih"], f32)
        bih = np.asarray(inp[pre + "_bih"], f32)
        bhh = np.asarray(inp[pre + "_bhh"], f32)
        aug = np.zeros((66, G), f32)
        aug[0:64, :] = wih.T
        bias = bih.copy()
        bias[0:2 * H] += bhh[0:2 * H]
        aug[64, :] = bias
        aug[65, H:2 * H] = BIG
        return bf(aug)

    d_Wih = np.asarray(inp["d_Wih"], f32)
    d_bih = np.asarray(inp["d_bih"], f32)
    d_bhh = np.asarray(inp["d_bhh"], f32)
    dc_b = d_bih.copy()
    dc_b[0:2 * H] += d_bhh[0:2 * H]

    shared["em_w1t"] = bf(np.asarray(inp["em_W1"], f32).T)
    shared["em_w2t"] = bf(np.asarray(inp["em_W2"], f32).T)
    shared["eo_wt"] = bf(np.asarray(inp["eo_W"], f32).T)
    shared["dc_wt"] = bf(d_Wih[:, 0:2 * H].T)
    midb = np.concatenate([dc_b, np.asarray(inp["em_b1"], f32),
                           np.asarray(inp["em_b2"], f32),
                           np.asarray(inp["eo_b"], f32)])[None, :]
    shared["mid_bias"] = bf(midb)
    shared["dwy_t"] = bf(d_Wih[:, 2 * H:].T)
    shared["dwhh_t"] = bf(np.asarray(inp["d_Whh"], f32).T)
    shared["dbhhn_row"] = bf(d_bhh[None, 2 * H:])
    shared["dm_w1t"] = bf(np.asarray(inp["dm_W1"], f32).T)
    shared["dm_b1row"] = bf(np.asarray(inp["dm_b1"], f32)[None, :])
    shared["dm_w2t"] = bf(np.asarray(inp["dm_W2"], f32).T)
    shared["dm_b2row"] = bf(np.asarray(inp["dm_b2"], f32)[None, :])
    shared["do_wt"] = bf(np.asarray(inp["do_W"], f32).T)
    shared["do_bcol"] = np.ascontiguousarray(
        np.asarray(inp["do_b"], f32)[:, None])

    for j in range(NCORE):
        chain, half = j // 2, j % 2
        pre, rev, seq = chains[chain]
        T = seq.shape[1]
        s = seq[128 * half:128 * (half + 1)]          # [128, T, 64]
        xin = np.zeros((66, et, BE), f32)
        xin[64, :, :] = 1.0
        pad = et - T
        if pad:
            xin[65, 0:pad, :] = 1.0
        order = np.arange(T)[::-1] if rev else np.arange(T)
        xin[0:64, pad:, :] = s[:, order, :].transpose(2, 1, 0)
        m = dict(shared)
        m["xin"] = bf(xin.reshape(66, et * BE))
        m["wih_aug"] = wih_aug(pre)
        m["whh_t"] = bf(np.asarray(inp[pre + "_Whh"], f32).T)
        m["bhhn_row"] = bf(np.asarray(inp[pre + "_bhh"], f32)[None, 2 * H:])
        xl = np.concatenate([x[16 * j:16 * j + 16, -1, :],
                             x[128 + 16 * j:128 + 16 * j + 16, -1, :]])
        m["xlast_t"] = bf(xl.T)
        in_maps.append(m)
    return in_maps


def unshard(results, hor=60):
    out = np.zeros((B, hor, NY), np.float32)
    for j in range(NCORE):
        o = results[j]["out"].reshape(hor, NY, BD).transpose(2, 0, 1)
        out[16 * j:16 * j + 16] = o[0:16]
        out[128 + 16 * j:128 + 16 * j + 16] = o[16:32]
    return out


_NC = None


def kernel(**inputs):
    global _NC
    from concourse.bass_utils import run_bass_kernel_spmd
    if _NC is None:
        _NC = build_nc()
    in_maps = shard_inputs(inputs)
    res = run_bass_kernel_spmd(_NC, in_maps, core_ids=list(range(NCORE)))
    return unshard(res.results)


# revision 35
# speedup vs baseline: 2.0530x; 1.0087x over previous
"""Trainium2 Bass kernel for the GRU autoencoder (v4).

Distribution (8 NeuronCores):
  Encode : chain-parallel x batch-parallel. Core j handles GRU chain j//2
           (xf, xb, ef, eb) on batch half j%2 (128 rows), uniform 100-step
           loop; the 50-step x-chains get 50 exact identity steps (z forced
           to 1 via a +BIG flag row). AllToAll reshards 16-row slices so each
           core decodes global rows [16j:16j+16] u [128+16j:+16].

v4: everything bf16 end-to-end (weights shipped as bf16 from the host — no
device-side casting), encoder gate matmuls split into N=256 halves so the
eltwise/transpose tail software-pipelines across halves, decoder fully
transposed (weights-stationary, [feature, batch] layout, zero transposes).
"""

import sys

sys.path.insert(0, "/opt/trn_rl_repo")

import ml_dtypes
import numpy as np

import concourse.bass as bass
import concourse.mybir as mybir
import concourse.tile as tile
from concourse import bacc
from concourse.masks import make_identity

dt = mybir.dt
AF = mybir.ActivationFunctionType
OP = mybir.AluOpType

B, TX, TY, NX, NY, H, HOR = 256, 50, 100, 64, 64, 512, 60
M1, M2 = 1024, 512
G = 3 * H
NCORE = 8
BE = 128   # encoder batch rows per core
BD = 32    # decoder batch rows per core
BIG = 30000.0

F32, BF16 = dt.float32, dt.bfloat16
BF = ml_dtypes.bfloat16


def build_nc(et=100, hor=60):
    nc = bacc.Bacc("TRN2", target_bir_lowering=False, debug=False,
                   num_devices=NCORE)

    # ---- DRAM parameters (bf16 except the ACT bias column) ----
    d_xin = nc.dram_tensor("xin", [66, et * BE], BF16, kind="ExternalInput")
    d_wih = nc.dram_tensor("wih_aug", [66, G], BF16, kind="ExternalInput")
    d_whh = nc.dram_tensor("whh_t", [H, G], BF16, kind="ExternalInput")
    d_bhhn = nc.dram_tensor("bhhn_row", [1, H], BF16, kind="ExternalInput")

    d_em1 = nc.dram_tensor("em_w1t", [2 * H, M1], BF16, kind="ExternalInput")
    d_em2 = nc.dram_tensor("em_w2t", [M1, M2], BF16, kind="ExternalInput")
    d_eow = nc.dram_tensor("eo_wt", [M2, H], BF16, kind="ExternalInput")
    d_dcw = nc.dram_tensor("dc_wt", [2 * H, G], BF16, kind="ExternalInput")
    d_midb = nc.dram_tensor("mid_bias", [1, 3584], BF16,
                            kind="ExternalInput")

    d_dwy = nc.dram_tensor("dwy_t", [NY, G], BF16, kind="ExternalInput")
    d_dwhh = nc.dram_tensor("dwhh_t", [H, G], BF16, kind="ExternalInput")
    d_dbhhn = nc.dram_tensor("dbhhn_row", [1, H], BF16, kind="ExternalInput")
    d_dm1 = nc.dram_tensor("dm_w1t", [H, M1], BF16, kind="ExternalInput")
    d_dm1b = nc.dram_tensor("dm_b1row", [1, M1], BF16, kind="ExternalInput")
    d_dm2 = nc.dram_tensor("dm_w2t", [M1, M2], BF16, kind="ExternalInput")
    d_dm2b = nc.dram_tensor("dm_b2row", [1, M2], BF16, kind="ExternalInput")
    d_dow = nc.dram_tensor("do_wt", [M2, NY], BF16, kind="ExternalInput")
    d_dobr = nc.dram_tensor("do_brow", [1, NY], BF16, kind="ExternalInput")
    d_xlast = nc.dram_tensor("xlast_t", [NX, BD], BF16, kind="ExternalInput")

    d_out = nc.dram_tensor("out", [hor * NY, BD], F32, kind="ExternalOutput")

    cc_in = nc.dram_tensor("cc_in", [BE, H], BF16)
    cc_out = nc.dram_tensor("cc_out", [NCORE, 16, H], BF16)

    with tile.TileContext(nc) as tc:
        with tc.tile_pool(name="pe", bufs=1) as pe, \
             tc.tile_pool(name="wts", bufs=1) as wts, \
             tc.tile_pool(name="xsp", bufs=2) as xsp, \
             tc.tile_pool(name="st", bufs=2) as st, \
             tc.tile_pool(name="tp", bufs=2) as tp, \
             tc.tile_pool(name="md", bufs=1) as md, \
             tc.tile_pool(name="pA", bufs=2, space="PSUM") as pA, \
             tc.tile_pool(name="pB", bufs=2, space="PSUM") as pB, \
             tc.tile_pool(name="pC", bufs=2, space="PSUM") as pC, \
             tc.tile_pool(name="pD", bufs=1, space="PSUM") as pD, \
             tc.tile_pool(name="pTR", bufs=1, space="PSUM") as pTR:

            # ---------- constants ----------
            idf = pe.tile([128, 128], F32, tag="idf")
            make_identity(nc, idf[:])
            idb = pe.tile([128, 128], BF16, tag="idb")
            nc.gpsimd.tensor_copy(idb[:], idf[:])
            ones_b = pe.tile([1, 128], BF16, tag="ones_b")
            nc.gpsimd.memset(ones_b[:], 1.0)
            zero_b = pe.tile([128, 512], BF16, tag="zero_b")
            nc.gpsimd.memset(zero_b[:], 0.0)

            def load_direct(pool, dram_ap, rows, cols, tag):
                r = pool.tile([rows, cols], BF16, tag=tag)
                nc.sync.dma_start(r[:], dram_ap)
                return r

            # Middle/decoder weights: allocate now, DMA lazily inside the
            # encode loop (one tile per step) so the startup xin load isn't
            # queued behind ~6MB of weight traffic.
            wload = []

            def load_lazy(dram_ap, rows, cols, tag, rdt=BF16):
                r = wts.tile([rows, cols], rdt, tag=tag)
                wload.append((r, dram_ap))
                return r

            # ---------- encoder weights (needed immediately) ----------
            wih_b = load_direct(wts, d_wih[:], 66, G, "wih")
            whh_b = [load_direct(wts, d_whh[128 * c:128 * (c + 1), :],
                                 128, G, f"whh{c}") for c in range(4)]
            ebhhn = load_direct(wts, d_bhhn[:], 1, H, "ebhhn")

            # ---------- encoder state ----------
            hT = pe.tile([128, H], BF16, tag="hT0")       # [feat%128, 4x128b]
            nc.vector.tensor_copy(hT[:], zero_b[:])
            h_bh = pe.tile([BE, H], BF16, tag="h0")       # [batch, feat]
            nc.gpsimd.memset(h_bh[:], 0.0)

            # ---------- middle + decoder weights (lazy bf16 DMA) ----------
            em1_b = [load_lazy(d_em1[128 * c:128 * (c + 1), :],
                               128, M1, f"em1_{c}") for c in range(8)]
            em2_b = [load_lazy(d_em2[128 * c:128 * (c + 1), :],
                               128, M2, f"em2_{c}") for c in range(8)]
            eo_b = [load_lazy(d_eow[128 * c:128 * (c + 1), :],
                              128, H, f"eo_{c}") for c in range(4)]
            dcw_b = [load_lazy(d_dcw[128 * c:128 * (c + 1), :],
                               128, G, f"dcw_{c}") for c in range(8)]
            bias_b = pe.tile([1, 3584], BF16, tag="bias_b")
            wload.append((bias_b, d_midb[:]))

            dwhh_b = [load_lazy(d_dwhh[128 * c:128 * (c + 1), :],
                                128, G, f"dwhh{c}") for c in range(4)]
            dm1_b = [load_lazy(d_dm1[128 * c:128 * (c + 1), :],
                               128, M1, f"dm1_{c}") for c in range(4)]
            dm2_b = [load_lazy(d_dm2[128 * c:128 * (c + 1), :],
                               128, M2, f"dm2_{c}") for c in range(8)]
            dow_b = [load_lazy(d_dow[128 * c:128 * (c + 1), :],
                               128, NY, f"dow_{c}") for c in range(4)]
            dbhhn_s = load_lazy(d_dbhhn[:], 1, H, "dbhhn")
            dm1b_s = load_lazy(d_dm1b[:], 1, M1, "dm1b")
            dm2b_s = load_lazy(d_dm2b[:], 1, M2, "dm2b")
            dob_r = load_direct(wts, d_dobr[:], 1, NY, "dobr")
            # ycw: rows 0:64 = Wy^T, rows 64:96 = const (filled post-middle).
            ycw = pe.tile([96, G], BF16, tag="ycw")
            wload.append((ycw[0:NY, :], d_dwy[:]))
            # ypc: rows 0:64 = y_t, rows 64:96 = I32 (selects const rows).
            ypc = pe.tile([96, BD], BF16, tag="ypc")
            nc.sync.dma_start(ypc[0:NX, :], d_xlast[:])
            nc.gpsimd.tensor_copy(ypc[64:96, :], idb[0:32, 0:32])

            # =======================================================
            # Encode loop, software-pipelined in feature halves.
            # =======================================================
            def enc_alloc():
                ga = pA.tile([BE, 512], F32, tag="A")
                gb = pB.tile([BE, 512], F32, tag="B")
                gc = pC.tile([BE, 512], F32, tag="C")
                gd = pD.tile([BE, 512], F32, tag="D")
                return ga, gb, gc, gd

            def enc_xs_mms(xs, ga, gb, gc):
                nc.tensor.matmul(ga[:], xs[:], wih_b[:, 0:512],
                                 start=True, stop=False)
                nc.tensor.matmul(gb[:], xs[:], wih_b[:, 512:1024],
                                 start=True, stop=False)
                nc.tensor.matmul(gc[:], xs[:], wih_b[:, 1024:1536],
                                 start=True, stop=True)

            def enc_bias_mm(gd):
                nc.tensor.matmul(gd[:], ones_b[0:1, 0:BE], ebhhn[:],
                                 start=True, stop=False)

            def load_xs(t):
                xb = xsp.tile([66, 128], BF16, tag="xs_b")
                nc.sync.dma_start(xb[:], d_xin[:, t * BE:(t + 1) * BE])
                return xb

            xs = load_xs(0)
            ga, gb, gc, gd = enc_alloc()
            enc_xs_mms(xs, ga, gb, gc)
            enc_bias_mm(gd)

            for t in range(et):
                last = (t == et - 1)
                # h-side matmuls, bank-major: r-gates, z-gates, n-h-gates
                # (z early so b=z*h and omz are off the critical path).
                for c in range(4):
                    nc.tensor.matmul(ga[:], hT[:, 128 * c:128 * (c + 1)],
                                     whh_b[c][:, 0:512],
                                     start=False, stop=(c == 3))
                for c in range(4):
                    nc.tensor.matmul(gb[:], hT[:, 128 * c:128 * (c + 1)],
                                     whh_b[c][:, 512:1024],
                                     start=False, stop=(c == 3))
                for c in range(4):
                    nc.tensor.matmul(gd[:], hT[:, 128 * c:128 * (c + 1)],
                                     whh_b[c][:, 1024:1536],
                                     start=False, stop=(c == 3))
                if not last:
                    xs_n = load_xs(t + 1)
                    if t < len(wload):
                        wa, wd = wload[t]
                        nc.sync.dma_start(wa[:], wd)
                    ga_n, gb_n, gc_n, gd_n = enc_alloc()

                # ---- eltwise: h' = (1-z)*n + z*h; rhn/npre and the tail
                # run in feature halves so tanh/hT chunks land early; dummy
                # transposes chained on eltwise temps keep the PE active
                # through the tail (HAM stays at K=8/8).
                r_t = tp.tile([BE, 512], BF16, tag="r")
                z_t = tp.tile([BE, 512], BF16, tag="z")
                n_t = tp.tile([BE, 512], BF16, tag="n")
                rhn = tp.tile([BE, 512], BF16, tag="rhn")
                npre = tp.tile([BE, 512], BF16, tag="npre")
                omz = tp.tile([BE, 512], BF16, tag="omz")
                b_t = tp.tile([BE, 512], BF16, tag="b")
                a_t = tp.tile([BE, 512], BF16, tag="a")
                h_new = st.tile([BE, H], BF16, tag="h")
                ptr = pTR.tile([128, 512], BF16, tag="TR")
                hT_new = st.tile([128, H], BF16, tag="hT")

                sl = [slice(0, 256), slice(256, 512)]
                nc.scalar.activation(r_t[:], ga[:], AF.Sigmoid)
                nc.scalar.activation(z_t[:], gb[:], AF.Sigmoid)
                nc.vector.tensor_scalar(omz[:], z_t[:], -1.0, 1.0,
                                        OP.mult, OP.add)
                nc.gpsimd.tensor_mul(b_t[:, sl[0]], z_t[:, sl[0]],
                                     h_bh[:, sl[0]])
                nc.gpsimd.tensor_mul(b_t[:, sl[1]], z_t[:, sl[1]],
                                     h_bh[:, sl[1]])
                for s in range(2):
                    nc.vector.tensor_mul(rhn[:, sl[s]], r_t[:, sl[s]],
                                         gd[:, sl[s]])
                    nc.vector.tensor_add(npre[:, sl[s]], rhn[:, sl[s]],
                                         gc[:, sl[s]])
                    nc.scalar.activation(n_t[:, sl[s]], npre[:, sl[s]],
                                         AF.Tanh)
                if not last:
                    # Interleave the (data-independent) next-step xs matmuls
                    # with filler transposes gated on eltwise temps so the
                    # PE keeps a high duty cycle through the tail and HAM
                    # stays at K=8/8 into the next gate burst.
                    nc.tensor.transpose(ptr[:, 0:128], r_t[:, 0:128],
                                        idb[:])
                    nc.tensor.transpose(ptr[:, 128:256], r_t[:, 128:256],
                                        idb[:])
                    nc.tensor.matmul(ga_n[:], xs_n[:], wih_b[:, 0:512],
                                     start=True, stop=False)
                    nc.tensor.transpose(ptr[:, 256:384], npre[:, 0:128],
                                        idb[:])
                    nc.tensor.transpose(ptr[:, 384:512], npre[:, 128:256],
                                        idb[:])
                    nc.tensor.matmul(gb_n[:], xs_n[:], wih_b[:, 512:1024],
                                     start=True, stop=False)
                    nc.tensor.transpose(ptr[:, 0:128], n_t[:, 0:128],
                                        idb[:])
                    nc.tensor.transpose(ptr[:, 128:256], n_t[:, 128:256],
                                        idb[:])
                    nc.tensor.matmul(gc_n[:], xs_n[:], wih_b[:, 1024:1536],
                                     start=True, stop=True)
                for s in range(2):
                    nc.vector.tensor_mul(a_t[:, sl[s]], omz[:, sl[s]],
                                         n_t[:, sl[s]])
                    nc.vector.tensor_add(h_new[:, sl[s]], a_t[:, sl[s]],
                                         b_t[:, sl[s]])
                    if not last:
                        for c in (2 * s, 2 * s + 1):
                            nc.tensor.transpose(
                                ptr[:, 128 * c:128 * (c + 1)],
                                h_new[:, 128 * c:128 * (c + 1)], idb[:])
                        if s == 0:
                            nc.scalar.copy(hT_new[:, sl[0]], ptr[:, sl[0]])
                        else:
                            nc.vector.tensor_copy(hT_new[:, sl[1]],
                                                  ptr[:, sl[1]])
                if not last:
                    enc_bias_mm(gd_n)
                    hT = hT_new
                    ga, gb, gc, gd = ga_n, gb_n, gc_n, gd_n
                h_bh = h_new

            # ---------- reshard: AllToAll of 16-row slices (bf16) ----------
            nc.sync.dma_start(cc_in[:], h_bh[:])
            nc.gpsimd.collective_compute(
                "AllToAll", OP.bypass,
                replica_groups=[list(range(NCORE))],
                ins=[cc_in[:]], outs=[cc_out[:]])

            pxa = md.tile([BD, H], BF16, tag="pA")
            pxb = md.tile([BD, H], BF16, tag="pB")
            pya = md.tile([BD, H], BF16, tag="pA")
            pyb = md.tile([BD, H], BF16, tag="pB")
            nc.sync.dma_start(pxa[0:16, :], cc_out[0][:])
            nc.sync.dma_start(pxa[16:32, :], cc_out[1][:])
            nc.sync.dma_start(pxb[0:16, :], cc_out[2][:])
            nc.sync.dma_start(pxb[16:32, :], cc_out[3][:])
            nc.sync.dma_start(pya[0:16, :], cc_out[4][:])
            nc.sync.dma_start(pya[16:32, :], cc_out[5][:])
            nc.sync.dma_start(pyb[0:16, :], cc_out[6][:])
            nc.sync.dma_start(pyb[16:32, :], cc_out[7][:])
            # PE warmup off the collective outputs: a dense burst of
            # transposes re-engages HAM before the middle MLP runs.
            pwu = pTR.tile([128, 512], BF16, tag="TR")
            for i, src in enumerate((pxa, pxb, pya, pyb)):
                for c in range(4):
                    nc.tensor.transpose(
                        pwu[:, (4 * i + c) * 32:(4 * i + c + 1) * 32],
                        src[:, 128 * c:128 * (c + 1)], idb[0:32, 0:32])
            hx = md.tile([BD, H], F32, tag="hx")
            hy = md.tile([BD, H], F32, tag="hy")
            nc.vector.tensor_add(hx[:], pxa[:], pxb[:])
            nc.vector.tensor_add(hy[:], pya[:], pyb[:])

            def trsp_b(src, cols, tag):
                """src [BD, cols] f32 -> bf16 [128, (cols//128)*BD] via PE."""
                nch = cols // 128
                p = pTR.tile([128, 512], F32, tag="TR")
                for c in range(nch):
                    nc.tensor.transpose(p[:, BD * c:BD * (c + 1)],
                                        src[:, 128 * c:128 * (c + 1)],
                                        idf[0:32, 0:32])
                o = md.tile([128, nch * BD], BF16, tag=tag)
                nc.scalar.copy(o[:], p[:, 0:nch * BD])
                return o

            hxT = trsp_b(hx, H, "hxT")
            hyT = trsp_b(hy, H, "hyT")

            # ---- middle MLP (batch-major, activations stationary) ----
            m1a = pA.tile([BD, 512], F32, tag="A")
            m1b = pB.tile([BD, 512], F32, tag="B")
            for c in range(8):
                wt = em1_b[c]
                s = (hxT if c < 4 else hyT)[:, BD * (c % 4):BD * (c % 4 + 1)]
                nc.tensor.matmul(m1a[:], s, wt[:, 0:512],
                                 start=(c == 0), stop=False)
                nc.tensor.matmul(m1b[:], s, wt[:, 512:1024],
                                 start=(c == 0), stop=False)
            nc.tensor.matmul(m1a[:], ones_b[0:1, 0:BD],
                             bias_b[0:1, 1536:2048], start=False, stop=True)
            nc.tensor.matmul(m1b[:], ones_b[0:1, 0:BD],
                             bias_b[0:1, 2048:2560], start=False, stop=True)
            hm1 = md.tile([BD, M1], F32, tag="hm1")
            nc.scalar.activation(hm1[:, 0:512], m1a[:], AF.Relu)
            nc.scalar.activation(hm1[:, 512:1024], m1b[:], AF.Relu)
            hm1T = trsp_b(hm1, M1, "hm1T_m")

            m2 = pC.tile([BD, M2], F32, tag="C")
            for c in range(8):
                nc.tensor.matmul(m2[:], hm1T[:, BD * c:BD * (c + 1)],
                                 em2_b[c][:], start=(c == 0), stop=False)
            nc.tensor.matmul(m2[:], ones_b[0:1, 0:BD], bias_b[0:1, 2560:3072],
                             start=False, stop=True)
            hm2 = md.tile([BD, M2], F32, tag="hm2")
            nc.scalar.activation(hm2[:], m2[:], AF.Relu)
            hm2T = trsp_b(hm2, M2, "hm2T_m")

            zp = pD.tile([BD, 512], F32, tag="D")
            for c in range(4):
                nc.tensor.matmul(zp[:, 0:H], hm2T[:, BD * c:BD * (c + 1)],
                                 eo_b[c][:], start=(c == 0), stop=False)
            nc.tensor.matmul(zp[:, 0:H], ones_b[0:1, 0:BD],
                             bias_b[0:1, 3072:3584], start=False, stop=True)
            z_sb = md.tile([BD, H], F32, tag="z_sb")
            nc.scalar.copy(z_sb[:], zp[:, 0:H])
            zT = trsp_b(z_sb, H, "zT")

            # const = cat(h_x, z) @ d_Wih[:, :2H].T + d_bih + d_bhh(r,z)
            cpa = pA.tile([BD, 512], F32, tag="A")
            cpb = pB.tile([BD, 512], F32, tag="B")
            cpn = pC.tile([BD, 512], F32, tag="C")
            for c in range(8):
                wt = dcw_b[c]
                s = (hxT if c < 4 else zT)[:, BD * (c % 4):BD * (c % 4 + 1)]
                nc.tensor.matmul(cpa[:], s, wt[:, 0:512],
                                 start=(c == 0), stop=False)
                nc.tensor.matmul(cpb[:], s, wt[:, 512:1024],
                                 start=(c == 0), stop=False)
                nc.tensor.matmul(cpn[:], s, wt[:, 1024:1536],
                                 start=(c == 0), stop=False)
            nc.tensor.matmul(cpa[:], ones_b[0:1, 0:BD],
                             bias_b[0:1, 0:512], start=False, stop=True)
            nc.tensor.matmul(cpb[:], ones_b[0:1, 0:BD],
                             bias_b[0:1, 512:1024], start=False, stop=True)
            nc.tensor.matmul(cpn[:], ones_b[0:1, 0:BD],
                             bias_b[0:1, 1024:1536], start=False, stop=True)
            nc.vector.tensor_copy(ycw[64:96, 0:512], cpa[:])
            nc.vector.tensor_copy(ycw[64:96, 512:1024], cpb[:])
            nc.vector.tensor_copy(ycw[64:96, 1024:1536], cpn[:])

            # =======================================================
            # Decode loop: fully transposed, h as [128, 4*32] bf16.
            # =======================================================
            hbf = st.tile([128, 4 * BD], BF16, tag="hbf")
            nc.gpsimd.memset(hbf[:], 0.0)

            def dec_whh(rz, hgn, hbf_src):
                for j in range(8):
                    for k in range(4):
                        nc.tensor.matmul(
                            rz[:, BD * j:BD * (j + 1)],
                            dwhh_b[k][:, 128 * j:128 * (j + 1)],
                            hbf_src[:, BD * k:BD * (k + 1)],
                            start=(j == 0 and k == 0), stop=False)
                for j in range(4):
                    for k in range(4):
                        nc.tensor.matmul(
                            hgn[:, BD * j:BD * (j + 1)],
                            dwhh_b[k][:, 128 * (8 + j):128 * (9 + j)],
                            hbf_src[:, BD * k:BD * (k + 1)],
                            start=(j == 0 and k == 0), stop=False)
                    nc.tensor.matmul(hgn[:, BD * j:BD * (j + 1)],
                                     dbhhn_s[0:1, 128 * j:128 * (j + 1)],
                                     ones_b[0:1, 0:BD],
                                     start=False, stop=(j == 3))

            def dec_m1bias(m1):
                for j in range(8):
                    nc.tensor.matmul(m1[:, BD * j:BD * (j + 1)],
                                     dm1b_s[0:1, 128 * j:128 * (j + 1)],
                                     ones_b[0:1, 0:BD],
                                     start=(j == 0), stop=False)

            def dec_m2bias(m2d):
                for j in range(4):
                    nc.tensor.matmul(m2d[:, BD * j:BD * (j + 1)],
                                     dm2b_s[0:1, 128 * j:128 * (j + 1)],
                                     ones_b[0:1, 0:BD],
                                     start=(j == 0), stop=False)

            rz = pA.tile([128, 512], F32, tag="A")
            hgn = pB.tile([128, 512], F32, tag="B")
            m1 = pD.tile([128, 512], F32, tag="D")
            m2d = pTR.tile([128, 512], F32, tag="TR")
            dec_m1bias(m1)
            dec_m2bias(m2d)
            # t=0: h=0, so no Whh matmuls; hgn(0) = bias only.
            for j in range(4):
                nc.tensor.matmul(hgn[:, BD * j:BD * (j + 1)],
                                 dbhhn_s[0:1, 128 * j:128 * (j + 1)],
                                 ones_b[0:1, 0:BD],
                                 start=(j == 0), stop=(j == 3))

            for t in range(hor):
                lastd = (t == hor - 1)
                # ---- y/const-side gate matmuls ----
                an = pC.tile([128, 512], F32, tag="C")
                for j in range(8):
                    nc.tensor.matmul(rz[:, BD * j:BD * (j + 1)],
                                     ycw[:, 128 * j:128 * (j + 1)], ypc[:],
                                     start=(t == 0 and j == 0),
                                     stop=(j == 7))
                for j in range(4):
                    nc.tensor.matmul(an[:, BD * j:BD * (j + 1)],
                                     ycw[:, 128 * (8 + j):128 * (9 + j)],
                                     ypc[:], start=(j == 0), stop=(j == 3))

                # ---- GRU eltwise, transposed layout [128, 4*32]; tail in
                # chunk-pair halves so M1 starts on half 0 early ----
                r_t = tp.tile([128, 4 * BD], BF16, tag="dr")
                nc.scalar.activation(r_t[:], rz[:, 0:4 * BD], AF.Sigmoid)
                z_t = tp.tile([128, 4 * BD], BF16, tag="dz")
                nc.scalar.activation(z_t[:], rz[:, 4 * BD:8 * BD], AF.Sigmoid)
                omz = tp.tile([128, 4 * BD], BF16, tag="domz")
                nc.vector.tensor_scalar(omz[:], z_t[:], -1.0, 1.0,
                                        OP.mult, OP.add)
                u_t = tp.tile([128, 4 * BD], BF16, tag="du")
                nc.gpsimd.tensor_mul(u_t[:], z_t[:], hbf[:])
                t2 = tp.tile([128, 4 * BD], BF16, tag="dt2")
                npre = tp.tile([128, 4 * BD], BF16, tag="dnp")
                n_t = tp.tile([128, 4 * BD], BF16, tag="dn")
                a_t = tp.tile([128, 4 * BD], BF16, tag="da")
                hbf_n = st.tile([128, 4 * BD], BF16, tag="hbf")
                for s in range(2):
                    dsl = slice(2 * BD * s, 2 * BD * (s + 1))
                    nc.vector.tensor_mul(t2[:, dsl], r_t[:, dsl],
                                         hgn[:, dsl])
                    nc.vector.tensor_add(npre[:, dsl], t2[:, dsl],
                                         an[:, dsl])
                    nc.scalar.activation(n_t[:, dsl], npre[:, dsl], AF.Tanh)
                    nc.vector.tensor_mul(a_t[:, dsl], omz[:, dsl],
                                         n_t[:, dsl])
                    nc.vector.tensor_add(hbf_n[:, dsl], a_t[:, dsl],
                                         u_t[:, dsl])
                    # M1 on the two ready h chunks
                    for k in (2 * s, 2 * s + 1):
                        for j in range(8):
                            nc.tensor.matmul(
                                m1[:, BD * j:BD * (j + 1)],
                                dm1_b[k][:, 128 * j:128 * (j + 1)],
                                hbf_n[:, BD * k:BD * (k + 1)],
                                start=False, stop=(k == 3 and j == 7))
                hbf = hbf_n
                hm1_s = tp.tile([128, 8 * BD], BF16, tag="dhm1")
                nc.vector.tensor_scalar_max(hm1_s[:], m1[:, 0:8 * BD], 0.0)

                # ---- M2 ----
                for k in range(8):
                    for j in range(4):
                        nc.tensor.matmul(m2d[:, BD * j:BD * (j + 1)],
                                         dm2_b[k][:, 128 * j:128 * (j + 1)],
                                         hm1_s[:, BD * k:BD * (k + 1)],
                                         start=False,
                                         stop=(k == 7 and j == 3))
                hm2_s = tp.tile([128, 4 * BD], BF16, tag="dhm2")
                nc.vector.tensor_scalar_max(hm2_s[:], m2d[:, 0:4 * BD], 0.0)

                # bias pre-issue for t+1 (fills the out-matmul wait)
                if not lastd:
                    m1_n = pD.tile([128, 512], F32, tag="D")
                    dec_m1bias(m1_n)
                    m2_n = pTR.tile([128, 512], F32, tag="TR")
                    dec_m2bias(m2_n)

                # ---- output head: y [64, 32]; do_b rides as a K=1 matmul --
                yb = pC.tile([128, 512], F32, tag="C")
                nc.tensor.matmul(yb[0:NY, 0:BD], dob_r[0:1, 0:NY],
                                 ones_b[0:1, 0:BD], start=True, stop=False)
                for k in range(4):
                    nc.tensor.matmul(yb[0:NY, 0:BD],
                                     dow_b[k][:, 0:NY],
                                     hm2_s[:, BD * k:BD * (k + 1)],
                                     start=False, stop=(k == 3))
                if not lastd:
                    # critical path: feed y back (bf16) before the f32 copy
                    nc.vector.tensor_copy(ypc[0:NY, :], yb[0:NY, 0:BD])
                y_f = tp.tile([NY, BD], F32, tag="dy")
                nc.scalar.copy(y_f[:], yb[0:NY, 0:BD])
                nc.sync.dma_start(d_out[NY * t:NY * (t + 1), :], y_f[:])
                if not lastd:
                    # pre-issue next step's h-side matmuls
                    rz_n = pA.tile([128, 512], F32, tag="A")
                    hgn_n = pB.tile([128, 512], F32, tag="B")
                    dec_whh(rz_n, hgn_n, hbf)
                    rz, hgn, m1, m2d = rz_n, hgn_n, m1_n, m2_n

    nc.compile()
    return nc


# ---------------------------------------------------------------------------
# Host-side sharding
# ---------------------------------------------------------------------------

def shard_inputs(inp, et=100, hor=60):
    f32 = np.float32

    def bf(a):
        return np.ascontiguousarray(np.asarray(a, f32).astype(BF))

    x, y = np.asarray(inp["x"], f32), np.asarray(inp["y"], f32)
    chains = [("xf", False, x), ("xb", True, x),
              ("ef", False, y), ("eb", True, y)]
    in_maps = []
    shared = {}

    def wih_aug(pre):
        wih = np.asarray(inp[pre + "_Wih"], f32)
        bih = np.asarray(inp[pre + "_bih"], f32)
        bhh = np.asarray(inp[pre + "_bhh"], f32)
        aug = np.zeros((66, G), f32)
        aug[0:64, :] = wih.T
        bias = bih.copy()
        bias[0:2 * H] += bhh[0:2 * H]
        aug[64, :] = bias
        aug[65, H:2 * H] = BIG
        return bf(aug)

    d_Wih = np.asarray(inp["d_Wih"], f32)
    d_bih = np.asarray(inp["d_bih"], f32)
    d_bhh = np.asarray(inp["d_bhh"], f32)
    dc_b = d_bih.copy()
    dc_b[0:2 * H] += d_bhh[0:2 * H]

    shared["em_w1t"] = bf(np.asarray(inp["em_W1"], f32).T)
    shared["em_w2t"] = bf(np.asarray(inp["em_W2"], f32).T)
    shared["eo_wt"] = bf(np.asarray(inp["eo_W"], f32).T)
    shared["dc_wt"] = bf(d_Wih[:, 0:2 * H].T)
    midb = np.concatenate([dc_b, np.asarray(inp["em_b1"], f32),
                           np.asarray(inp["em_b2"], f32),
                           np.asarray(inp["eo_b"], f32)])[None, :]
    shared["mid_bias"] = bf(midb)
    shared["dwy_t"] = bf(d_Wih[:, 2 * H:].T)
    shared["dwhh_t"] = bf(np.asarray(inp["d_Whh"], f32).T)
    shared["dbhhn_row"] = bf(d_bhh[None, 2 * H:])
    shared["dm_w1t"] = bf(np.asarray(inp["dm_W1"], f32).T)
    shared["dm_b1row"] = bf(np.asarray(inp["dm_b1"], f32)[None, :])
    shared["dm_w2t"] = bf(np.asarray(inp["dm_W2"], f32).T)
    shared["dm_b2row"] = bf(np.asarray(inp["dm_b2"], f32)[None, :])
    shared["do_wt"] = bf(np.asarray(inp["do_W"], f32).T)
    shared["do_brow"] = bf(np.asarray(inp["do_b"], f32)[None, :])

    for j in range(NCORE):
        chain, half = j // 2, j % 2
        pre, rev, seq = chains[chain]
        T = seq.shape[1]
        s = seq[128 * half:128 * (half + 1)]          # [128, T, 64]
        xin = np.zeros((66, et, BE), f32)
        xin[64, :, :] = 1.0
        pad = et - T
        if pad:
            xin[65, 0:pad, :] = 1.0
        order = np.arange(T)[::-1] if rev else np.arange(T)
        xin[0:64, pad:, :] = s[:, order, :].transpose(2, 1, 0)
        m = dict(shared)
        m["xin"] = bf(xin.reshape(66, et * BE))
        m["wih_aug"] = wih_aug(pre)
        m["whh_t"] = bf(np.asarray(inp[pre + "_Whh"], f32).T)
        m["bhhn_row"] = bf(np.asarray(inp[pre + "_bhh"], f32)[None, 2 * H:])
        xl = np.concatenate([x[16 * j:16 * j + 16, -1, :],
                             x[128 + 16 * j:128 + 16 * j + 16, -1, :]])
        m["xlast_t"] = bf(xl.T)
        in_maps.append(m)
    return in_maps


def unshard(results, hor=60):
    out = np.zeros((B, hor, NY), np.float32)
    for j in range(NCORE):
        o = results[j]["out"].reshape(hor, NY, BD).transpose(2, 0, 1)
        out[16 * j:16 * j + 16] = o[0:16]
        out[128 + 16 * j:128 + 16 * j + 16] = o[16:32]
    return out


_NC = None


def kernel(**inputs):
    global _NC
    from concourse.bass_utils import run_bass_kernel_spmd
    if _NC is None:
        _NC = build_nc()
    in_maps = shard_inputs(inputs)
    res = run_bass_kernel_spmd(_NC, in_maps, core_ids=list(range(NCORE)))
    return unshard(res.results)
